# revision 2
# baseline (speedup 1.0000x reference)
"""Causal multi-head attention (B=4, T=2048, D=1024, H=16) on 8 Trainium2 cores.

Sharding (data + tensor parallel): core c handles batch b = c//2 and head-group
g = c%2 (8 of the 16 heads). Wq/Wk/Wv are column-sharded by head, Wp is
row-sharded; the two per-batch partial outputs are summed on the host (this
replaces the device all-reduce — the host-side sum is the unshard step).

v2 restructure vs the 293us baseline (the cost model charges matmuls by output
free-dim rows only; PE re-ramps to half speed after every idle gap):
  - all matmul operands bf16 (same 1 cyc/row as f32r, but exact causal
    narrowing is allowed — no >=256-wide f32r constraint — and DMA/SBUF halve)
  - scores St [kpos, q] per k-tile pair, exactly causal-narrowed
  - attention*V flipped: stationary = E-tile [128 kpos, 128 q] (slice of the
    already-transposed e2), moving = V-aug [128 kpos, 65] -> ctx^T [128 q, 65]
    costs 65 rows/tile instead of 128; the softmax denominator rides along as
    column 64 via an all-ones column in V-aug
  - normalization on DVE: per-partition reciprocal + tensor_scalar multiply
    while copying ctx^T out of PSUM (q is the partition axis there), then PE
    transposes back to hd-major [64, 512] per head
  - software-pipelined emission: every "beat" issues one score pair + exp,
    then runs one delayed thunk (the previous pair's ctx matmuls, or head-tail
    work), then filler matmuls (next-block QKV paced evenly; output
    projections deferred into block 3 where exp latency would otherwise
    starve the PE)
"""
import collections
import numpy as np

T = 2048
D = 1024
B = 4
H = 16
HL = 8            # heads per core
NP = 4            # head pairs per core
QB = 512          # q-block width
NQB = T // QB     # 4 q-blocks
NKT = T // 128    # 16 k-tiles

_COMPILED = None


# --------------------------------------------------------------------------
# bass kernel build
# --------------------------------------------------------------------------
def _build_bass():
    import concourse.bass as bass
    import concourse.mybir as mybir
    from concourse.tile import TileContext

    F32 = mybir.dt.float32
    BF16 = mybir.dt.bfloat16
    Act = mybir.ActivationFunctionType
    Alu = mybir.AluOpType

    nc = bass.Bass()
    xt = nc.dram_tensor("xt", [D, T], BF16, kind="ExternalInput")
    wq = nc.dram_tensor("wq", [D, 512], BF16, kind="ExternalInput")
    wk = nc.dram_tensor("wk", [D, 512], BF16, kind="ExternalInput")
    wv = nc.dram_tensor("wv", [D, 512], BF16, kind="ExternalInput")
    wp = nc.dram_tensor("wp", [512, D], BF16, kind="ExternalInput")
    mask1 = nc.dram_tensor("mask1", [128, 128], BF16, kind="ExternalInput")
    ident = nc.dram_tensor("ident", [128, 128], BF16, kind="ExternalInput")
    vones = nc.dram_tensor("vones", [128, NKT * HL], BF16, kind="ExternalInput")
    outt = nc.dram_tensor("outt", [D, T], F32, kind="ExternalOutput")

    with TileContext(nc) as tc, nc.allow_low_precision(reason="bf16 pipeline"):
        with tc.tile_pool(name="wts", bufs=1) as wts, \
             tc.tile_pool(name="xp", bufs=2) as xp, \
             tc.tile_pool(name="big", bufs=1) as big, \
             tc.tile_pool(name="qtp", bufs=8) as qtp, \
             tc.tile_pool(name="ep", bufs=4) as ep, \
             tc.tile_pool(name="ctsp", bufs=2) as ctsp, \
             tc.tile_pool(name="rcq", bufs=2) as rcq, \
             tc.tile_pool(name="cxp", bufs=16) as cxp, \
             tc.tile_pool(name="sm", bufs=1) as sm, \
             tc.tile_pool(name="osb", bufs=4) as osb, \
             tc.tile_pool(name="pp3", bufs=8) as pp3, \
             tc.tile_pool(name="pst", bufs=2, space="PSUM") as pst, \
             tc.tile_pool(name="pctx", bufs=1, space="PSUM") as pctx, \
             tc.tile_pool(name="ptt", bufs=1, space="PSUM") as ptt, \
             tc.tile_pool(name="paux", bufs=2, space="PSUM") as paux:

            # ---------------- weights/constants ----------------
            wq_t = wts.tile([128, 8, 512], BF16, tag="wq")
            wk_t = wts.tile([128, 8, 512], BF16, tag="wk")
            wv_t = wts.tile([128, 8, 512], BF16, tag="wv")
            wp_t = wts.tile([128, 4, 1024], BF16, tag="wp")
            wqr = wq[:].rearrange("(n p) m -> p n m", p=128)
            wkr = wk[:].rearrange("(n p) m -> p n m", p=128)
            wvr = wv[:].rearrange("(n p) m -> p n m", p=128)
            x_tiles = {}

            def load_x(tb):
                # 2-slice chunks on the SP queue: 4 issues instead of 8
                x_t = xp.tile([128, 8, 512], BF16, tag="x", name=f"x_t{tb}")
                xr = xt[:, QB * tb:QB * (tb + 1)].rearrange("(n p) m -> p n m", p=128)
                for c in range(4):
                    nc.sync.dma_start(x_t[:, 2 * c:2 * c + 2, :], xr[:, 2 * c:2 * c + 2, :])
                x_tiles[tb] = x_t

            # DMA issue is serialized per DGE queue (~600ns each), so spread
            # the prologue loads across the two HWDGE queues (SP + Act):
            #   SP:  wq, x0 (critical path for the first Q chains), then x1..
            #   Act: wk, wv, wp, constants (needed a few us later)
            x_t0 = xp.tile([128, 8, 512], BF16, tag="x", name="x_t0")
            xr0 = xt[:, 0:QB].rearrange("(n p) m -> p n m", p=128)
            for c in range(4):
                nc.sync.dma_start(wq_t[:, 2 * c:2 * c + 2, :], wqr[:, 2 * c:2 * c + 2, :])
                nc.sync.dma_start(x_t0[:, 2 * c:2 * c + 2, :], xr0[:, 2 * c:2 * c + 2, :])
            x_tiles[0] = x_t0
            for c in range(4):
                nc.scalar.dma_start(wk_t[:, 2 * c:2 * c + 2, :], wkr[:, 2 * c:2 * c + 2, :])
            for c in range(4):
                nc.scalar.dma_start(wv_t[:, 2 * c:2 * c + 2, :], wvr[:, 2 * c:2 * c + 2, :])
            m1 = sm.tile([128, 128], BF16, tag="m1")
            nc.scalar.dma_start(m1[:], mask1[:])
            idn = sm.tile([128, 128], BF16, tag="idn")
            nc.scalar.dma_start(idn[:], ident[:])

            kt_t = big.tile([128, 4, T], BF16, tag="kt")
            va_t = big.tile([128, NKT, HL, 65], BF16, tag="va")
            nc.scalar.dma_start(
                va_t[:, :, :, 64:65].squeeze(3),
                vones[:].rearrange("p (n h) -> p n h", n=NKT))
            wpr = wp[:].rearrange("(n p) m -> p n m", p=128)
            for c in range(2):
                nc.scalar.dma_start(wp_t[:, 2 * c:2 * c + 2, :], wpr[:, 2 * c:2 * c + 2, :])

            q_tiles = {}     # (j, p) -> sbuf tile [128, 512]
            ctx_tiles = {}   # (j, p) -> sbuf tile [128, 512] (normalized pair)

            # ---------------- thunk plumbing ----------------
            # a thunk is (rows, fn); rows = PE rows it will emit (for pacing)
            pending = collections.deque()   # attention work delayed >=1 beat
            qkv_q = collections.deque()     # next-block QKV (must finish)
            proj_q = collections.deque()    # deferred output projections
            late_q = collections.deque()    # deferred head tails (blocks 0-1)

            def gen_q(j):
                x_t = x_tiles[j]
                out = []
                for p in range(4):
                    box = {}
                    for kk in range(8):
                        def f(p=p, kk=kk, box=box, x_t=x_t, j=j):
                            if kk == 0:
                                box["ps"] = paux.tile([128, 512], F32, tag="aux",
                                                      name=f"psq{j}_{p}")
                                box["q"] = qtp.tile([128, 512], BF16, tag="qt",
                                                    name=f"qt{j}_{p}")
                                q_tiles[(j, p)] = box["q"]
                            nc.tensor.matmul(box["ps"][:],
                                             wq_t[:, kk, 128 * p:128 * (p + 1)],
                                             x_t[:, kk, :], start=(kk == 0), stop=(kk == 7))
                            if kk == 7:
                                nc.vector.tensor_copy(box["q"][:], box["ps"][:])
                        out.append((512, f))
                return out

            def gen_kv(j):
                x_t = x_tiles[j]
                out = []
                for p in range(4):
                    box = {}
                    for kk in range(8):
                        def f(p=p, kk=kk, box=box, x_t=x_t, j=j):
                            if kk == 0:
                                box["ps"] = paux.tile([128, 512], F32, tag="aux",
                                                      name=f"psk{j}_{p}")
                            nc.tensor.matmul(box["ps"][:],
                                             wk_t[:, kk, 128 * p:128 * (p + 1)],
                                             x_t[:, kk, :], start=(kk == 0), stop=(kk == 7))
                            if kk == 7:
                                nc.vector.tensor_copy(
                                    kt_t[:, p, QB * j:QB * (j + 1)], box["ps"][:])
                        out.append((512, f))
                for tt in range(4):
                    box = {}
                    for kk in range(8):
                        def f(tt=tt, kk=kk, box=box, x_t=x_t, j=j):
                            if kk == 0:
                                box["ps"] = paux.tile([128, 512], F32, tag="aux",
                                                      name=f"psv{j}_{tt}")
                            nc.tensor.matmul(box["ps"][:],
                                             x_t[:, kk, 128 * tt:128 * (tt + 1)],
                                             wv_t[:, kk, :], start=(kk == 0), stop=(kk == 7))
                            if kk == 7:
                                nc.vector.tensor_copy(
                                    va_t[:, 4 * j + tt, :, 0:64],
                                    box["ps"][:].rearrange("p (h d) -> p h d", h=HL))
                        out.append((512, f))
                return out

            def proj_steps(j, m, ps, use_pst, dma_eng, box):
                """Thunks for output-projection chain m of block j over the
                p-range `ps`."""
                out = []
                for p in ps:
                    def f(m=m, p=p, box=box, j=j, use_pst=use_pst, dma_eng=dma_eng):
                        if "pf" not in box:
                            if use_pst:
                                t = pst.tile([128, 1024], F32, tag="st",
                                             name=f"pf{j}_{m}")
                                box["pf"] = t[:, 0:512]
                            else:
                                box["pf"] = paux.tile([128, 512], F32, tag="aux",
                                                      name=f"pf{j}_{m}")[:]
                        nc.tensor.matmul(box["pf"],
                                         wp_t[:, p, 128 * m:128 * (m + 1)],
                                         ctx_tiles[(j, p)][:], start=(p == 0), stop=(p == 3))
                        if p == 3:
                            ob = osb.tile([128, 512], F32, tag="ob", name=f"ob{j}_{m}")
                            if use_pst:
                                # split the epilogue copies across DVE and the
                                # (idle) ACT engine — Copy lives in the same
                                # act-table set as Exp, and gpsimd can't read
                                # PSUM
                                nc.scalar.copy(ob[:], box["pf"])
                            else:
                                nc.vector.tensor_copy(ob[:], box["pf"])
                            dma_eng.dma_start(
                                outt[128 * m:128 * (m + 1), QB * j:QB * (j + 1)], ob[:])
                    out.append((512, f))
                return out

            part3 = {}

            def gen_proj3_stage_a():
                """Last block's projections, p=0..2: run during the final two
                heads' beats on transient paux slots; partials park in SBUF
                (bf16) so no PSUM is held across the last pair's completion."""
                out = []
                for m in range(8):
                    box = {}
                    for p in range(3):
                        def f(m=m, p=p, box=box):
                            j = NQB - 1
                            if p == 0:
                                box["pf"] = paux.tile([128, 512], F32, tag="aux",
                                                      name=f"pf3a_{m}")[:]
                            nc.tensor.matmul(box["pf"],
                                             wp_t[:, p, 128 * m:128 * (m + 1)],
                                             ctx_tiles[(j, p)][:],
                                             start=(p == 0), stop=(p == 2))
                            if p == 2:
                                pt = pp3.tile([128, 512], BF16, tag="pp",
                                              name=f"part3_{m}")
                                part3[m] = pt
                                nc.vector.tensor_copy(pt[:], box["pf"])
                        out.append((512, f))
                return out

            def gen_proj3_stage_b():
                """Epilogue: re-inject the parked partial via an identity
                matmul, add the p=3 term, write out."""
                out = []
                for m in range(8):
                    def f(m=m):
                        j = NQB - 1
                        if m % 2 == 1:
                            t = pst.tile([128, 1024], F32, tag="st", name=f"pf3b_{m}")
                            pf = t[:, 0:512]
                        else:
                            pf = paux.tile([128, 512], F32, tag="aux",
                                           name=f"pf3b_{m}")[:]
                        nc.tensor.matmul(pf, idn[:], part3[m][:],
                                         start=True, stop=False)
                        nc.tensor.matmul(pf, wp_t[:, 3, 128 * m:128 * (m + 1)],
                                         ctx_tiles[(j, 3)][:], start=False, stop=True)
                        ob = osb.tile([128, 512], F32, tag="ob", name=f"ob3_{m}")
                        if m % 2 == 1:
                            nc.scalar.copy(ob[:], pf)
                        else:
                            nc.vector.tensor_copy(ob[:], pf)
                        dma_eng = nc.scalar if m % 2 == 1 else nc.sync
                        dma_eng.dma_start(
                            outt[128 * m:128 * (m + 1), QB * j:QB * (j + 1)], ob[:])
                    out.append((1024, f))
                return out

            def gen_proj(j):
                last = (j == NQB - 1)
                out = []
                for m in range(8):
                    out.extend(proj_steps(j, m, range(4), last and m % 2 == 1,
                                          nc.scalar if last and m % 2 == 1 else nc.sync,
                                          {}))
                return out

            # ---------------- per-beat emission ----------------
            ROW_TARGET = 2500   # ~1038ns of exp per beat, in PE rows

            state = {"qkv_done": 0, "qkv_total": 0, "beat": 0, "beats_total": 1,
                     "allow_proj": False, "proj_floor": 0,
                     "proj_done": 0, "proj_total": 0, "prefill_proj": False}

            def pop_one_filler():
                """Emit one independent filler matmul; returns its rows or
                None when nothing is available. Both queues are paced evenly
                over the block's beats so late beats (where exp latency
                dominates) still have cover."""
                tgt = -(-state["qkv_total"] * state["beat"] // state["beats_total"])
                if state["qkv_done"] < tgt and qkv_q:
                    r, f = qkv_q.popleft()
                    f()
                    state["qkv_done"] += 1
                    return r
                ptgt = -(-state["proj_total"] * state["beat"] // state["beats_total"])
                if state["prefill_proj"] and state["proj_done"] < ptgt:
                    if late_q:
                        r, f = late_q.popleft()
                        f()
                        state["proj_done"] += 1
                        return r
                    if len(proj_q) > state["proj_floor"]:
                        r, f = proj_q.popleft()
                        f()
                        state["proj_done"] += 1
                        return r
                return None

            def run_beat_tail(rows):
                # a few independent fillers ahead of the
                # dependency-stalled attention thunk
                for _ in range(3):
                    r = pop_one_filler()
                    if r is None:
                        break
                    rows += r
                # one delayed attention thunk
                if pending:
                    r, f = pending.popleft()
                    f()
                    rows += r
                # paced QKV
                tgt = -(-state["qkv_total"] * state["beat"] // state["beats_total"])
                while state["qkv_done"] < tgt and qkv_q:
                    r, f = qkv_q.popleft()
                    f()
                    state["qkv_done"] += 1
                    rows += r
                # top up with deferred tails/projections (paced)
                while rows < ROW_TARGET:
                    ptgt = -(-state["proj_total"] * state["beat"]
                             // state["beats_total"])
                    if not (state["allow_proj"] and state["proj_done"] < ptgt):
                        break
                    if late_q:
                        r, f = late_q.popleft()
                    elif len(proj_q) > state["proj_floor"]:
                        r, f = proj_q.popleft()
                    else:
                        break
                    f()
                    state["proj_done"] += 1
                    rows += r
                # bound the delayed-thunk backlog (shallow blocks append
                # faster than one-per-beat pops)
                while len(pending) > 5:
                    pop_one_filler()
                    r, f = pending.popleft()
                    f()

            def drain_pending(keep=2):
                while len(pending) > keep:
                    r, f = pending.popleft()
                    f()
                    pop_one_filler()

            tt_tiles = {}

            def attn_pair_beat(j, h, ip):
                """Emit one beat: scores pair + exp + mask, then delayed work
                and fillers; queue this pair's ctx matmuls."""
                p, s = divmod(h, 2)
                q_tile = q_tiles[(j, p)]
                hs = slice(64 * s, 64 * s + 64)
                tp = (64 * s, 0)
                i0, i1 = 2 * ip, 2 * ip + 1
                o0, o1 = i0 - 4 * j, i1 - 4 * j
                cs0 = max(0, 128 * o0)
                cs1 = max(0, 128 * o1)
                if ip == 2 * (j + 1) - 1:
                    # first (diagonal-most) pair of the head: allocate the
                    # ctx^T accumulator bank
                    tt_tiles[("c", j, h)] = pctx.tile([128, 260], F32, tag="ctxa",
                                                      name=f"ctxa{j}_{h}")
                st = pst.tile([128, 1024], F32, tag="st", name=f"st{j}_{h}_{ip}")
                nc.tensor.matmul(st[:, cs0:512],
                                 kt_t[hs, p, 128 * i0:128 * (i0 + 1)],
                                 q_tile[hs, cs0:512],
                                 start=True, stop=True, tile_position=tp)
                nc.tensor.matmul(st[:, 512 + cs1:1024],
                                 kt_t[hs, p, 128 * i1:128 * (i1 + 1)],
                                 q_tile[hs, cs1:512],
                                 start=True, stop=True, tile_position=tp)
                e2 = ep.tile([128, 1024], BF16, tag="e", name=f"e{j}_{h}_{ip}")
                if o1 < 0:
                    nc.scalar.activation(e2[:], st[:], Act.Exp)
                else:
                    nc.scalar.activation(e2[:, cs0:512], st[:, cs0:512], Act.Exp)
                    nc.scalar.activation(e2[:, 512 + cs1:1024],
                                         st[:, 512 + cs1:1024], Act.Exp)
                    for (oo, base) in ((o0, 0), (o1, 512)):
                        if 0 <= oo:
                            z = slice(base + 128 * oo, base + 128 * (oo + 1))
                            nc.gpsimd.tensor_tensor(e2[:, z], e2[:, z], m1[:],
                                                    op=Alu.mult)
                rows = (512 - cs0) + (512 - cs1)
                nctx = (4 - max(0, o1)) + (4 - max(0, o0))

                def ctx_f(j=j, h=h, ip=ip, e2=e2, i0=i0, i1=i1, o0=o0, o1=o1):
                    # the whole [128, 260] accumulator is ONE hardware
                    # accumulation group: start=True clears the full PSUM
                    # bank, so only the head's first matmul (diag ktile,
                    # subtile 3) starts; every other subtile accumulates
                    # onto the cleared bank
                    ctxa = tt_tiles[("c", j, h)]
                    for (i, base, o) in ((i1, 512, o1), (i0, 0, o0)):
                        for qq in range(max(0, o), 4):
                            nc.tensor.matmul(
                                ctxa[:, 65 * qq:65 * (qq + 1)],
                                e2[:, base + 128 * qq:base + 128 * (qq + 1)],
                                va_t[:, i, h, :],
                                start=(i == 4 * j + 3 and qq == 3),
                                stop=(i == 0 and qq == 3),
                                skip_group_check=True)
                return rows, (65 * nctx, ctx_f)

            def tail_a(j, h):
                def f(j=j, h=h):
                    ctxa = tt_tiles[("c", j, h)]
                    # ONE plain copy out of PSUM, so the single ctx^T
                    # accumulator bank frees for the next head after ~400ns;
                    # reciprocal + scaling run from the SBUF copy (2x DVE)
                    raw = ctsp.tile([128, 260], BF16, tag="raw", bufs=2,
                                    name=f"raw{j}_{h}")
                    nc.vector.tensor_copy(raw[:], ctxa[:])
                    rc = rcq.tile([128, 4], F32, tag="rc", name=f"rc{j}_{h}")
                    # blocks 0-1 defer their transposes into blocks 2-3, so
                    # up to 16 cts tiles stay live
                    cts = ctsp.tile([128, 256], BF16, tag="cts", bufs=18,
                                    name=f"cts{j}_{h}")
                    tt_tiles[("s", j, h)] = cts
                    raw4 = raw[:].rearrange("p (q c) -> p q c", q=4)
                    nc.vector.reciprocal(rc[:], raw4[:, :, 64:65].squeeze(2))
                    for qq in range(4):
                        nc.vector.tensor_scalar_mul(
                            cts[:, 64 * qq:64 * (qq + 1)],
                            raw[:, 65 * qq:65 * qq + 64],
                            rc[:, qq:qq + 1])
                return (0, f)

            def tail_b(j, h):
                def f(j=j, h=h):
                    p, s = divmod(h, 2)
                    if s == 0:
                        tt_tiles[("t", j, p)] = ptt.tile([128, 512], BF16, tag="tt",
                                                      name=f"tt{j}_{p}")
                    tt = tt_tiles[("t", j, p)]
                    cts = tt_tiles[("s", j, h)]
                    for qq in range(4):
                        nc.tensor.transpose(
                            tt[64 * s:64 * s + 64, 128 * qq:128 * (qq + 1)],
                            cts[:, 64 * qq:64 * (qq + 1)], idn[:])
                return (512, f)

            def tail_c(j, p):
                def f(j=j, p=p):
                    ctx_pair = cxp.tile([128, 512], BF16, tag="ctx", name=f"ctx{j}_{p}")
                    ctx_tiles[(j, p)] = ctx_pair
                    nc.vector.tensor_copy(ctx_pair[:], tt_tiles[("t", j, p)][:])
                    if p == 3:
                        proj_q.extend(gen_proj(j))
                return (0, f)

            # ---------------- prologue: QKV for block 0 ----------------
            # Q runs kk-major across 4 interleaved accumulation chains (2
            # paux slots + 2 borrowed score slots) so each arriving x-chunk
            # DMA feeds 4 matmuls — the chain-major order would stall on the
            # serialized x0 chunk issues
            psq, qts = [], []
            for p in range(4):
                if p < 2:
                    ps = paux.tile([128, 512], F32, tag="aux", name=f"psq0_{p}")[:]
                else:
                    ps = pst.tile([128, 1024], F32, tag="st", name=f"psq0_{p}")[:, 0:512]
                psq.append(ps)
                qt = qtp.tile([128, 512], BF16, tag="qt", name=f"qt0_{p}")
                q_tiles[(0, p)] = qt
                qts.append(qt)
            for kk in range(8):
                for p in range(4):
                    nc.tensor.matmul(psq[p], wq_t[:, kk, 128 * p:128 * (p + 1)],
                                     x_t0[:, kk, :], start=(kk == 0), stop=(kk == 7))
            for p in range(4):
                nc.vector.tensor_copy(qts[p][:], psq[p])
            for r, f in gen_kv(0):
                f()

            # ---------------- main loop ----------------
            for j in range(NQB):
                qkv_q.clear()
                if j + 1 < NQB:
                    load_x(j + 1)
                    qkv_q.extend(gen_q(j + 1))
                    qkv_q.extend(gen_kv(j + 1))
                npair = 2 * (j + 1)
                state["qkv_total"] = len(qkv_q)
                state["qkv_done"] = 0
                state["beats_total"] = 8 * npair
                state["beat"] = 0
                state["allow_proj"] = (j >= 2)
                state["prefill_proj"] = (j == NQB - 1)
                state["proj_floor"] = 8
                state["proj_done"] = 0
                state["proj_total"] = len(proj_q) + len(late_q)
                for h in range(HL):
                    first = True
                    for ip in reversed(range(npair)):
                        state["beat"] += 1
                        rows, ctx_thunk = attn_pair_beat(j, h, ip)
                        if first:
                            # head boundary: clear old ctx thunks behind the
                            # freshly issued scores+exp, but leave the
                            # previous head's tail chain (A/B/C) to spread
                            # over the next beats — B stalls on A's DVE work
                            # if popped in the same beat
                            drain_pending(keep=2 if len(pending) <= 4 else 3)
                            first = False
                        run_beat_tail(rows)
                        pending.append(ctx_thunk)
                    pending.append(tail_a(j, h))
                    if h % 2 == 1:
                        # transpose + assemble as one atomic per-pair entry
                        # (the single tt PSUM slot must not interleave two
                        # pairs); blocks 0-1 defer theirs into blocks 2-3,
                        # where exp latency otherwise starves the PE
                        ra, fa = tail_b(j, h - 1)
                        rb, fb = tail_b(j, h)
                        rc_, fc = tail_c(j, h // 2)

                        def bc(fa=fa, fb=fb, fc=fc):
                            fa()
                            fb()
                            fc()
                        entry = (ra + rb + rc_, bc)
                        if j < 2:
                            late_q.append(entry)
                        else:
                            pending.append(entry)
                # block end: QKV for next block must be complete
                while qkv_q:
                    r, f = qkv_q.popleft()
                    f()

            # ---------------- epilogue ----------------
            state["allow_proj"] = True
            state["prefill_proj"] = True
            state["proj_floor"] = 0
            drain_pending(keep=0)
            while late_q:
                r, f = late_q.popleft()
                f()
            while proj_q:
                r, f = proj_q.popleft()
                f()
    return nc


def _split_waits(nc, limit=1):
    """This walrus build accepts only one sync wait per TPB_CTRL instruction;
    move excess waits onto preceding same-engine NOPs."""
    import concourse.mybir as mybir
    for f in nc.m.functions:
        for bb in f.blocks:
            new_insts = []
            for inst in bb.instructions:
                si = inst.sync_info
                if si is not None and si.on_wait and len(si.on_wait) > limit:
                    waits = list(si.on_wait)
                    k = 0
                    while len(waits) - k > limit:
                        chunk = waits[k:k + limit]
                        k += limit
                        nop = mybir.InstNoOp(name=f"{inst.name}_ws{k}")
                        nop.engine = inst.engine
                        nop.sync_info = mybir.SyncInfo(on_wait=chunk, on_update=[])
                        new_insts.append(nop)
                    si.on_wait = waits[k:]
                new_insts.append(inst)
            bb.instructions = new_insts


# --------------------------------------------------------------------------
# compile + SPMD execution via PJRT (axon) — jit once, reuse
# --------------------------------------------------------------------------
class _Compiled:
    def __init__(self, n_cores=8):
        import jax
        from jax.sharding import Mesh, PartitionSpec
        from jax.experimental.shard_map import shard_map
        import concourse.mybir as mybir
        from concourse.bass2jax import (_bass_exec_p, install_neuronx_cc_hook,
                                        partition_id_tensor)

        nc = _build_bass()
        _split_waits(nc)
        install_neuronx_cc_hook()
        partition_name = nc.partition_id_tensor.name if nc.partition_id_tensor else None
        in_names, out_names, out_avals, zero_outs = [], [], [], []
        for alloc in nc.m.functions[0].allocations:
            if not isinstance(alloc, mybir.MemoryLocationSet):
                continue
            name = alloc.memorylocations[0].name
            if alloc.kind == "ExternalInput":
                if name != partition_name:
                    in_names.append(name)
            elif alloc.kind == "ExternalOutput":
                shape = tuple(alloc.tensor_shape)
                dtype = mybir.dt.np(alloc.dtype)
                out_names.append(name)
                out_avals.append(jax.core.ShapedArray(shape, dtype))
                zero_outs.append(np.zeros(shape, dtype))
        n_params = len(in_names)
        all_in_names = list(in_names) + list(out_names)
        if partition_name is not None:
            all_in_names.append(partition_name)

        def _body(*args):
            operands = list(args)
            if partition_name is not None:
                operands.append(partition_id_tensor())
            outs = _bass_exec_p.bind(
                *operands,
                out_avals=tuple(out_avals),
                in_names=tuple(all_in_names),
                out_names=tuple(out_names),
                lowering_input_output_aliases=(),
                sim_require_finite=True,
                sim_require_nnan=True,
                nc=nc,
            )
            return tuple(outs)

        devices = jax.devices()[:n_cores]
        assert len(devices) >= n_cores, f"need {n_cores} cores, have {len(devices)}"
        self.n_cores = n_cores
        self.in_names, self.out_names = in_names, out_names
        self.out_avals, self.zero_outs = out_avals, zero_outs
        mesh = Mesh(np.asarray(devices[:n_cores]), ("core",))
        in_specs = (PartitionSpec("core"),) * (n_params + len(out_names))
        out_specs = (PartitionSpec("core"),) * len(out_names)
        self.fn = jax.jit(
            shard_map(_body, mesh=mesh, in_specs=in_specs,
                      out_specs=out_specs, check_rep=False),
            keep_unused=True)

    def run(self, in_maps):
        import jax
        args = []
        for name in self.in_names:
            args.append(np.concatenate([np.asarray(m[name]) for m in in_maps], axis=0))
        for z in self.zero_outs:
            args.append(np.zeros((self.n_cores * z.shape[0], *z.shape[1:]), z.dtype))
        outs = self.fn(*args)
        jax.block_until_ready(outs)
        res = []
        for c in range(self.n_cores):
            d = {}
            for i, name in enumerate(self.out_names):
                a = np.asarray(outs[i]).reshape(self.n_cores, *self.out_avals[i].shape)[c]
                d[name] = a
            res.append(d)
        return res


# --------------------------------------------------------------------------
# host-side shard / unshard
# --------------------------------------------------------------------------
def _bf16(a):
    import ml_dtypes
    return np.ascontiguousarray(a).astype(ml_dtypes.bfloat16)


def _make_core_inputs(x, Wq, Wk, Wv, Wp, core):
    g = core % 2
    b = core // 2
    rows = slice(512 * g, 512 * (g + 1))
    kl = np.arange(128)
    return {
        "xt": _bf16(x[b].T),
        # fold the 1/sqrt(head_dim) score scale into Wq
        "wq": _bf16(Wq[rows, :].T * 0.125),
        "wk": _bf16(Wk[rows, :].T),
        "wv": _bf16(Wv[rows, :].T),
        "wp": _bf16(Wp[:, rows].T),
        "mask1": _bf16((kl[:, None] <= kl[None, :]).astype(np.float32)),
        "ident": _bf16(np.eye(128, dtype=np.float32)),
        "vones": _bf16(np.ones((128, NKT * HL), np.float32)),
    }


def kernel(x, Wq, Wk, Wv, Wp):
    """Full-input / full-output causal MHA. x: (4, 2048, 1024) fp32;
    Wq/Wk/Wv/Wp: (1024, 1024) fp32. Returns (4, 2048, 1024) fp32."""
    global _COMPILED
    x = np.asarray(x, dtype=np.float32)
    Wq = np.asarray(Wq, dtype=np.float32)
    Wk = np.asarray(Wk, dtype=np.float32)
    Wv = np.asarray(Wv, dtype=np.float32)
    Wp = np.asarray(Wp, dtype=np.float32)
    assert x.shape == (B, T, D), x.shape

    if _COMPILED is None:
        _COMPILED = _Compiled(8)
    in_maps = [_make_core_inputs(x, Wq, Wk, Wv, Wp, c) for c in range(8)]
    results = _COMPILED.run(in_maps)

    out = np.empty((B, T, D), np.float32)
    for b in range(B):
        acc = results[2 * b]["outt"] + results[2 * b + 1]["outt"]
        out[b] = acc.T
    return out


# revision 4
# speedup vs baseline: 1.0044x; 1.0044x over previous
"""Causal multi-head attention (B=4, T=2048, D=1024, H=16) on 8 Trainium2 cores.

Sharding (data + tensor parallel): core c handles batch b = c//2 and head-group
g = c%2 (8 of the 16 heads). Wq/Wk/Wv are column-sharded by head, Wp is
row-sharded; the two per-batch partial outputs are summed on the host (this
replaces the device all-reduce — the host-side sum is the unshard step).

v2 restructure vs the 293us baseline (the cost model charges matmuls by output
free-dim rows only; PE re-ramps to half speed after every idle gap):
  - all matmul operands bf16 (same 1 cyc/row as f32r, but exact causal
    narrowing is allowed — no >=256-wide f32r constraint — and DMA/SBUF halve)
  - scores St [kpos, q] per k-tile pair, exactly causal-narrowed
  - attention*V flipped: stationary = E-tile [128 kpos, 128 q] (slice of the
    already-transposed e2), moving = V-aug [128 kpos, 65] -> ctx^T [128 q, 65]
    costs 65 rows/tile instead of 128; the softmax denominator rides along as
    column 64 via an all-ones column in V-aug
  - normalization on DVE: per-partition reciprocal + tensor_scalar multiply
    while copying ctx^T out of PSUM (q is the partition axis there), then PE
    transposes back to hd-major [64, 512] per head
  - software-pipelined emission: every "beat" issues one score pair + exp,
    then runs one delayed thunk (the previous pair's ctx matmuls, or head-tail
    work), then filler matmuls (next-block QKV paced evenly; output
    projections deferred into block 3 where exp latency would otherwise
    starve the PE)
"""
import collections
import numpy as np

T = 2048
D = 1024
B = 4
H = 16
HL = 8            # heads per core
NP = 4            # head pairs per core
QB = 512          # q-block width
NQB = T // QB     # 4 q-blocks
NKT = T // 128    # 16 k-tiles

_COMPILED = None


# --------------------------------------------------------------------------
# bass kernel build
# --------------------------------------------------------------------------
def _build_bass():
    import concourse.bass as bass
    import concourse.mybir as mybir
    from concourse.tile import TileContext

    F32 = mybir.dt.float32
    BF16 = mybir.dt.bfloat16
    Act = mybir.ActivationFunctionType
    Alu = mybir.AluOpType

    nc = bass.Bass()
    xt = nc.dram_tensor("xt", [D, T], BF16, kind="ExternalInput")
    wq = nc.dram_tensor("wq", [D, 512], BF16, kind="ExternalInput")
    wk = nc.dram_tensor("wk", [D, 512], BF16, kind="ExternalInput")
    wv = nc.dram_tensor("wv", [D, 512], BF16, kind="ExternalInput")
    wp = nc.dram_tensor("wp", [512, D], BF16, kind="ExternalInput")
    mask1 = nc.dram_tensor("mask1", [128, 128], BF16, kind="ExternalInput")
    ident = nc.dram_tensor("ident", [128, 128], BF16, kind="ExternalInput")
    vones = nc.dram_tensor("vones", [128, NKT * HL], BF16, kind="ExternalInput")
    outt = nc.dram_tensor("outt", [D, T], F32, kind="ExternalOutput")

    with TileContext(nc) as tc, nc.allow_low_precision(reason="bf16 pipeline"):
        with tc.tile_pool(name="wts", bufs=1) as wts, \
             tc.tile_pool(name="xp", bufs=2) as xp, \
             tc.tile_pool(name="big", bufs=1) as big, \
             tc.tile_pool(name="qtp", bufs=8) as qtp, \
             tc.tile_pool(name="ep", bufs=4) as ep, \
             tc.tile_pool(name="ctsp", bufs=2) as ctsp, \
             tc.tile_pool(name="rcq", bufs=2) as rcq, \
             tc.tile_pool(name="cxp", bufs=16) as cxp, \
             tc.tile_pool(name="sm", bufs=1) as sm, \
             tc.tile_pool(name="osb", bufs=8) as osb, \
             tc.tile_pool(name="pp3", bufs=8) as pp3, \
             tc.tile_pool(name="pst", bufs=2, space="PSUM") as pst, \
             tc.tile_pool(name="pctx", bufs=1, space="PSUM") as pctx, \
             tc.tile_pool(name="ptt", bufs=1, space="PSUM") as ptt, \
             tc.tile_pool(name="paux", bufs=2, space="PSUM") as paux:

            # ---------------- weights/constants ----------------
            wq_t = wts.tile([128, 8, 512], BF16, tag="wq")
            wk_t = wts.tile([128, 8, 512], BF16, tag="wk")
            wv_t = wts.tile([128, 8, 512], BF16, tag="wv")
            wp_t = wts.tile([128, 4, 1024], BF16, tag="wp")
            wqr = wq[:].rearrange("(n p) m -> p n m", p=128)
            wkr = wk[:].rearrange("(n p) m -> p n m", p=128)
            wvr = wv[:].rearrange("(n p) m -> p n m", p=128)
            x_tiles = {}

            def load_x(tb):
                # 2-slice chunks on the SP queue: 4 issues instead of 8
                x_t = xp.tile([128, 8, 512], BF16, tag="x", name=f"x_t{tb}")
                xr = xt[:, QB * tb:QB * (tb + 1)].rearrange("(n p) m -> p n m", p=128)
                for c in range(4):
                    nc.sync.dma_start(x_t[:, 2 * c:2 * c + 2, :], xr[:, 2 * c:2 * c + 2, :])
                x_tiles[tb] = x_t

            # DMA issue is serialized per DGE queue (~600ns each), so spread
            # the prologue loads across the two HWDGE queues (SP + Act):
            #   SP:  wq, x0 (critical path for the first Q chains), then x1..
            #   Act: wk, wv, wp, constants (needed a few us later)
            x_t0 = xp.tile([128, 8, 512], BF16, tag="x", name="x_t0")
            xr0 = xt[:, 0:QB].rearrange("(n p) m -> p n m", p=128)
            for c in range(4):
                nc.sync.dma_start(wq_t[:, 2 * c:2 * c + 2, :], wqr[:, 2 * c:2 * c + 2, :])
                nc.sync.dma_start(x_t0[:, 2 * c:2 * c + 2, :], xr0[:, 2 * c:2 * c + 2, :])
            x_tiles[0] = x_t0
            for c in range(4):
                nc.scalar.dma_start(wk_t[:, 2 * c:2 * c + 2, :], wkr[:, 2 * c:2 * c + 2, :])
            for c in range(4):
                nc.scalar.dma_start(wv_t[:, 2 * c:2 * c + 2, :], wvr[:, 2 * c:2 * c + 2, :])
            m1 = sm.tile([128, 128], BF16, tag="m1")
            nc.scalar.dma_start(m1[:], mask1[:])
            idn = sm.tile([128, 128], BF16, tag="idn")
            nc.scalar.dma_start(idn[:], ident[:])

            kt_t = big.tile([128, 4, T], BF16, tag="kt")
            va_t = big.tile([128, NKT, HL, 65], BF16, tag="va")
            nc.scalar.dma_start(
                va_t[:, :, :, 64:65].squeeze(3),
                vones[:].rearrange("p (n h) -> p n h", n=NKT))
            wpr = wp[:].rearrange("(n p) m -> p n m", p=128)
            for c in range(2):
                nc.scalar.dma_start(wp_t[:, 2 * c:2 * c + 2, :], wpr[:, 2 * c:2 * c + 2, :])

            q_tiles = {}     # (j, p) -> sbuf tile [128, 512]
            ctx_tiles = {}   # (j, p) -> sbuf tile [128, 512] (normalized pair)

            # ---------------- thunk plumbing ----------------
            # a thunk is (rows, fn); rows = PE rows it will emit (for pacing)
            pending = collections.deque()   # attention work delayed >=1 beat
            qkv_q = collections.deque()     # next-block QKV (must finish)
            proj_q = collections.deque()    # deferred output projections
            late_q = collections.deque()    # deferred head tails (blocks 0-1)

            def gen_q(j):
                x_t = x_tiles[j]
                out = []
                for p in range(4):
                    box = {}
                    for kk in range(8):
                        def f(p=p, kk=kk, box=box, x_t=x_t, j=j):
                            if kk == 0:
                                box["ps"] = paux.tile([128, 512], F32, tag="aux",
                                                      name=f"psq{j}_{p}")
                                box["q"] = qtp.tile([128, 512], BF16, tag="qt",
                                                    name=f"qt{j}_{p}")
                                q_tiles[(j, p)] = box["q"]
                            nc.tensor.matmul(box["ps"][:],
                                             wq_t[:, kk, 128 * p:128 * (p + 1)],
                                             x_t[:, kk, :], start=(kk == 0), stop=(kk == 7))
                            if kk == 7:
                                nc.vector.tensor_copy(box["q"][:], box["ps"][:])
                        out.append((512, f))
                return out

            def gen_kv(j):
                x_t = x_tiles[j]
                out = []
                for p in range(4):
                    box = {}
                    for kk in range(8):
                        def f(p=p, kk=kk, box=box, x_t=x_t, j=j):
                            if kk == 0:
                                box["ps"] = paux.tile([128, 512], F32, tag="aux",
                                                      name=f"psk{j}_{p}")
                            nc.tensor.matmul(box["ps"][:],
                                             wk_t[:, kk, 128 * p:128 * (p + 1)],
                                             x_t[:, kk, :], start=(kk == 0), stop=(kk == 7))
                            if kk == 7:
                                nc.vector.tensor_copy(
                                    kt_t[:, p, QB * j:QB * (j + 1)], box["ps"][:])
                        out.append((512, f))
                for tt in range(4):
                    box = {}
                    for kk in range(8):
                        def f(tt=tt, kk=kk, box=box, x_t=x_t, j=j):
                            if kk == 0:
                                box["ps"] = paux.tile([128, 512], F32, tag="aux",
                                                      name=f"psv{j}_{tt}")
                            nc.tensor.matmul(box["ps"][:],
                                             x_t[:, kk, 128 * tt:128 * (tt + 1)],
                                             wv_t[:, kk, :], start=(kk == 0), stop=(kk == 7))
                            if kk == 7:
                                nc.vector.tensor_copy(
                                    va_t[:, 4 * j + tt, :, 0:64],
                                    box["ps"][:].rearrange("p (h d) -> p h d", h=HL))
                        out.append((512, f))
                return out

            def proj_steps(j, m, ps, use_pst, dma_eng, box):
                """Thunks for output-projection chain m of block j over the
                p-range `ps`."""
                out = []
                for p in ps:
                    def f(m=m, p=p, box=box, j=j, use_pst=use_pst, dma_eng=dma_eng):
                        if "pf" not in box:
                            if use_pst:
                                t = pst.tile([128, 1024], F32, tag="st",
                                             name=f"pf{j}_{m}")
                                box["pf"] = t[:, 0:512]
                            else:
                                box["pf"] = paux.tile([128, 512], F32, tag="aux",
                                                      name=f"pf{j}_{m}")[:]
                        nc.tensor.matmul(box["pf"],
                                         wp_t[:, p, 128 * m:128 * (m + 1)],
                                         ctx_tiles[(j, p)][:], start=(p == 0), stop=(p == 3))
                        if p == 3:
                            ob = osb.tile([128, 512], F32, tag="ob", name=f"ob{j}_{m}")
                            if use_pst:
                                # split the epilogue copies across DVE and the
                                # (idle) ACT engine — Copy lives in the same
                                # act-table set as Exp, and gpsimd can't read
                                # PSUM
                                nc.scalar.copy(ob[:], box["pf"])
                            else:
                                nc.vector.tensor_copy(ob[:], box["pf"])
                            dma_eng.dma_start(
                                outt[128 * m:128 * (m + 1), QB * j:QB * (j + 1)], ob[:])
                    out.append((512, f))
                return out

            part3 = {}

            def gen_proj3_stage_a():
                """Last block's projections, p=0..2: run during the final two
                heads' beats on transient paux slots; partials park in SBUF
                (bf16) so no PSUM is held across the last pair's completion."""
                out = []
                for m in range(8):
                    box = {}
                    for p in range(3):
                        def f(m=m, p=p, box=box):
                            j = NQB - 1
                            if p == 0:
                                box["pf"] = paux.tile([128, 512], F32, tag="aux",
                                                      name=f"pf3a_{m}")[:]
                            nc.tensor.matmul(box["pf"],
                                             wp_t[:, p, 128 * m:128 * (m + 1)],
                                             ctx_tiles[(j, p)][:],
                                             start=(p == 0), stop=(p == 2))
                            if p == 2:
                                pt = pp3.tile([128, 512], BF16, tag="pp",
                                              name=f"part3_{m}")
                                part3[m] = pt
                                nc.vector.tensor_copy(pt[:], box["pf"])
                        out.append((512, f))
                return out

            def gen_proj3_stage_b():
                """Epilogue: re-inject the parked partial via an identity
                matmul, add the p=3 term, write out."""
                out = []
                for m in range(8):
                    def f(m=m):
                        j = NQB - 1
                        if m % 2 == 1:
                            t = pst.tile([128, 1024], F32, tag="st", name=f"pf3b_{m}")
                            pf = t[:, 0:512]
                        else:
                            pf = paux.tile([128, 512], F32, tag="aux",
                                           name=f"pf3b_{m}")[:]
                        nc.tensor.matmul(pf, idn[:], part3[m][:],
                                         start=True, stop=False)
                        nc.tensor.matmul(pf, wp_t[:, 3, 128 * m:128 * (m + 1)],
                                         ctx_tiles[(j, 3)][:], start=False, stop=True)
                        ob = osb.tile([128, 512], F32, tag="ob", name=f"ob3_{m}")
                        if m % 2 == 1:
                            nc.scalar.copy(ob[:], pf)
                        else:
                            nc.vector.tensor_copy(ob[:], pf)
                        dma_eng = nc.scalar if m % 2 == 1 else nc.sync
                        dma_eng.dma_start(
                            outt[128 * m:128 * (m + 1), QB * j:QB * (j + 1)], ob[:])
                    out.append((1024, f))
                return out

            def gen_proj(j):
                last = (j == NQB - 1)
                out = []
                for m in range(8):
                    out.extend(proj_steps(j, m, range(4), last and m % 2 == 1,
                                          nc.scalar if last and m % 2 == 1 else nc.sync,
                                          {}))
                return out

            # ---------------- per-beat emission ----------------
            ROW_TARGET = 2700   # ~1038ns of exp per beat, in PE rows

            state = {"qkv_done": 0, "qkv_total": 0, "beat": 0, "beats_total": 1,
                     "allow_proj": False, "proj_floor": 0,
                     "proj_done": 0, "proj_total": 0, "prefill_proj": False}

            def pop_one_filler():
                """Emit one independent filler matmul; returns its rows or
                None when nothing is available. Both queues are paced evenly
                over the block's beats so late beats (where exp latency
                dominates) still have cover."""
                tgt = -(-state["qkv_total"] * state["beat"] // state["beats_total"])
                if state["qkv_done"] < tgt and qkv_q:
                    r, f = qkv_q.popleft()
                    f()
                    state["qkv_done"] += 1
                    return r
                ptgt = -(-state["proj_total"] * state["beat"] // state["beats_total"])
                if state["prefill_proj"] and state["proj_done"] < ptgt:
                    if late_q:
                        r, f = late_q.popleft()
                        f()
                        state["proj_done"] += 1
                        return r
                    if len(proj_q) > state["proj_floor"]:
                        r, f = proj_q.popleft()
                        f()
                        state["proj_done"] += 1
                        return r
                return None

            def run_beat_tail(rows):
                # a few independent fillers ahead of the
                # dependency-stalled attention thunk
                for _ in range(3):
                    r = pop_one_filler()
                    if r is None:
                        break
                    rows += r
                # one delayed attention thunk
                if pending:
                    r, f = pending.popleft()
                    f()
                    rows += r
                # paced QKV
                tgt = -(-state["qkv_total"] * state["beat"] // state["beats_total"])
                while state["qkv_done"] < tgt and qkv_q:
                    r, f = qkv_q.popleft()
                    f()
                    state["qkv_done"] += 1
                    rows += r
                # top up with deferred tails/projections (paced)
                while rows < ROW_TARGET:
                    ptgt = -(-state["proj_total"] * state["beat"]
                             // state["beats_total"])
                    if not (state["allow_proj"] and state["proj_done"] < ptgt):
                        break
                    if late_q:
                        r, f = late_q.popleft()
                    elif len(proj_q) > state["proj_floor"]:
                        r, f = proj_q.popleft()
                    else:
                        break
                    f()
                    state["proj_done"] += 1
                    rows += r
                # bound the delayed-thunk backlog (shallow blocks append
                # faster than one-per-beat pops)
                while len(pending) > 5:
                    pop_one_filler()
                    r, f = pending.popleft()
                    f()

            def drain_pending(keep=2):
                while len(pending) > keep:
                    r, f = pending.popleft()
                    f()
                    pop_one_filler()

            tt_tiles = {}

            def attn_pair_beat(j, h, ip):
                """Emit one beat: scores pair + exp + mask, then delayed work
                and fillers; queue this pair's ctx matmuls."""
                p, s = divmod(h, 2)
                q_tile = q_tiles[(j, p)]
                hs = slice(64 * s, 64 * s + 64)
                tp = (64 * s, 0)
                i0, i1 = 2 * ip, 2 * ip + 1
                o0, o1 = i0 - 4 * j, i1 - 4 * j
                cs0 = max(0, 128 * o0)
                cs1 = max(0, 128 * o1)
                if ip == 2 * (j + 1) - 1:
                    # first (diagonal-most) pair of the head: allocate the
                    # ctx^T accumulator bank
                    tt_tiles[("c", j, h)] = pctx.tile([128, 260], F32, tag="ctxa",
                                                      name=f"ctxa{j}_{h}")
                st = pst.tile([128, 1024], F32, tag="st", name=f"st{j}_{h}_{ip}")
                nc.tensor.matmul(st[:, cs0:512],
                                 kt_t[hs, p, 128 * i0:128 * (i0 + 1)],
                                 q_tile[hs, cs0:512],
                                 start=True, stop=True, tile_position=tp)
                nc.tensor.matmul(st[:, 512 + cs1:1024],
                                 kt_t[hs, p, 128 * i1:128 * (i1 + 1)],
                                 q_tile[hs, cs1:512],
                                 start=True, stop=True, tile_position=tp)
                e2 = ep.tile([128, 1024], BF16, tag="e", name=f"e{j}_{h}_{ip}")
                if o1 < 0:
                    nc.scalar.activation(e2[:], st[:], Act.Exp)
                else:
                    nc.scalar.activation(e2[:, cs0:512], st[:, cs0:512], Act.Exp)
                    nc.scalar.activation(e2[:, 512 + cs1:1024],
                                         st[:, 512 + cs1:1024], Act.Exp)
                    for (oo, base) in ((o0, 0), (o1, 512)):
                        if 0 <= oo:
                            z = slice(base + 128 * oo, base + 128 * (oo + 1))
                            nc.gpsimd.tensor_tensor(e2[:, z], e2[:, z], m1[:],
                                                    op=Alu.mult)
                rows = (512 - cs0) + (512 - cs1)
                nctx = (4 - max(0, o1)) + (4 - max(0, o0))

                def ctx_f(j=j, h=h, ip=ip, e2=e2, i0=i0, i1=i1, o0=o0, o1=o1):
                    # the whole [128, 260] accumulator is ONE hardware
                    # accumulation group: start=True clears the full PSUM
                    # bank, so only the head's first matmul (diag ktile,
                    # subtile 3) starts; every other subtile accumulates
                    # onto the cleared bank
                    ctxa = tt_tiles[("c", j, h)]
                    for (i, base, o) in ((i1, 512, o1), (i0, 0, o0)):
                        for qq in range(max(0, o), 4):
                            nc.tensor.matmul(
                                ctxa[:, 65 * qq:65 * (qq + 1)],
                                e2[:, base + 128 * qq:base + 128 * (qq + 1)],
                                va_t[:, i, h, :],
                                start=(i == 4 * j + 3 and qq == 3),
                                stop=(i == 0 and qq == 3),
                                skip_group_check=True)
                return rows, (65 * nctx, ctx_f)

            def tail_a(j, h):
                def f(j=j, h=h):
                    ctxa = tt_tiles[("c", j, h)]
                    # ONE plain copy out of PSUM, so the single ctx^T
                    # accumulator bank frees for the next head after ~400ns;
                    # reciprocal + scaling run from the SBUF copy (2x DVE)
                    raw = ctsp.tile([128, 260], BF16, tag="raw", bufs=2,
                                    name=f"raw{j}_{h}")
                    nc.vector.tensor_copy(raw[:], ctxa[:])
                    rc = rcq.tile([128, 4], F32, tag="rc", name=f"rc{j}_{h}")
                    # blocks 0-1 defer their transposes into blocks 2-3, so
                    # up to 16 cts tiles stay live
                    cts = ctsp.tile([128, 256], BF16, tag="cts", bufs=18,
                                    name=f"cts{j}_{h}")
                    tt_tiles[("s", j, h)] = cts
                    raw4 = raw[:].rearrange("p (q c) -> p q c", q=4)
                    nc.vector.reciprocal(rc[:], raw4[:, :, 64:65].squeeze(2))
                    for qq in range(4):
                        nc.vector.tensor_scalar_mul(
                            cts[:, 64 * qq:64 * (qq + 1)],
                            raw[:, 65 * qq:65 * qq + 64],
                            rc[:, qq:qq + 1])
                return (0, f)

            def tail_b(j, h):
                def f(j=j, h=h):
                    p, s = divmod(h, 2)
                    if s == 0:
                        tt_tiles[("t", j, p)] = ptt.tile([128, 512], BF16, tag="tt",
                                                      name=f"tt{j}_{p}")
                    tt = tt_tiles[("t", j, p)]
                    cts = tt_tiles[("s", j, h)]
                    for qq in range(4):
                        nc.tensor.transpose(
                            tt[64 * s:64 * s + 64, 128 * qq:128 * (qq + 1)],
                            cts[:, 64 * qq:64 * (qq + 1)], idn[:])
                return (512, f)

            def tail_c(j, p):
                def f(j=j, p=p):
                    ctx_pair = cxp.tile([128, 512], BF16, tag="ctx", name=f"ctx{j}_{p}")
                    ctx_tiles[(j, p)] = ctx_pair
                    nc.vector.tensor_copy(ctx_pair[:], tt_tiles[("t", j, p)][:])
                    if p == 3:
                        proj_q.extend(gen_proj(j))
                return (0, f)

            # ---------------- prologue: QKV for block 0 ----------------
            # Q runs kk-major across 4 interleaved accumulation chains (2
            # paux slots + 2 borrowed score slots) so each arriving x-chunk
            # DMA feeds 4 matmuls — the chain-major order would stall on the
            # serialized x0 chunk issues
            psq, qts = [], []
            for p in range(4):
                if p < 2:
                    ps = paux.tile([128, 512], F32, tag="aux", name=f"psq0_{p}")[:]
                else:
                    ps = pst.tile([128, 1024], F32, tag="st", name=f"psq0_{p}")[:, 0:512]
                psq.append(ps)
                qt = qtp.tile([128, 512], BF16, tag="qt", name=f"qt0_{p}")
                q_tiles[(0, p)] = qt
                qts.append(qt)
            for kk in range(8):
                for p in range(4):
                    nc.tensor.matmul(psq[p], wq_t[:, kk, 128 * p:128 * (p + 1)],
                                     x_t0[:, kk, :], start=(kk == 0), stop=(kk == 7))
            for p in range(4):
                nc.vector.tensor_copy(qts[p][:], psq[p])
            for r, f in gen_kv(0):
                f()

            # ---------------- main loop ----------------
            for j in range(NQB):
                qkv_q.clear()
                if j + 1 < NQB:
                    load_x(j + 1)
                    qkv_q.extend(gen_q(j + 1))
                    qkv_q.extend(gen_kv(j + 1))
                npair = 2 * (j + 1)
                state["qkv_total"] = len(qkv_q)
                state["qkv_done"] = 0
                state["beats_total"] = 8 * npair
                state["beat"] = 0
                state["allow_proj"] = (j >= 2)
                state["prefill_proj"] = (j == NQB - 1)
                state["proj_floor"] = 8
                state["proj_done"] = 0
                state["proj_total"] = len(proj_q) + len(late_q)
                for h in range(HL):
                    first = True
                    for ip in reversed(range(npair)):
                        state["beat"] += 1
                        rows, ctx_thunk = attn_pair_beat(j, h, ip)
                        if first:
                            # head boundary: clear old ctx thunks behind the
                            # freshly issued scores+exp, but leave the
                            # previous head's tail chain (A/B/C) to spread
                            # over the next beats — B stalls on A's DVE work
                            # if popped in the same beat
                            drain_pending(keep=2 if len(pending) <= 4 else 3)
                            first = False
                        run_beat_tail(rows)
                        pending.append(ctx_thunk)
                    pending.append(tail_a(j, h))
                    if h % 2 == 1:
                        # transpose + assemble as one atomic per-pair entry
                        # (the single tt PSUM slot must not interleave two
                        # pairs); blocks 0-1 defer theirs into blocks 2-3,
                        # where exp latency otherwise starves the PE
                        ra, fa = tail_b(j, h - 1)
                        rb, fb = tail_b(j, h)
                        rc_, fc = tail_c(j, h // 2)

                        def bc(fa=fa, fb=fb, fc=fc):
                            fa()
                            fb()
                            fc()
                        entry = (ra + rb + rc_, bc)
                        if j < 2:
                            late_q.append(entry)
                        else:
                            pending.append(entry)
                # block end: QKV for next block must be complete
                while qkv_q:
                    r, f = qkv_q.popleft()
                    f()

            # ---------------- epilogue ----------------
            state["allow_proj"] = True
            state["prefill_proj"] = True
            state["proj_floor"] = 0
            drain_pending(keep=0)
            while late_q:
                r, f = late_q.popleft()
                f()
            while proj_q:
                r, f = proj_q.popleft()
                f()
    return nc


def _split_waits(nc, limit=1):
    """This walrus build accepts only one sync wait per TPB_CTRL instruction;
    move excess waits onto preceding same-engine NOPs."""
    import concourse.mybir as mybir
    for f in nc.m.functions:
        for bb in f.blocks:
            new_insts = []
            for inst in bb.instructions:
                si = inst.sync_info
                if si is not None and si.on_wait and len(si.on_wait) > limit:
                    waits = list(si.on_wait)
                    k = 0
                    while len(waits) - k > limit:
                        chunk = waits[k:k + limit]
                        k += limit
                        nop = mybir.InstNoOp(name=f"{inst.name}_ws{k}")
                        nop.engine = inst.engine
                        nop.sync_info = mybir.SyncInfo(on_wait=chunk, on_update=[])
                        new_insts.append(nop)
                    si.on_wait = waits[k:]
                new_insts.append(inst)
            bb.instructions = new_insts


# --------------------------------------------------------------------------
# compile + SPMD execution via PJRT (axon) — jit once, reuse
# --------------------------------------------------------------------------
class _Compiled:
    def __init__(self, n_cores=8):
        import jax
        from jax.sharding import Mesh, PartitionSpec
        from jax.experimental.shard_map import shard_map
        import concourse.mybir as mybir
        from concourse.bass2jax import (_bass_exec_p, install_neuronx_cc_hook,
                                        partition_id_tensor)

        nc = _build_bass()
        _split_waits(nc)
        install_neuronx_cc_hook()
        partition_name = nc.partition_id_tensor.name if nc.partition_id_tensor else None
        in_names, out_names, out_avals, zero_outs = [], [], [], []
        for alloc in nc.m.functions[0].allocations:
            if not isinstance(alloc, mybir.MemoryLocationSet):
                continue
            name = alloc.memorylocations[0].name
            if alloc.kind == "ExternalInput":
                if name != partition_name:
                    in_names.append(name)
            elif alloc.kind == "ExternalOutput":
                shape = tuple(alloc.tensor_shape)
                dtype = mybir.dt.np(alloc.dtype)
                out_names.append(name)
                out_avals.append(jax.core.ShapedArray(shape, dtype))
                zero_outs.append(np.zeros(shape, dtype))
        n_params = len(in_names)
        all_in_names = list(in_names) + list(out_names)
        if partition_name is not None:
            all_in_names.append(partition_name)

        def _body(*args):
            operands = list(args)
            if partition_name is not None:
                operands.append(partition_id_tensor())
            outs = _bass_exec_p.bind(
                *operands,
                out_avals=tuple(out_avals),
                in_names=tuple(all_in_names),
                out_names=tuple(out_names),
                lowering_input_output_aliases=(),
                sim_require_finite=True,
                sim_require_nnan=True,
                nc=nc,
            )
            return tuple(outs)

        devices = jax.devices()[:n_cores]
        assert len(devices) >= n_cores, f"need {n_cores} cores, have {len(devices)}"
        self.n_cores = n_cores
        self.in_names, self.out_names = in_names, out_names
        self.out_avals, self.zero_outs = out_avals, zero_outs
        mesh = Mesh(np.asarray(devices[:n_cores]), ("core",))
        in_specs = (PartitionSpec("core"),) * (n_params + len(out_names))
        out_specs = (PartitionSpec("core"),) * len(out_names)
        self.fn = jax.jit(
            shard_map(_body, mesh=mesh, in_specs=in_specs,
                      out_specs=out_specs, check_rep=False),
            keep_unused=True)

    def run(self, in_maps):
        import jax
        args = []
        for name in self.in_names:
            args.append(np.concatenate([np.asarray(m[name]) for m in in_maps], axis=0))
        for z in self.zero_outs:
            args.append(np.zeros((self.n_cores * z.shape[0], *z.shape[1:]), z.dtype))
        if not getattr(self, "_warm", False):
            # the very first execution after device bring-up can read
            # uninitialized PSUM; do one discarded warm-up pass
            jax.block_until_ready(self.fn(*args))
            self._warm = True
        outs = self.fn(*args)
        jax.block_until_ready(outs)
        res = []
        for c in range(self.n_cores):
            d = {}
            for i, name in enumerate(self.out_names):
                a = np.asarray(outs[i]).reshape(self.n_cores, *self.out_avals[i].shape)[c]
                d[name] = a
            res.append(d)
        return res


# --------------------------------------------------------------------------
# host-side shard / unshard
# --------------------------------------------------------------------------
def _bf16(a):
    import ml_dtypes
    return np.ascontiguousarray(a).astype(ml_dtypes.bfloat16)


def _make_core_inputs(x, Wq, Wk, Wv, Wp, core):
    g = core % 2
    b = core // 2
    rows = slice(512 * g, 512 * (g + 1))
    kl = np.arange(128)
    return {
        "xt": _bf16(x[b].T),
        # fold the 1/sqrt(head_dim) score scale into Wq
        "wq": _bf16(Wq[rows, :].T * 0.125),
        "wk": _bf16(Wk[rows, :].T),
        "wv": _bf16(Wv[rows, :].T),
        "wp": _bf16(Wp[:, rows].T),
        "mask1": _bf16((kl[:, None] <= kl[None, :]).astype(np.float32)),
        "ident": _bf16(np.eye(128, dtype=np.float32)),
        "vones": _bf16(np.ones((128, NKT * HL), np.float32)),
    }


def kernel(x, Wq, Wk, Wv, Wp):
    """Full-input / full-output causal MHA. x: (4, 2048, 1024) fp32;
    Wq/Wk/Wv/Wp: (1024, 1024) fp32. Returns (4, 2048, 1024) fp32."""
    global _COMPILED
    x = np.asarray(x, dtype=np.float32)
    Wq = np.asarray(Wq, dtype=np.float32)
    Wk = np.asarray(Wk, dtype=np.float32)
    Wv = np.asarray(Wv, dtype=np.float32)
    Wp = np.asarray(Wp, dtype=np.float32)
    assert x.shape == (B, T, D), x.shape

    if _COMPILED is None:
        _COMPILED = _Compiled(8)
    in_maps = [_make_core_inputs(x, Wq, Wk, Wv, Wp, c) for c in range(8)]
    results = _COMPILED.run(in_maps)

    out = np.empty((B, T, D), np.float32)
    for b in range(B):
        acc = results[2 * b]["outt"] + results[2 * b + 1]["outt"]
        out[b] = acc.T
    return out


# revision 5
# speedup vs baseline: 1.0058x; 1.0014x over previous
"""Causal multi-head attention (B=4, T=2048, D=1024, H=16) on 8 Trainium2 cores.

Sharding (data + tensor parallel): core c handles batch b = c//2 and head-group
g = c%2 (8 of the 16 heads). Wq/Wk/Wv are column-sharded by head, Wp is
row-sharded; the two per-batch partial outputs are summed on the host (this
replaces the device all-reduce — the host-side sum is the unshard step).

v2 restructure vs the 293us baseline (the cost model charges matmuls by output
free-dim rows only; PE re-ramps to half speed after every idle gap):
  - all matmul operands bf16 (same 1 cyc/row as f32r, but exact causal
    narrowing is allowed — no >=256-wide f32r constraint — and DMA/SBUF halve)
  - scores St [kpos, q] per k-tile pair, exactly causal-narrowed
  - attention*V flipped: stationary = E-tile [128 kpos, 128 q] (slice of the
    already-transposed e2), moving = V-aug [128 kpos, 65] -> ctx^T [128 q, 65]
    costs 65 rows/tile instead of 128; the softmax denominator rides along as
    column 64 via an all-ones column in V-aug
  - normalization on DVE: per-partition reciprocal + tensor_scalar multiply
    while copying ctx^T out of PSUM (q is the partition axis there), then PE
    transposes back to hd-major [64, 512] per head
  - software-pipelined emission: every "beat" issues one score pair + exp,
    then runs one delayed thunk (the previous pair's ctx matmuls, or head-tail
    work), then filler matmuls (next-block QKV paced evenly; output
    projections deferred into block 3 where exp latency would otherwise
    starve the PE)
"""
import collections
import numpy as np

T = 2048
D = 1024
B = 4
H = 16
HL = 8            # heads per core
NP = 4            # head pairs per core
QB = 512          # q-block width
NQB = T // QB     # 4 q-blocks
NKT = T // 128    # 16 k-tiles

_COMPILED = None


# --------------------------------------------------------------------------
# bass kernel build
# --------------------------------------------------------------------------
def _build_bass():
    import concourse.bass as bass
    import concourse.mybir as mybir
    from concourse.tile import TileContext

    F32 = mybir.dt.float32
    BF16 = mybir.dt.bfloat16
    Act = mybir.ActivationFunctionType
    Alu = mybir.AluOpType

    nc = bass.Bass()
    xt = nc.dram_tensor("xt", [D, T], BF16, kind="ExternalInput")
    wq = nc.dram_tensor("wq", [D, 512], BF16, kind="ExternalInput")
    wk = nc.dram_tensor("wk", [D, 512], BF16, kind="ExternalInput")
    wv = nc.dram_tensor("wv", [D, 512], BF16, kind="ExternalInput")
    wp = nc.dram_tensor("wp", [512, D], BF16, kind="ExternalInput")
    mask1 = nc.dram_tensor("mask1", [128, 128], BF16, kind="ExternalInput")
    ident = nc.dram_tensor("ident", [128, 128], BF16, kind="ExternalInput")
    vones = nc.dram_tensor("vones", [128, NKT * HL], BF16, kind="ExternalInput")
    outt = nc.dram_tensor("outt", [D, T], F32, kind="ExternalOutput")

    with TileContext(nc) as tc, nc.allow_low_precision(reason="bf16 pipeline"):
        with tc.tile_pool(name="wts", bufs=1) as wts, \
             tc.tile_pool(name="xp", bufs=2) as xp, \
             tc.tile_pool(name="big", bufs=1) as big, \
             tc.tile_pool(name="qtp", bufs=8) as qtp, \
             tc.tile_pool(name="ep", bufs=4) as ep, \
             tc.tile_pool(name="elp", bufs=18) as elp, \
             tc.tile_pool(name="ctsp", bufs=2) as ctsp, \
             tc.tile_pool(name="rcq", bufs=2) as rcq, \
             tc.tile_pool(name="cxp", bufs=16) as cxp, \
             tc.tile_pool(name="sm", bufs=1) as sm, \
             tc.tile_pool(name="osb", bufs=8) as osb, \
             tc.tile_pool(name="pp3", bufs=8) as pp3, \
             tc.tile_pool(name="pst", bufs=2, space="PSUM") as pst, \
             tc.tile_pool(name="pctx", bufs=1, space="PSUM") as pctx, \
             tc.tile_pool(name="ptt", bufs=1, space="PSUM") as ptt, \
             tc.tile_pool(name="paux", bufs=2, space="PSUM") as paux:

            # ---------------- weights/constants ----------------
            wq_t = wts.tile([128, 8, 512], BF16, tag="wq")
            wk_t = wts.tile([128, 8, 512], BF16, tag="wk")
            wv_t = wts.tile([128, 8, 512], BF16, tag="wv")
            wp_t = wts.tile([128, 4, 1024], BF16, tag="wp")
            wqr = wq[:].rearrange("(n p) m -> p n m", p=128)
            wkr = wk[:].rearrange("(n p) m -> p n m", p=128)
            wvr = wv[:].rearrange("(n p) m -> p n m", p=128)
            x_tiles = {}

            def load_x(tb):
                # 2-slice chunks on the SP queue: 4 issues instead of 8
                x_t = xp.tile([128, 8, 512], BF16, tag="x", name=f"x_t{tb}")
                xr = xt[:, QB * tb:QB * (tb + 1)].rearrange("(n p) m -> p n m", p=128)
                for c in range(4):
                    nc.sync.dma_start(x_t[:, 2 * c:2 * c + 2, :], xr[:, 2 * c:2 * c + 2, :])
                x_tiles[tb] = x_t

            # DMA issue is serialized per DGE queue (~600ns each), so spread
            # the prologue loads across the two HWDGE queues (SP + Act):
            #   SP:  wq, x0 (critical path for the first Q chains), then x1..
            #   Act: wk, wv, wp, constants (needed a few us later)
            x_t0 = xp.tile([128, 8, 512], BF16, tag="x", name="x_t0")
            xr0 = xt[:, 0:QB].rearrange("(n p) m -> p n m", p=128)
            for c in range(4):
                nc.sync.dma_start(wq_t[:, 2 * c:2 * c + 2, :], wqr[:, 2 * c:2 * c + 2, :])
                nc.sync.dma_start(x_t0[:, 2 * c:2 * c + 2, :], xr0[:, 2 * c:2 * c + 2, :])
            x_tiles[0] = x_t0
            for c in range(4):
                nc.scalar.dma_start(wk_t[:, 2 * c:2 * c + 2, :], wkr[:, 2 * c:2 * c + 2, :])
            for c in range(4):
                nc.scalar.dma_start(wv_t[:, 2 * c:2 * c + 2, :], wvr[:, 2 * c:2 * c + 2, :])
            m1 = sm.tile([128, 128], BF16, tag="m1")
            nc.scalar.dma_start(m1[:], mask1[:])
            idn = sm.tile([128, 128], BF16, tag="idn")
            nc.scalar.dma_start(idn[:], ident[:])

            kt_t = big.tile([128, 4, T], BF16, tag="kt")
            va_t = big.tile([128, NKT, HL, 65], BF16, tag="va")
            nc.scalar.dma_start(
                va_t[:, :, :, 64:65].squeeze(3),
                vones[:].rearrange("p (n h) -> p n h", n=NKT))
            wpr = wp[:].rearrange("(n p) m -> p n m", p=128)
            for c in range(2):
                nc.scalar.dma_start(wp_t[:, 2 * c:2 * c + 2, :], wpr[:, 2 * c:2 * c + 2, :])

            q_tiles = {}     # (j, p) -> sbuf tile [128, 512]
            ctx_tiles = {}   # (j, p) -> sbuf tile [128, 512] (normalized pair)

            # ---------------- thunk plumbing ----------------
            # a thunk is (rows, fn); rows = PE rows it will emit (for pacing)
            pending = collections.deque()   # attention work delayed >=1 beat
            qkv_q = collections.deque()     # next-block QKV (must finish)
            proj_q = collections.deque()    # deferred output projections
            late_q = collections.deque()    # deferred head tails (blocks 0-1)

            def gen_q(j):
                x_t = x_tiles[j]
                out = []
                for p in range(4):
                    box = {}
                    for kk in range(8):
                        def f(p=p, kk=kk, box=box, x_t=x_t, j=j):
                            if kk == 0:
                                box["ps"] = paux.tile([128, 512], F32, tag="aux",
                                                      name=f"psq{j}_{p}")
                                box["q"] = qtp.tile([128, 512], BF16, tag="qt",
                                                    name=f"qt{j}_{p}")
                                q_tiles[(j, p)] = box["q"]
                            nc.tensor.matmul(box["ps"][:],
                                             wq_t[:, kk, 128 * p:128 * (p + 1)],
                                             x_t[:, kk, :], start=(kk == 0), stop=(kk == 7))
                            if kk == 7:
                                nc.vector.tensor_copy(box["q"][:], box["ps"][:])
                        out.append((512, f))
                return out

            def gen_kv(j):
                x_t = x_tiles[j]
                out = []
                for p in range(4):
                    box = {}
                    for kk in range(8):
                        def f(p=p, kk=kk, box=box, x_t=x_t, j=j):
                            if kk == 0:
                                box["ps"] = paux.tile([128, 512], F32, tag="aux",
                                                      name=f"psk{j}_{p}")
                            nc.tensor.matmul(box["ps"][:],
                                             wk_t[:, kk, 128 * p:128 * (p + 1)],
                                             x_t[:, kk, :], start=(kk == 0), stop=(kk == 7))
                            if kk == 7:
                                nc.vector.tensor_copy(
                                    kt_t[:, p, QB * j:QB * (j + 1)], box["ps"][:])
                        out.append((512, f))
                for tt in range(4):
                    box = {}
                    for kk in range(8):
                        def f(tt=tt, kk=kk, box=box, x_t=x_t, j=j):
                            if kk == 0:
                                box["ps"] = paux.tile([128, 512], F32, tag="aux",
                                                      name=f"psv{j}_{tt}")
                            nc.tensor.matmul(box["ps"][:],
                                             x_t[:, kk, 128 * tt:128 * (tt + 1)],
                                             wv_t[:, kk, :], start=(kk == 0), stop=(kk == 7))
                            if kk == 7:
                                nc.vector.tensor_copy(
                                    va_t[:, 4 * j + tt, :, 0:64],
                                    box["ps"][:].rearrange("p (h d) -> p h d", h=HL))
                        out.append((512, f))
                return out

            def proj_steps(j, m, ps, use_pst, dma_eng, box):
                """Thunks for output-projection chain m of block j over the
                p-range `ps`."""
                out = []
                for p in ps:
                    def f(m=m, p=p, box=box, j=j, use_pst=use_pst, dma_eng=dma_eng):
                        if "pf" not in box:
                            if use_pst:
                                t = pst.tile([128, 1024], F32, tag="st",
                                             name=f"pf{j}_{m}")
                                box["pf"] = t[:, 0:512]
                            else:
                                box["pf"] = paux.tile([128, 512], F32, tag="aux",
                                                      name=f"pf{j}_{m}")[:]
                        nc.tensor.matmul(box["pf"],
                                         wp_t[:, p, 128 * m:128 * (m + 1)],
                                         ctx_tiles[(j, p)][:], start=(p == 0), stop=(p == 3))
                        if p == 3:
                            ob = osb.tile([128, 512], F32, tag="ob", name=f"ob{j}_{m}")
                            if use_pst:
                                # split the epilogue copies across DVE and the
                                # (idle) ACT engine — Copy lives in the same
                                # act-table set as Exp, and gpsimd can't read
                                # PSUM
                                nc.scalar.copy(ob[:], box["pf"])
                            else:
                                nc.vector.tensor_copy(ob[:], box["pf"])
                            dma_eng.dma_start(
                                outt[128 * m:128 * (m + 1), QB * j:QB * (j + 1)], ob[:])
                    out.append((512, f))
                return out

            part3 = {}

            def gen_proj3_stage_a():
                """Last block's projections, p=0..2: run during the final two
                heads' beats on transient paux slots; partials park in SBUF
                (bf16) so no PSUM is held across the last pair's completion."""
                out = []
                for m in range(8):
                    box = {}
                    for p in range(3):
                        def f(m=m, p=p, box=box):
                            j = NQB - 1
                            if p == 0:
                                box["pf"] = paux.tile([128, 512], F32, tag="aux",
                                                      name=f"pf3a_{m}")[:]
                            nc.tensor.matmul(box["pf"],
                                             wp_t[:, p, 128 * m:128 * (m + 1)],
                                             ctx_tiles[(j, p)][:],
                                             start=(p == 0), stop=(p == 2))
                            if p == 2:
                                pt = pp3.tile([128, 512], BF16, tag="pp",
                                              name=f"part3_{m}")
                                part3[m] = pt
                                nc.vector.tensor_copy(pt[:], box["pf"])
                        out.append((512, f))
                return out

            def gen_proj3_stage_b():
                """Epilogue: re-inject the parked partial via an identity
                matmul, add the p=3 term, write out."""
                out = []
                for m in range(8):
                    def f(m=m):
                        j = NQB - 1
                        if m % 2 == 1:
                            t = pst.tile([128, 1024], F32, tag="st", name=f"pf3b_{m}")
                            pf = t[:, 0:512]
                        else:
                            pf = paux.tile([128, 512], F32, tag="aux",
                                           name=f"pf3b_{m}")[:]
                        nc.tensor.matmul(pf, idn[:], part3[m][:],
                                         start=True, stop=False)
                        nc.tensor.matmul(pf, wp_t[:, 3, 128 * m:128 * (m + 1)],
                                         ctx_tiles[(j, 3)][:], start=False, stop=True)
                        ob = osb.tile([128, 512], F32, tag="ob", name=f"ob3_{m}")
                        if m % 2 == 1:
                            nc.scalar.copy(ob[:], pf)
                        else:
                            nc.vector.tensor_copy(ob[:], pf)
                        dma_eng = nc.scalar if m % 2 == 1 else nc.sync
                        dma_eng.dma_start(
                            outt[128 * m:128 * (m + 1), QB * j:QB * (j + 1)], ob[:])
                    out.append((1024, f))
                return out

            def gen_proj(j):
                last = (j == NQB - 1)
                out = []
                for m in range(8):
                    out.extend(proj_steps(j, m, range(4), last and m % 2 == 1,
                                          nc.scalar if last and m % 2 == 1 else nc.sync,
                                          {}))
                return out

            # ---------------- per-beat emission ----------------
            ROW_TARGET = 2700   # ~1038ns of exp per beat, in PE rows

            state = {"qkv_done": 0, "qkv_total": 0, "beat": 0, "beats_total": 1,
                     "allow_proj": False, "proj_floor": 0,
                     "proj_done": 0, "proj_total": 0, "prefill_proj": False}

            def pop_one_filler():
                """Emit one independent filler matmul; returns its rows or
                None when nothing is available. Both queues are paced evenly
                over the block's beats so late beats (where exp latency
                dominates) still have cover."""
                tgt = -(-state["qkv_total"] * state["beat"] // state["beats_total"])
                if state["qkv_done"] < tgt and qkv_q:
                    r, f = qkv_q.popleft()
                    f()
                    state["qkv_done"] += 1
                    return r
                ptgt = -(-state["proj_total"] * state["beat"] // state["beats_total"])
                if state["prefill_proj"] and state["proj_done"] < ptgt:
                    if late_q:
                        r, f = late_q.popleft()
                        f()
                        state["proj_done"] += 1
                        return r
                    if len(proj_q) > state["proj_floor"]:
                        r, f = proj_q.popleft()
                        f()
                        state["proj_done"] += 1
                        return r
                return None

            def run_beat_tail(rows):
                # a few independent fillers ahead of the
                # dependency-stalled attention thunk
                for _ in range(3):
                    r = pop_one_filler()
                    if r is None:
                        break
                    rows += r
                # one delayed attention thunk
                if pending:
                    r, f = pending.popleft()
                    f()
                    rows += r
                # paced QKV
                tgt = -(-state["qkv_total"] * state["beat"] // state["beats_total"])
                while state["qkv_done"] < tgt and qkv_q:
                    r, f = qkv_q.popleft()
                    f()
                    state["qkv_done"] += 1
                    rows += r
                # top up with deferred tails/projections (paced)
                while rows < ROW_TARGET:
                    ptgt = -(-state["proj_total"] * state["beat"]
                             // state["beats_total"])
                    if not (state["allow_proj"] and state["proj_done"] < ptgt):
                        break
                    if late_q:
                        r, f = late_q.popleft()
                    elif len(proj_q) > state["proj_floor"]:
                        r, f = proj_q.popleft()
                    else:
                        break
                    f()
                    state["proj_done"] += 1
                    rows += r
                # bound the delayed-thunk backlog (shallow blocks append
                # faster than one-per-beat pops)
                while len(pending) > 5:
                    pop_one_filler()
                    r, f = pending.popleft()
                    f()

            def drain_pending(keep=2):
                while len(pending) > keep:
                    r, f = pending.popleft()
                    f()
                    pop_one_filler()

            tt_tiles = {}

            def attn_pair_beat(j, h, ip):
                """Emit one beat: scores pair + exp + mask, then delayed work
                and fillers; queue this pair's ctx matmuls."""
                p, s = divmod(h, 2)
                q_tile = q_tiles[(j, p)]
                hs = slice(64 * s, 64 * s + 64)
                tp = (64 * s, 0)
                i0, i1 = 2 * ip, 2 * ip + 1
                o0, o1 = i0 - 4 * j, i1 - 4 * j
                cs0 = max(0, 128 * o0)
                cs1 = max(0, 128 * o1)
                if ip == 2 * (j + 1) - 1:
                    # first (diagonal-most) pair of the head: allocate the
                    # ctx^T accumulator bank
                    tt_tiles[("c", j, h)] = pctx.tile([128, 260], F32, tag="ctxa",
                                                      name=f"ctxa{j}_{h}")
                st = pst.tile([128, 1024], F32, tag="st", name=f"st{j}_{h}_{ip}")
                nc.tensor.matmul(st[:, cs0:512],
                                 kt_t[hs, p, 128 * i0:128 * (i0 + 1)],
                                 q_tile[hs, cs0:512],
                                 start=True, stop=True, tile_position=tp)
                nc.tensor.matmul(st[:, 512 + cs1:1024],
                                 kt_t[hs, p, 128 * i1:128 * (i1 + 1)],
                                 q_tile[hs, cs1:512],
                                 start=True, stop=True, tile_position=tp)
                e2 = ep.tile([128, 1024], BF16, tag="e", name=f"e{j}_{h}_{ip}")
                if o1 < 0:
                    nc.scalar.activation(e2[:], st[:], Act.Exp)
                else:
                    nc.scalar.activation(e2[:, cs0:512], st[:, cs0:512], Act.Exp)
                    nc.scalar.activation(e2[:, 512 + cs1:1024],
                                         st[:, 512 + cs1:1024], Act.Exp)
                    for (oo, base) in ((o0, 0), (o1, 512)):
                        if 0 <= oo:
                            z = slice(base + 128 * oo, base + 128 * (oo + 1))
                            nc.gpsimd.tensor_tensor(e2[:, z], e2[:, z], m1[:],
                                                    op=Alu.mult)
                rows = (512 - cs0) + (512 - cs1)
                return rows, make_ctx(j, h, e2, i0, i1, o0, o1)

            def make_ctx(j, h, e2, i0, i1, o0, o1):
                nctx = (4 - max(0, o1)) + (4 - max(0, o0))

                def ctx_f(j=j, h=h, e2=e2, i0=i0, i1=i1, o0=o0, o1=o1):
                    # the whole [128, 260] accumulator is ONE hardware
                    # accumulation group: start=True clears the full PSUM
                    # bank, so only the head's first matmul (diag ktile,
                    # subtile 3) starts; every other subtile accumulates
                    # onto the cleared bank
                    ctxa = tt_tiles[("c", j, h)]
                    for (i, base, o) in ((i1, 512, o1), (i0, 0, o0)):
                        for qq in range(max(0, o), 4):
                            nc.tensor.matmul(
                                ctxa[:, 65 * qq:65 * (qq + 1)],
                                e2[:, base + 128 * qq:base + 128 * (qq + 1)],
                                va_t[:, i, h, :],
                                start=(i == 4 * j + 3 and qq == 3),
                                stop=(i == 0 and qq == 3),
                                skip_group_check=True)
                return (65 * nctx, ctx_f)

            la_e2 = {}

            def emit_la(jn, h):
                """Lookahead: scores+exp for (block jn, head h, k-tiles 0-1)
                emitted a block early, while the ACT engine is otherwise
                idle; e2 parks in SBUF until block jn's ctx matmuls."""
                p, s = divmod(h, 2)
                q_tile = q_tiles[(jn, p)]
                hs = slice(64 * s, 64 * s + 64)
                tp = (64 * s, 0)
                st = pst.tile([128, 1024], F32, tag="st", name=f"lst{jn}_{h}")
                nc.tensor.matmul(st[:, 0:512], kt_t[hs, p, 0:128],
                                 q_tile[hs, :], start=True, stop=True,
                                 tile_position=tp)
                nc.tensor.matmul(st[:, 512:1024], kt_t[hs, p, 128:256],
                                 q_tile[hs, :], start=True, stop=True,
                                 tile_position=tp)
                e2 = elp.tile([128, 1024], BF16, tag="ela", name=f"le{jn}_{h}")
                nc.scalar.activation(e2[:], st[:], Act.Exp)
                la_e2[(jn, h)] = e2
                return 1024

            def tail_a(j, h):
                def f(j=j, h=h):
                    ctxa = tt_tiles[("c", j, h)]
                    # ONE plain copy out of PSUM, so the single ctx^T
                    # accumulator bank frees for the next head after ~400ns;
                    # reciprocal + scaling run from the SBUF copy (2x DVE)
                    raw = ctsp.tile([128, 260], BF16, tag="raw", bufs=2,
                                    name=f"raw{j}_{h}")
                    nc.vector.tensor_copy(raw[:], ctxa[:])
                    rc = rcq.tile([128, 4], F32, tag="rc", name=f"rc{j}_{h}")
                    # blocks 0-1 defer their transposes into blocks 2-3, so
                    # up to 16 cts tiles stay live
                    cts = ctsp.tile([128, 256], BF16, tag="cts", bufs=18,
                                    name=f"cts{j}_{h}")
                    tt_tiles[("s", j, h)] = cts
                    raw4 = raw[:].rearrange("p (q c) -> p q c", q=4)
                    nc.vector.reciprocal(rc[:], raw4[:, :, 64:65].squeeze(2))
                    for qq in range(4):
                        nc.vector.tensor_scalar_mul(
                            cts[:, 64 * qq:64 * (qq + 1)],
                            raw[:, 65 * qq:65 * qq + 64],
                            rc[:, qq:qq + 1])
                return (0, f)

            def tail_b(j, h):
                def f(j=j, h=h):
                    p, s = divmod(h, 2)
                    if s == 0:
                        tt_tiles[("t", j, p)] = ptt.tile([128, 512], BF16, tag="tt",
                                                      name=f"tt{j}_{p}")
                    tt = tt_tiles[("t", j, p)]
                    cts = tt_tiles[("s", j, h)]
                    for qq in range(4):
                        nc.tensor.transpose(
                            tt[64 * s:64 * s + 64, 128 * qq:128 * (qq + 1)],
                            cts[:, 64 * qq:64 * (qq + 1)], idn[:])
                return (512, f)

            def tail_c(j, p):
                def f(j=j, p=p):
                    ctx_pair = cxp.tile([128, 512], BF16, tag="ctx", name=f"ctx{j}_{p}")
                    ctx_tiles[(j, p)] = ctx_pair
                    nc.vector.tensor_copy(ctx_pair[:], tt_tiles[("t", j, p)][:])
                    if p == 3:
                        proj_q.extend(gen_proj(j))
                return (0, f)

            # ---------------- prologue: QKV for block 0 ----------------
            # Q runs kk-major across 4 interleaved accumulation chains (2
            # paux slots + 2 borrowed score slots) so each arriving x-chunk
            # DMA feeds 4 matmuls — the chain-major order would stall on the
            # serialized x0 chunk issues
            psq, qts = [], []
            for p in range(4):
                if p < 2:
                    ps = paux.tile([128, 512], F32, tag="aux", name=f"psq0_{p}")[:]
                else:
                    ps = pst.tile([128, 1024], F32, tag="st", name=f"psq0_{p}")[:, 0:512]
                psq.append(ps)
                qt = qtp.tile([128, 512], BF16, tag="qt", name=f"qt0_{p}")
                q_tiles[(0, p)] = qt
                qts.append(qt)
            for kk in range(8):
                for p in range(4):
                    nc.tensor.matmul(psq[p], wq_t[:, kk, 128 * p:128 * (p + 1)],
                                     x_t0[:, kk, :], start=(kk == 0), stop=(kk == 7))
            for p in range(4):
                nc.vector.tensor_copy(qts[p][:], psq[p])
            for r, f in gen_kv(0):
                f()

            # ---------------- main loop ----------------
            for j in range(NQB):
                qkv_q.clear()
                if j + 1 < NQB:
                    load_x(j + 1)
                    qkv_q.extend(gen_q(j + 1))
                    qkv_q.extend(gen_kv(j + 1))
                npair = 2 * (j + 1)
                state["qkv_total"] = len(qkv_q)
                state["qkv_done"] = 0
                state["beats_total"] = 8 * npair + (8 if j < 3 else 0)
                state["beat"] = 0
                state["allow_proj"] = (j >= 2)
                state["prefill_proj"] = (j == NQB - 1)
                state["proj_floor"] = 8
                state["proj_done"] = 0
                state["proj_total"] = len(proj_q) + len(late_q)
                for h in range(HL):
                    first = True
                    for ip in reversed(range(npair)):
                        state["beat"] += 1
                        if ip == 0 and (j, h) in la_e2:
                            # this pair's scores+exp were precomputed a
                            # block early — only the ctx matmuls remain
                            rows = 0
                            ctx_thunk = make_ctx(j, h, la_e2.pop((j, h)),
                                                 0, 1, -4 * j, 1 - 4 * j)
                        else:
                            rows, ctx_thunk = attn_pair_beat(j, h, ip)
                        if first:
                            # head boundary: clear old ctx thunks behind the
                            # freshly issued scores+exp, but leave the
                            # previous head's tail chain (A/B/C) to spread
                            # over the next beats — B stalls on A's DVE work
                            # if popped in the same beat
                            drain_pending(keep=2 if len(pending) <= 4 else 3)
                            first = False
                        run_beat_tail(rows)
                        pending.append(ctx_thunk)
                    if j < 3:
                        # lookahead beat: next block's far pair for this head
                        state["beat"] += 1
                        run_beat_tail(emit_la(j + 1, h))
                    pending.append(tail_a(j, h))
                    if h % 2 == 1:
                        # transpose + assemble as one atomic per-pair entry
                        # (the single tt PSUM slot must not interleave two
                        # pairs); blocks 0-1 defer theirs into blocks 2-3,
                        # where exp latency otherwise starves the PE
                        ra, fa = tail_b(j, h - 1)
                        rb, fb = tail_b(j, h)
                        rc_, fc = tail_c(j, h // 2)

                        def bc(fa=fa, fb=fb, fc=fc):
                            fa()
                            fb()
                            fc()
                        entry = (ra + rb + rc_, bc)
                        if j < 2:
                            late_q.append(entry)
                        else:
                            pending.append(entry)
                # block end: QKV for next block must be complete
                while qkv_q:
                    r, f = qkv_q.popleft()
                    f()

            # ---------------- epilogue ----------------
            state["allow_proj"] = True
            state["prefill_proj"] = True
            state["proj_floor"] = 0
            drain_pending(keep=0)
            while late_q:
                r, f = late_q.popleft()
                f()
            while proj_q:
                r, f = proj_q.popleft()
                f()
    return nc


def _split_waits(nc, limit=1):
    """This walrus build accepts only one sync wait per TPB_CTRL instruction;
    move excess waits onto preceding same-engine NOPs."""
    import concourse.mybir as mybir
    for f in nc.m.functions:
        for bb in f.blocks:
            new_insts = []
            for inst in bb.instructions:
                si = inst.sync_info
                if si is not None and si.on_wait and len(si.on_wait) > limit:
                    waits = list(si.on_wait)
                    k = 0
                    while len(waits) - k > limit:
                        chunk = waits[k:k + limit]
                        k += limit
                        nop = mybir.InstNoOp(name=f"{inst.name}_ws{k}")
                        nop.engine = inst.engine
                        nop.sync_info = mybir.SyncInfo(on_wait=chunk, on_update=[])
                        new_insts.append(nop)
                    si.on_wait = waits[k:]
                new_insts.append(inst)
            bb.instructions = new_insts


# --------------------------------------------------------------------------
# compile + SPMD execution via PJRT (axon) — jit once, reuse
# --------------------------------------------------------------------------
class _Compiled:
    def __init__(self, n_cores=8):
        import jax
        from jax.sharding import Mesh, PartitionSpec
        from jax.experimental.shard_map import shard_map
        import concourse.mybir as mybir
        from concourse.bass2jax import (_bass_exec_p, install_neuronx_cc_hook,
                                        partition_id_tensor)

        nc = _build_bass()
        _split_waits(nc)
        install_neuronx_cc_hook()
        partition_name = nc.partition_id_tensor.name if nc.partition_id_tensor else None
        in_names, out_names, out_avals, zero_outs = [], [], [], []
        for alloc in nc.m.functions[0].allocations:
            if not isinstance(alloc, mybir.MemoryLocationSet):
                continue
            name = alloc.memorylocations[0].name
            if alloc.kind == "ExternalInput":
                if name != partition_name:
                    in_names.append(name)
            elif alloc.kind == "ExternalOutput":
                shape = tuple(alloc.tensor_shape)
                dtype = mybir.dt.np(alloc.dtype)
                out_names.append(name)
                out_avals.append(jax.core.ShapedArray(shape, dtype))
                zero_outs.append(np.zeros(shape, dtype))
        n_params = len(in_names)
        all_in_names = list(in_names) + list(out_names)
        if partition_name is not None:
            all_in_names.append(partition_name)

        def _body(*args):
            operands = list(args)
            if partition_name is not None:
                operands.append(partition_id_tensor())
            outs = _bass_exec_p.bind(
                *operands,
                out_avals=tuple(out_avals),
                in_names=tuple(all_in_names),
                out_names=tuple(out_names),
                lowering_input_output_aliases=(),
                sim_require_finite=True,
                sim_require_nnan=True,
                nc=nc,
            )
            return tuple(outs)

        devices = jax.devices()[:n_cores]
        assert len(devices) >= n_cores, f"need {n_cores} cores, have {len(devices)}"
        self.n_cores = n_cores
        self.in_names, self.out_names = in_names, out_names
        self.out_avals, self.zero_outs = out_avals, zero_outs
        mesh = Mesh(np.asarray(devices[:n_cores]), ("core",))
        in_specs = (PartitionSpec("core"),) * (n_params + len(out_names))
        out_specs = (PartitionSpec("core"),) * len(out_names)
        self.fn = jax.jit(
            shard_map(_body, mesh=mesh, in_specs=in_specs,
                      out_specs=out_specs, check_rep=False),
            keep_unused=True)

    def run(self, in_maps):
        import jax
        args = []
        for name in self.in_names:
            args.append(np.concatenate([np.asarray(m[name]) for m in in_maps], axis=0))
        for z in self.zero_outs:
            args.append(np.zeros((self.n_cores * z.shape[0], *z.shape[1:]), z.dtype))
        if not getattr(self, "_warm", False):
            # the very first execution after device bring-up can read
            # uninitialized PSUM; do one discarded warm-up pass
            jax.block_until_ready(self.fn(*args))
            self._warm = True
        outs = self.fn(*args)
        jax.block_until_ready(outs)
        res = []
        for c in range(self.n_cores):
            d = {}
            for i, name in enumerate(self.out_names):
                a = np.asarray(outs[i]).reshape(self.n_cores, *self.out_avals[i].shape)[c]
                d[name] = a
            res.append(d)
        return res


# --------------------------------------------------------------------------
# host-side shard / unshard
# --------------------------------------------------------------------------
def _bf16(a):
    import ml_dtypes
    return np.ascontiguousarray(a).astype(ml_dtypes.bfloat16)


def _make_core_inputs(x, Wq, Wk, Wv, Wp, core):
    g = core % 2
    b = core // 2
    rows = slice(512 * g, 512 * (g + 1))
    kl = np.arange(128)
    return {
        "xt": _bf16(x[b].T),
        # fold the 1/sqrt(head_dim) score scale into Wq
        "wq": _bf16(Wq[rows, :].T * 0.125),
        "wk": _bf16(Wk[rows, :].T),
        "wv": _bf16(Wv[rows, :].T),
        "wp": _bf16(Wp[:, rows].T),
        "mask1": _bf16((kl[:, None] <= kl[None, :]).astype(np.float32)),
        "ident": _bf16(np.eye(128, dtype=np.float32)),
        "vones": _bf16(np.ones((128, NKT * HL), np.float32)),
    }


def kernel(x, Wq, Wk, Wv, Wp):
    """Full-input / full-output causal MHA. x: (4, 2048, 1024) fp32;
    Wq/Wk/Wv/Wp: (1024, 1024) fp32. Returns (4, 2048, 1024) fp32."""
    global _COMPILED
    x = np.asarray(x, dtype=np.float32)
    Wq = np.asarray(Wq, dtype=np.float32)
    Wk = np.asarray(Wk, dtype=np.float32)
    Wv = np.asarray(Wv, dtype=np.float32)
    Wp = np.asarray(Wp, dtype=np.float32)
    assert x.shape == (B, T, D), x.shape

    if _COMPILED is None:
        _COMPILED = _Compiled(8)
    in_maps = [_make_core_inputs(x, Wq, Wk, Wv, Wp, c) for c in range(8)]
    results = _COMPILED.run(in_maps)

    out = np.empty((B, T, D), np.float32)
    for b in range(B):
        acc = results[2 * b]["outt"] + results[2 * b + 1]["outt"]
        out[b] = acc.T
    return out


# revision 6
# speedup vs baseline: 1.0166x; 1.0108x over previous
"""Causal multi-head attention (B=4, T=2048, D=1024, H=16) on 8 Trainium2 cores.

Sharding (data + tensor parallel): core c handles batch b = c//2 and head-group
g = c%2 (8 of the 16 heads). Wq/Wk/Wv are column-sharded by head, Wp is
row-sharded; the two per-batch partial outputs are summed on the host (this
replaces the device all-reduce — the host-side sum is the unshard step).

v2 restructure vs the 293us baseline (the cost model charges matmuls by output
free-dim rows only; PE re-ramps to half speed after every idle gap):
  - all matmul operands bf16 (same 1 cyc/row as f32r, but exact causal
    narrowing is allowed — no >=256-wide f32r constraint — and DMA/SBUF halve)
  - scores St [kpos, q] per k-tile pair, exactly causal-narrowed
  - attention*V flipped: stationary = E-tile [128 kpos, 128 q] (slice of the
    already-transposed e2), moving = V-aug [128 kpos, 65] -> ctx^T [128 q, 65]
    costs 65 rows/tile instead of 128; the softmax denominator rides along as
    column 64 via an all-ones column in V-aug
  - normalization on DVE: per-partition reciprocal + tensor_scalar multiply
    while copying ctx^T out of PSUM (q is the partition axis there), then PE
    transposes back to hd-major [64, 512] per head
  - software-pipelined emission: every "beat" issues one score pair + exp,
    then runs one delayed thunk (the previous pair's ctx matmuls, or head-tail
    work), then filler matmuls (next-block QKV paced evenly; output
    projections deferred into block 3 where exp latency would otherwise
    starve the PE)
"""
import collections
import numpy as np

T = 2048
D = 1024
B = 4
H = 16
HL = 8            # heads per core
NP = 4            # head pairs per core
QB = 512          # q-block width
NQB = T // QB     # 4 q-blocks
NKT = T // 128    # 16 k-tiles

_COMPILED = None


# --------------------------------------------------------------------------
# bass kernel build
# --------------------------------------------------------------------------
def _build_bass():
    import concourse.bass as bass
    import concourse.mybir as mybir
    from concourse.tile import TileContext

    F32 = mybir.dt.float32
    BF16 = mybir.dt.bfloat16
    Act = mybir.ActivationFunctionType
    Alu = mybir.AluOpType

    nc = bass.Bass()
    xt = nc.dram_tensor("xt", [D, T], BF16, kind="ExternalInput")
    wq = nc.dram_tensor("wq", [D, 512], BF16, kind="ExternalInput")
    wk = nc.dram_tensor("wk", [D, 512], BF16, kind="ExternalInput")
    wv = nc.dram_tensor("wv", [D, 512], BF16, kind="ExternalInput")
    wp = nc.dram_tensor("wp", [512, D], BF16, kind="ExternalInput")
    mask1 = nc.dram_tensor("mask1", [128, 128], BF16, kind="ExternalInput")
    ident = nc.dram_tensor("ident", [128, 128], BF16, kind="ExternalInput")
    vones = nc.dram_tensor("vones", [128, NKT * HL], BF16, kind="ExternalInput")
    outt = nc.dram_tensor("outt", [D, T], F32, kind="ExternalOutput")

    with TileContext(nc) as tc, nc.allow_low_precision(reason="bf16 pipeline"):
        with tc.tile_pool(name="wts", bufs=1) as wts, \
             tc.tile_pool(name="xp", bufs=2) as xp, \
             tc.tile_pool(name="big", bufs=1) as big, \
             tc.tile_pool(name="qtp", bufs=8) as qtp, \
             tc.tile_pool(name="ep", bufs=8) as ep, \
             tc.tile_pool(name="elp", bufs=18) as elp, \
             tc.tile_pool(name="ctsp", bufs=2) as ctsp, \
             tc.tile_pool(name="rcq", bufs=2) as rcq, \
             tc.tile_pool(name="cxp", bufs=16) as cxp, \
             tc.tile_pool(name="sm", bufs=1) as sm, \
             tc.tile_pool(name="osb", bufs=8) as osb, \
             tc.tile_pool(name="pp3", bufs=8) as pp3, \
             tc.tile_pool(name="pst", bufs=2, space="PSUM") as pst, \
             tc.tile_pool(name="pctx", bufs=1, space="PSUM") as pctx, \
             tc.tile_pool(name="ptt", bufs=1, space="PSUM") as ptt, \
             tc.tile_pool(name="paux", bufs=2, space="PSUM") as paux:

            # ---------------- weights/constants ----------------
            wq_t = wts.tile([128, 8, 512], BF16, tag="wq")
            wk_t = wts.tile([128, 8, 512], BF16, tag="wk")
            wv_t = wts.tile([128, 8, 512], BF16, tag="wv")
            wp_t = wts.tile([128, 4, 1024], BF16, tag="wp")
            wqr = wq[:].rearrange("(n p) m -> p n m", p=128)
            wkr = wk[:].rearrange("(n p) m -> p n m", p=128)
            wvr = wv[:].rearrange("(n p) m -> p n m", p=128)
            x_tiles = {}

            def load_x(tb):
                # 2-slice chunks on the SP queue: 4 issues instead of 8
                x_t = xp.tile([128, 8, 512], BF16, tag="x", name=f"x_t{tb}")
                xr = xt[:, QB * tb:QB * (tb + 1)].rearrange("(n p) m -> p n m", p=128)
                for c in range(4):
                    nc.sync.dma_start(x_t[:, 2 * c:2 * c + 2, :], xr[:, 2 * c:2 * c + 2, :])
                x_tiles[tb] = x_t

            # DMA issue is serialized per DGE queue (~600ns each), so spread
            # the prologue loads across the two HWDGE queues (SP + Act):
            #   SP:  wq, x0 (critical path for the first Q chains), then x1..
            #   Act: wk, wv, wp, constants (needed a few us later)
            x_t0 = xp.tile([128, 8, 512], BF16, tag="x", name="x_t0")
            xr0 = xt[:, 0:QB].rearrange("(n p) m -> p n m", p=128)
            for c in range(4):
                nc.sync.dma_start(wq_t[:, 2 * c:2 * c + 2, :], wqr[:, 2 * c:2 * c + 2, :])
                nc.sync.dma_start(x_t0[:, 2 * c:2 * c + 2, :], xr0[:, 2 * c:2 * c + 2, :])
            x_tiles[0] = x_t0
            for c in range(4):
                nc.scalar.dma_start(wk_t[:, 2 * c:2 * c + 2, :], wkr[:, 2 * c:2 * c + 2, :])
            for c in range(4):
                nc.scalar.dma_start(wv_t[:, 2 * c:2 * c + 2, :], wvr[:, 2 * c:2 * c + 2, :])
            m1 = sm.tile([128, 128], BF16, tag="m1")
            nc.scalar.dma_start(m1[:], mask1[:])
            idn = sm.tile([128, 128], BF16, tag="idn")
            nc.scalar.dma_start(idn[:], ident[:])

            kt_t = big.tile([128, 4, T], BF16, tag="kt")
            va_t = big.tile([128, NKT, HL, 65], BF16, tag="va")
            nc.scalar.dma_start(
                va_t[:, :, :, 64:65].squeeze(3),
                vones[:].rearrange("p (n h) -> p n h", n=NKT))
            wpr = wp[:].rearrange("(n p) m -> p n m", p=128)
            for c in range(2):
                nc.scalar.dma_start(wp_t[:, 2 * c:2 * c + 2, :], wpr[:, 2 * c:2 * c + 2, :])

            q_tiles = {}     # (j, p) -> sbuf tile [128, 512]
            ctx_tiles = {}   # (j, p) -> sbuf tile [128, 512] (normalized pair)

            # ---------------- thunk plumbing ----------------
            # a thunk is (rows, fn); rows = PE rows it will emit (for pacing)
            pending = collections.deque()   # attention work delayed >=1 beat
            qkv_q = collections.deque()     # next-block QKV (must finish)
            proj_q = collections.deque()    # deferred output projections
            late_q = collections.deque()    # deferred head tails (blocks 0-1)

            def gen_q(j):
                x_t = x_tiles[j]
                out = []
                for p in range(4):
                    box = {}
                    for kk in range(8):
                        def f(p=p, kk=kk, box=box, x_t=x_t, j=j):
                            if kk == 0:
                                box["ps"] = paux.tile([128, 512], F32, tag="aux",
                                                      name=f"psq{j}_{p}")
                                box["q"] = qtp.tile([128, 512], BF16, tag="qt",
                                                    name=f"qt{j}_{p}")
                                q_tiles[(j, p)] = box["q"]
                            nc.tensor.matmul(box["ps"][:],
                                             wq_t[:, kk, 128 * p:128 * (p + 1)],
                                             x_t[:, kk, :], start=(kk == 0), stop=(kk == 7))
                            if kk == 7:
                                nc.vector.tensor_copy(box["q"][:], box["ps"][:])
                        out.append((512, f))
                return out

            def gen_kv(j):
                x_t = x_tiles[j]
                out = []
                for p in range(4):
                    box = {}
                    for kk in range(8):
                        def f(p=p, kk=kk, box=box, x_t=x_t, j=j):
                            if kk == 0:
                                box["ps"] = paux.tile([128, 512], F32, tag="aux",
                                                      name=f"psk{j}_{p}")
                            nc.tensor.matmul(box["ps"][:],
                                             wk_t[:, kk, 128 * p:128 * (p + 1)],
                                             x_t[:, kk, :], start=(kk == 0), stop=(kk == 7))
                            if kk == 7:
                                nc.vector.tensor_copy(
                                    kt_t[:, p, QB * j:QB * (j + 1)], box["ps"][:])
                        out.append((512, f))
                for tt in range(4):
                    box = {}
                    for kk in range(8):
                        def f(tt=tt, kk=kk, box=box, x_t=x_t, j=j):
                            if kk == 0:
                                box["ps"] = paux.tile([128, 512], F32, tag="aux",
                                                      name=f"psv{j}_{tt}")
                            nc.tensor.matmul(box["ps"][:],
                                             x_t[:, kk, 128 * tt:128 * (tt + 1)],
                                             wv_t[:, kk, :], start=(kk == 0), stop=(kk == 7))
                            if kk == 7:
                                nc.vector.tensor_copy(
                                    va_t[:, 4 * j + tt, :, 0:64],
                                    box["ps"][:].rearrange("p (h d) -> p h d", h=HL))
                        out.append((512, f))
                return out

            def proj_steps(j, m, ps, use_pst, dma_eng, box):
                """Thunks for output-projection chain m of block j over the
                p-range `ps`."""
                out = []
                for p in ps:
                    def f(m=m, p=p, box=box, j=j, use_pst=use_pst, dma_eng=dma_eng):
                        if "pf" not in box:
                            if use_pst:
                                t = pst.tile([128, 1024], F32, tag="st",
                                             name=f"pf{j}_{m}")
                                box["pf"] = t[:, 0:512]
                            else:
                                box["pf"] = paux.tile([128, 512], F32, tag="aux",
                                                      name=f"pf{j}_{m}")[:]
                        nc.tensor.matmul(box["pf"],
                                         wp_t[:, p, 128 * m:128 * (m + 1)],
                                         ctx_tiles[(j, p)][:], start=(p == 0), stop=(p == 3))
                        if p == 3:
                            ob = osb.tile([128, 512], F32, tag="ob", name=f"ob{j}_{m}")
                            if use_pst:
                                # split the epilogue copies across DVE and the
                                # (idle) ACT engine — Copy lives in the same
                                # act-table set as Exp, and gpsimd can't read
                                # PSUM
                                nc.scalar.copy(ob[:], box["pf"])
                            else:
                                nc.vector.tensor_copy(ob[:], box["pf"])
                            dma_eng.dma_start(
                                outt[128 * m:128 * (m + 1), QB * j:QB * (j + 1)], ob[:])
                    out.append((512, f))
                return out

            part3 = {}

            def gen_proj3_stage_a():
                """Last block's projections, p=0..2: run during the final two
                heads' beats on transient paux slots; partials park in SBUF
                (bf16) so no PSUM is held across the last pair's completion."""
                out = []
                for m in range(8):
                    box = {}
                    for p in range(3):
                        def f(m=m, p=p, box=box):
                            j = NQB - 1
                            if p == 0:
                                box["pf"] = paux.tile([128, 512], F32, tag="aux",
                                                      name=f"pf3a_{m}")[:]
                            nc.tensor.matmul(box["pf"],
                                             wp_t[:, p, 128 * m:128 * (m + 1)],
                                             ctx_tiles[(j, p)][:],
                                             start=(p == 0), stop=(p == 2))
                            if p == 2:
                                pt = pp3.tile([128, 512], BF16, tag="pp",
                                              name=f"part3_{m}")
                                part3[m] = pt
                                nc.vector.tensor_copy(pt[:], box["pf"])
                        out.append((512, f))
                return out

            def gen_proj3_stage_b():
                """Epilogue: re-inject the parked partial via an identity
                matmul, add the p=3 term, write out."""
                out = []
                for m in range(8):
                    def f(m=m):
                        j = NQB - 1
                        if m % 2 == 1:
                            t = pst.tile([128, 1024], F32, tag="st", name=f"pf3b_{m}")
                            pf = t[:, 0:512]
                        else:
                            pf = paux.tile([128, 512], F32, tag="aux",
                                           name=f"pf3b_{m}")[:]
                        nc.tensor.matmul(pf, idn[:], part3[m][:],
                                         start=True, stop=False)
                        nc.tensor.matmul(pf, wp_t[:, 3, 128 * m:128 * (m + 1)],
                                         ctx_tiles[(j, 3)][:], start=False, stop=True)
                        ob = osb.tile([128, 512], F32, tag="ob", name=f"ob3_{m}")
                        if m % 2 == 1:
                            nc.scalar.copy(ob[:], pf)
                        else:
                            nc.vector.tensor_copy(ob[:], pf)
                        dma_eng = nc.scalar if m % 2 == 1 else nc.sync
                        dma_eng.dma_start(
                            outt[128 * m:128 * (m + 1), QB * j:QB * (j + 1)], ob[:])
                    out.append((1024, f))
                return out

            def gen_proj(j):
                last = (j == NQB - 1)
                out = []
                for m in range(8):
                    out.extend(proj_steps(j, m, range(4), last and m % 2 == 1,
                                          nc.scalar if last and m % 2 == 1 else nc.sync,
                                          {}))
                return out

            # ---------------- per-beat emission ----------------
            ROW_TARGET = 2700   # ~1038ns of exp per beat, in PE rows

            state = {"qkv_done": 0, "qkv_total": 0, "beat": 0, "beats_total": 1,
                     "allow_proj": False, "proj_floor": 0,
                     "proj_done": 0, "proj_total": 0, "prefill_proj": False}

            def pop_one_filler():
                """Emit one independent filler matmul; returns its rows or
                None when nothing is available. Both queues are paced evenly
                over the block's beats so late beats (where exp latency
                dominates) still have cover."""
                tgt = -(-state["qkv_total"] * state["beat"] // state["beats_total"])
                if state["qkv_done"] < tgt and qkv_q:
                    r, f = qkv_q.popleft()
                    f()
                    state["qkv_done"] += 1
                    return r
                ptgt = -(-state["proj_total"] * state["beat"] // state["beats_total"])
                if state["prefill_proj"] and state["proj_done"] < ptgt:
                    if late_q:
                        r, f = late_q.popleft()
                        f()
                        state["proj_done"] += 1
                        return r
                    if len(proj_q) > state["proj_floor"]:
                        r, f = proj_q.popleft()
                        f()
                        state["proj_done"] += 1
                        return r
                return None

            def run_beat_tail(rows):
                # a few independent fillers ahead of the
                # dependency-stalled attention thunk
                for _ in range(3):
                    r = pop_one_filler()
                    if r is None:
                        break
                    rows += r
                # one delayed attention thunk
                if pending:
                    r, f = pending.popleft()
                    f()
                    rows += r
                # paced QKV
                tgt = -(-state["qkv_total"] * state["beat"] // state["beats_total"])
                while state["qkv_done"] < tgt and qkv_q:
                    r, f = qkv_q.popleft()
                    f()
                    state["qkv_done"] += 1
                    rows += r
                # top up with deferred tails/projections (paced)
                while rows < ROW_TARGET:
                    ptgt = -(-state["proj_total"] * state["beat"]
                             // state["beats_total"])
                    if not (state["allow_proj"] and state["proj_done"] < ptgt):
                        break
                    if late_q:
                        r, f = late_q.popleft()
                    elif len(proj_q) > state["proj_floor"]:
                        r, f = proj_q.popleft()
                    else:
                        break
                    f()
                    state["proj_done"] += 1
                    rows += r
                # bound the delayed-thunk backlog (shallow blocks append
                # faster than one-per-beat pops)
                while len(pending) > 5:
                    pop_one_filler()
                    r, f = pending.popleft()
                    f()

            def drain_pending(keep=2):
                while len(pending) > keep:
                    r, f = pending.popleft()
                    f()
                    pop_one_filler()

            tt_tiles = {}

            def attn_pair_beat(j, h, ip):
                """Emit one beat: scores pair + exp + mask, then delayed work
                and fillers; queue this pair's ctx matmuls."""
                p, s = divmod(h, 2)
                q_tile = q_tiles[(j, p)]
                hs = slice(64 * s, 64 * s + 64)
                tp = (64 * s, 0)
                i0, i1 = 2 * ip, 2 * ip + 1
                o0, o1 = i0 - 4 * j, i1 - 4 * j
                cs0 = max(0, 128 * o0)
                cs1 = max(0, 128 * o1)
                if ip == 2 * (j + 1) - 1:
                    # first (diagonal-most) pair of the head: allocate the
                    # ctx^T accumulator bank
                    tt_tiles[("c", j, h)] = pctx.tile([128, 260], F32, tag="ctxa",
                                                      name=f"ctxa{j}_{h}")
                st = pst.tile([128, 1024], F32, tag="st", name=f"st{j}_{h}_{ip}")
                nc.tensor.matmul(st[:, cs0:512],
                                 kt_t[hs, p, 128 * i0:128 * (i0 + 1)],
                                 q_tile[hs, cs0:512],
                                 start=True, stop=True, tile_position=tp)
                nc.tensor.matmul(st[:, 512 + cs1:1024],
                                 kt_t[hs, p, 128 * i1:128 * (i1 + 1)],
                                 q_tile[hs, cs1:512],
                                 start=True, stop=True, tile_position=tp)
                e2 = ep.tile([128, 1024], BF16, tag="e", name=f"e{j}_{h}_{ip}")
                if o1 < 0:
                    nc.scalar.activation(e2[:], st[:], Act.Exp)
                else:
                    nc.scalar.activation(e2[:, cs0:512], st[:, cs0:512], Act.Exp)
                    nc.scalar.activation(e2[:, 512 + cs1:1024],
                                         st[:, 512 + cs1:1024], Act.Exp)
                    for (oo, base) in ((o0, 0), (o1, 512)):
                        if 0 <= oo:
                            z = slice(base + 128 * oo, base + 128 * (oo + 1))
                            nc.gpsimd.tensor_tensor(e2[:, z], e2[:, z], m1[:],
                                                    op=Alu.mult)
                rows = (512 - cs0) + (512 - cs1)
                return rows, make_ctx(j, h, e2, i0, i1, o0, o1)

            def make_ctx(j, h, e2, i0, i1, o0, o1):
                nctx = (4 - max(0, o1)) + (4 - max(0, o0))

                def ctx_f(j=j, h=h, e2=e2, i0=i0, i1=i1, o0=o0, o1=o1):
                    # the whole [128, 260] accumulator is ONE hardware
                    # accumulation group: start=True clears the full PSUM
                    # bank, so only the head's first matmul (diag ktile,
                    # subtile 3) starts; every other subtile accumulates
                    # onto the cleared bank
                    ctxa = tt_tiles[("c", j, h)]
                    for (i, base, o) in ((i1, 512, o1), (i0, 0, o0)):
                        for qq in range(max(0, o), 4):
                            nc.tensor.matmul(
                                ctxa[:, 65 * qq:65 * (qq + 1)],
                                e2[:, base + 128 * qq:base + 128 * (qq + 1)],
                                va_t[:, i, h, :],
                                start=(i == 4 * j + 3 and qq == 3),
                                stop=(i == 0 and qq == 3),
                                skip_group_check=True)
                return (65 * nctx, ctx_f)

            la_e2 = {}

            def emit_la(jn, h):
                """Lookahead: scores+exp for (block jn, head h, k-tiles 0-1)
                emitted a block early, while the ACT engine is otherwise
                idle; e2 parks in SBUF until block jn's ctx matmuls."""
                p, s = divmod(h, 2)
                q_tile = q_tiles[(jn, p)]
                hs = slice(64 * s, 64 * s + 64)
                tp = (64 * s, 0)
                st = pst.tile([128, 1024], F32, tag="st", name=f"lst{jn}_{h}")
                nc.tensor.matmul(st[:, 0:512], kt_t[hs, p, 0:128],
                                 q_tile[hs, :], start=True, stop=True,
                                 tile_position=tp)
                nc.tensor.matmul(st[:, 512:1024], kt_t[hs, p, 128:256],
                                 q_tile[hs, :], start=True, stop=True,
                                 tile_position=tp)
                e2 = elp.tile([128, 1024], BF16, tag="ela", name=f"le{jn}_{h}")
                nc.scalar.activation(e2[:], st[:], Act.Exp)
                la_e2[(jn, h)] = e2
                return 1024

            def tail_a(j, h):
                def f(j=j, h=h):
                    ctxa = tt_tiles[("c", j, h)]
                    # ONE plain copy out of PSUM, so the single ctx^T
                    # accumulator bank frees for the next head after ~400ns;
                    # reciprocal + scaling run from the SBUF copy (2x DVE)
                    raw = ctsp.tile([128, 260], BF16, tag="raw", bufs=2,
                                    name=f"raw{j}_{h}")
                    nc.vector.tensor_copy(raw[:], ctxa[:])
                    rc = rcq.tile([128, 4], F32, tag="rc", name=f"rc{j}_{h}")
                    # blocks 0-1 defer their transposes into blocks 2-3, so
                    # up to 16 cts tiles stay live
                    cts = ctsp.tile([128, 256], BF16, tag="cts", bufs=18,
                                    name=f"cts{j}_{h}")
                    tt_tiles[("s", j, h)] = cts
                    raw4 = raw[:].rearrange("p (q c) -> p q c", q=4)
                    nc.vector.reciprocal(rc[:], raw4[:, :, 64:65].squeeze(2))
                    for qq in range(4):
                        nc.vector.tensor_scalar_mul(
                            cts[:, 64 * qq:64 * (qq + 1)],
                            raw[:, 65 * qq:65 * qq + 64],
                            rc[:, qq:qq + 1])
                return (0, f)

            def tail_b(j, h):
                def f(j=j, h=h):
                    p, s = divmod(h, 2)
                    if s == 0:
                        tt_tiles[("t", j, p)] = ptt.tile([128, 512], BF16, tag="tt",
                                                      name=f"tt{j}_{p}")
                    tt = tt_tiles[("t", j, p)]
                    cts = tt_tiles[("s", j, h)]
                    for qq in range(4):
                        nc.tensor.transpose(
                            tt[64 * s:64 * s + 64, 128 * qq:128 * (qq + 1)],
                            cts[:, 64 * qq:64 * (qq + 1)], idn[:])
                return (512, f)

            def tail_c(j, p):
                def f(j=j, p=p):
                    ctx_pair = cxp.tile([128, 512], BF16, tag="ctx", name=f"ctx{j}_{p}")
                    ctx_tiles[(j, p)] = ctx_pair
                    nc.vector.tensor_copy(ctx_pair[:], tt_tiles[("t", j, p)][:])
                    if p == 3:
                        proj_q.extend(gen_proj(j))
                return (0, f)

            # ---------------- prologue: QKV for block 0 ----------------
            # Q runs kk-major across 4 interleaved accumulation chains (2
            # paux slots + 2 borrowed score slots) so each arriving x-chunk
            # DMA feeds 4 matmuls — the chain-major order would stall on the
            # serialized x0 chunk issues
            psq, qts = [], []
            for p in range(4):
                if p < 2:
                    ps = paux.tile([128, 512], F32, tag="aux", name=f"psq0_{p}")[:]
                else:
                    ps = pst.tile([128, 1024], F32, tag="st", name=f"psq0_{p}")[:, 0:512]
                psq.append(ps)
                qt = qtp.tile([128, 512], BF16, tag="qt", name=f"qt0_{p}")
                q_tiles[(0, p)] = qt
                qts.append(qt)
            for kk in range(8):
                for p in range(4):
                    nc.tensor.matmul(psq[p], wq_t[:, kk, 128 * p:128 * (p + 1)],
                                     x_t0[:, kk, :], start=(kk == 0), stop=(kk == 7))
            for p in range(4):
                nc.vector.tensor_copy(qts[p][:], psq[p])
            for r, f in gen_kv(0):
                f()

            # ---------------- main loop ----------------
            for j in range(NQB):
                qkv_q.clear()
                if j + 1 < NQB:
                    load_x(j + 1)
                    qkv_q.extend(gen_q(j + 1))
                    qkv_q.extend(gen_kv(j + 1))
                npair = 2 * (j + 1)
                state["qkv_total"] = len(qkv_q)
                state["qkv_done"] = 0
                state["beats_total"] = 8 * npair + (8 if j < 3 else 0)
                state["beat"] = 0
                state["allow_proj"] = (j >= 2)
                state["prefill_proj"] = (j == NQB - 1)
                state["proj_floor"] = 8
                state["proj_done"] = 0
                state["proj_total"] = len(proj_q) + len(late_q)
                for h in range(HL):
                    first = True
                    for ip in reversed(range(npair)):
                        state["beat"] += 1
                        if ip == 0 and (j, h) in la_e2:
                            # this pair's scores+exp were precomputed a
                            # block early — only the ctx matmuls remain
                            rows = 0
                            ctx_thunk = make_ctx(j, h, la_e2.pop((j, h)),
                                                 0, 1, -4 * j, 1 - 4 * j)
                        else:
                            rows, ctx_thunk = attn_pair_beat(j, h, ip)
                        if first:
                            # head boundary: clear old ctx thunks behind the
                            # freshly issued scores+exp, but leave the
                            # previous head's tail chain (A/B/C) to spread
                            # over the next beats — B stalls on A's DVE work
                            # if popped in the same beat
                            drain_pending(keep=5 if len(pending) <= 6 else 6)
                            first = False
                        run_beat_tail(rows)
                        pending.append(ctx_thunk)
                    if j < 3:
                        # lookahead beat: next block's far pair for this head
                        state["beat"] += 1
                        run_beat_tail(emit_la(j + 1, h))
                    pending.append(tail_a(j, h))
                    if h % 2 == 1:
                        # transpose + assemble as one atomic per-pair entry
                        # (the single tt PSUM slot must not interleave two
                        # pairs); blocks 0-1 defer theirs into blocks 2-3,
                        # where exp latency otherwise starves the PE
                        ra, fa = tail_b(j, h - 1)
                        rb, fb = tail_b(j, h)
                        rc_, fc = tail_c(j, h // 2)

                        def bc(fa=fa, fb=fb, fc=fc):
                            fa()
                            fb()
                            fc()
                        entry = (ra + rb + rc_, bc)
                        if j < 2:
                            late_q.append(entry)
                        else:
                            pending.append(entry)
                # block end: QKV for next block must be complete
                while qkv_q:
                    r, f = qkv_q.popleft()
                    f()

            # ---------------- epilogue ----------------
            state["allow_proj"] = True
            state["prefill_proj"] = True
            state["proj_floor"] = 0
            drain_pending(keep=0)
            while late_q:
                r, f = late_q.popleft()
                f()
            while proj_q:
                r, f = proj_q.popleft()
                f()
    return nc


def _split_waits(nc, limit=1):
    """This walrus build accepts only one sync wait per TPB_CTRL instruction;
    move excess waits onto preceding same-engine NOPs."""
    import concourse.mybir as mybir
    for f in nc.m.functions:
        for bb in f.blocks:
            new_insts = []
            for inst in bb.instructions:
                si = inst.sync_info
                if si is not None and si.on_wait and len(si.on_wait) > limit:
                    waits = list(si.on_wait)
                    k = 0
                    while len(waits) - k > limit:
                        chunk = waits[k:k + limit]
                        k += limit
                        nop = mybir.InstNoOp(name=f"{inst.name}_ws{k}")
                        nop.engine = inst.engine
                        nop.sync_info = mybir.SyncInfo(on_wait=chunk, on_update=[])
                        new_insts.append(nop)
                    si.on_wait = waits[k:]
                new_insts.append(inst)
            bb.instructions = new_insts


# --------------------------------------------------------------------------
# compile + SPMD execution via PJRT (axon) — jit once, reuse
# --------------------------------------------------------------------------
class _Compiled:
    def __init__(self, n_cores=8):
        import jax
        from jax.sharding import Mesh, PartitionSpec
        from jax.experimental.shard_map import shard_map
        import concourse.mybir as mybir
        from concourse.bass2jax import (_bass_exec_p, install_neuronx_cc_hook,
                                        partition_id_tensor)

        nc = _build_bass()
        _split_waits(nc)
        install_neuronx_cc_hook()
        partition_name = nc.partition_id_tensor.name if nc.partition_id_tensor else None
        in_names, out_names, out_avals, zero_outs = [], [], [], []
        for alloc in nc.m.functions[0].allocations:
            if not isinstance(alloc, mybir.MemoryLocationSet):
                continue
            name = alloc.memorylocations[0].name
            if alloc.kind == "ExternalInput":
                if name != partition_name:
                    in_names.append(name)
            elif alloc.kind == "ExternalOutput":
                shape = tuple(alloc.tensor_shape)
                dtype = mybir.dt.np(alloc.dtype)
                out_names.append(name)
                out_avals.append(jax.core.ShapedArray(shape, dtype))
                zero_outs.append(np.zeros(shape, dtype))
        n_params = len(in_names)
        all_in_names = list(in_names) + list(out_names)
        if partition_name is not None:
            all_in_names.append(partition_name)

        def _body(*args):
            operands = list(args)
            if partition_name is not None:
                operands.append(partition_id_tensor())
            outs = _bass_exec_p.bind(
                *operands,
                out_avals=tuple(out_avals),
                in_names=tuple(all_in_names),
                out_names=tuple(out_names),
                lowering_input_output_aliases=(),
                sim_require_finite=True,
                sim_require_nnan=True,
                nc=nc,
            )
            return tuple(outs)

        devices = jax.devices()[:n_cores]
        assert len(devices) >= n_cores, f"need {n_cores} cores, have {len(devices)}"
        self.n_cores = n_cores
        self.in_names, self.out_names = in_names, out_names
        self.out_avals, self.zero_outs = out_avals, zero_outs
        mesh = Mesh(np.asarray(devices[:n_cores]), ("core",))
        in_specs = (PartitionSpec("core"),) * (n_params + len(out_names))
        out_specs = (PartitionSpec("core"),) * len(out_names)
        self.fn = jax.jit(
            shard_map(_body, mesh=mesh, in_specs=in_specs,
                      out_specs=out_specs, check_rep=False),
            keep_unused=True)

    def run(self, in_maps):
        import jax
        args = []
        for name in self.in_names:
            args.append(np.concatenate([np.asarray(m[name]) for m in in_maps], axis=0))
        for z in self.zero_outs:
            args.append(np.zeros((self.n_cores * z.shape[0], *z.shape[1:]), z.dtype))
        if not getattr(self, "_warm", False):
            # the very first execution after device bring-up can read
            # uninitialized PSUM; do one discarded warm-up pass
            jax.block_until_ready(self.fn(*args))
            self._warm = True
        outs = self.fn(*args)
        jax.block_until_ready(outs)
        res = []
        for c in range(self.n_cores):
            d = {}
            for i, name in enumerate(self.out_names):
                a = np.asarray(outs[i]).reshape(self.n_cores, *self.out_avals[i].shape)[c]
                d[name] = a
            res.append(d)
        return res


# --------------------------------------------------------------------------
# host-side shard / unshard
# --------------------------------------------------------------------------
def _bf16(a):
    import ml_dtypes
    return np.ascontiguousarray(a).astype(ml_dtypes.bfloat16)


def _make_core_inputs(x, Wq, Wk, Wv, Wp, core):
    g = core % 2
    b = core // 2
    rows = slice(512 * g, 512 * (g + 1))
    kl = np.arange(128)
    return {
        "xt": _bf16(x[b].T),
        # fold the 1/sqrt(head_dim) score scale into Wq
        "wq": _bf16(Wq[rows, :].T * 0.125),
        "wk": _bf16(Wk[rows, :].T),
        "wv": _bf16(Wv[rows, :].T),
        "wp": _bf16(Wp[:, rows].T),
        "mask1": _bf16((kl[:, None] <= kl[None, :]).astype(np.float32)),
        "ident": _bf16(np.eye(128, dtype=np.float32)),
        "vones": _bf16(np.ones((128, NKT * HL), np.float32)),
    }


def kernel(x, Wq, Wk, Wv, Wp):
    """Full-input / full-output causal MHA. x: (4, 2048, 1024) fp32;
    Wq/Wk/Wv/Wp: (1024, 1024) fp32. Returns (4, 2048, 1024) fp32."""
    global _COMPILED
    x = np.asarray(x, dtype=np.float32)
    Wq = np.asarray(Wq, dtype=np.float32)
    Wk = np.asarray(Wk, dtype=np.float32)
    Wv = np.asarray(Wv, dtype=np.float32)
    Wp = np.asarray(Wp, dtype=np.float32)
    assert x.shape == (B, T, D), x.shape

    if _COMPILED is None:
        _COMPILED = _Compiled(8)
    in_maps = [_make_core_inputs(x, Wq, Wk, Wv, Wp, c) for c in range(8)]
    results = _COMPILED.run(in_maps)

    out = np.empty((B, T, D), np.float32)
    for b in range(B):
        acc = results[2 * b]["outt"] + results[2 * b + 1]["outt"]
        out[b] = acc.T
    return out


# revision 7
# speedup vs baseline: 1.0221x; 1.0055x over previous
"""Causal multi-head attention (B=4, T=2048, D=1024, H=16) on 8 Trainium2 cores.

Sharding (data + tensor parallel): core c handles batch b = c//2 and head-group
g = c%2 (8 of the 16 heads). Wq/Wk/Wv are column-sharded by head, Wp is
row-sharded; the two per-batch partial outputs are summed on the host (this
replaces the device all-reduce — the host-side sum is the unshard step).

v2 restructure vs the 293us baseline (the cost model charges matmuls by output
free-dim rows only; PE re-ramps to half speed after every idle gap):
  - all matmul operands bf16 (same 1 cyc/row as f32r, but exact causal
    narrowing is allowed — no >=256-wide f32r constraint — and DMA/SBUF halve)
  - scores St [kpos, q] per k-tile pair, exactly causal-narrowed
  - attention*V flipped: stationary = E-tile [128 kpos, 128 q] (slice of the
    already-transposed e2), moving = V-aug [128 kpos, 65] -> ctx^T [128 q, 65]
    costs 65 rows/tile instead of 128; the softmax denominator rides along as
    column 64 via an all-ones column in V-aug
  - normalization on DVE: per-partition reciprocal + tensor_scalar multiply
    while copying ctx^T out of PSUM (q is the partition axis there), then PE
    transposes back to hd-major [64, 512] per head
  - software-pipelined emission: every "beat" issues one score pair + exp,
    then runs one delayed thunk (the previous pair's ctx matmuls, or head-tail
    work), then filler matmuls (next-block QKV paced evenly; output
    projections deferred into block 3 where exp latency would otherwise
    starve the PE)
"""
import collections
import numpy as np

T = 2048
D = 1024
B = 4
H = 16
HL = 8            # heads per core
NP = 4            # head pairs per core
QB = 512          # q-block width
NQB = T // QB     # 4 q-blocks
NKT = T // 128    # 16 k-tiles

_COMPILED = None


# --------------------------------------------------------------------------
# bass kernel build
# --------------------------------------------------------------------------
def _build_bass():
    import concourse.bass as bass
    import concourse.mybir as mybir
    from concourse.tile import TileContext

    F32 = mybir.dt.float32
    BF16 = mybir.dt.bfloat16
    Act = mybir.ActivationFunctionType
    Alu = mybir.AluOpType

    nc = bass.Bass()
    xt = nc.dram_tensor("xt", [D, T], BF16, kind="ExternalInput")
    wq = nc.dram_tensor("wq", [D, 512], BF16, kind="ExternalInput")
    wk = nc.dram_tensor("wk", [D, 512], BF16, kind="ExternalInput")
    wv = nc.dram_tensor("wv", [D, 512], BF16, kind="ExternalInput")
    wp = nc.dram_tensor("wp", [512, D], BF16, kind="ExternalInput")
    mask1 = nc.dram_tensor("mask1", [128, 128], BF16, kind="ExternalInput")
    ident = nc.dram_tensor("ident", [128, 128], BF16, kind="ExternalInput")
    vones = nc.dram_tensor("vones", [128, NKT * HL], BF16, kind="ExternalInput")
    outt = nc.dram_tensor("outt", [D, T], F32, kind="ExternalOutput")

    with TileContext(nc) as tc, nc.allow_low_precision(reason="bf16 pipeline"):
        with tc.tile_pool(name="wts", bufs=1) as wts, \
             tc.tile_pool(name="xp", bufs=2) as xp, \
             tc.tile_pool(name="big", bufs=1) as big, \
             tc.tile_pool(name="qtp", bufs=8) as qtp, \
             tc.tile_pool(name="ep", bufs=8) as ep, \
             tc.tile_pool(name="elp", bufs=18) as elp, \
             tc.tile_pool(name="ctsp", bufs=2) as ctsp, \
             tc.tile_pool(name="rcq", bufs=2) as rcq, \
             tc.tile_pool(name="cxp", bufs=16) as cxp, \
             tc.tile_pool(name="sm", bufs=1) as sm, \
             tc.tile_pool(name="osb", bufs=8) as osb, \
             tc.tile_pool(name="pp3", bufs=8) as pp3, \
             tc.tile_pool(name="pst", bufs=2, space="PSUM") as pst, \
             tc.tile_pool(name="pctx", bufs=1, space="PSUM") as pctx, \
             tc.tile_pool(name="ptt", bufs=1, space="PSUM") as ptt, \
             tc.tile_pool(name="paux", bufs=2, space="PSUM") as paux:

            # ---------------- weights/constants ----------------
            wq_t = wts.tile([128, 8, 512], BF16, tag="wq")
            wk_t = wts.tile([128, 8, 512], BF16, tag="wk")
            wv_t = wts.tile([128, 8, 512], BF16, tag="wv")
            wp_t = wts.tile([128, 4, 1024], BF16, tag="wp")
            wqr = wq[:].rearrange("(n p) m -> p n m", p=128)
            wkr = wk[:].rearrange("(n p) m -> p n m", p=128)
            wvr = wv[:].rearrange("(n p) m -> p n m", p=128)
            x_tiles = {}

            def load_x(tb):
                # 2-slice chunks on the SP queue: 4 issues instead of 8
                x_t = xp.tile([128, 8, 512], BF16, tag="x", name=f"x_t{tb}")
                xr = xt[:, QB * tb:QB * (tb + 1)].rearrange("(n p) m -> p n m", p=128)
                for c in range(4):
                    nc.sync.dma_start(x_t[:, 2 * c:2 * c + 2, :], xr[:, 2 * c:2 * c + 2, :])
                x_tiles[tb] = x_t

            # DMA issue is serialized per DGE queue (~600ns each), so spread
            # the prologue loads across the two HWDGE queues (SP + Act):
            #   SP:  wq, x0 (critical path for the first Q chains), then x1..
            #   Act: wk, wv, wp, constants (needed a few us later)
            x_t0 = xp.tile([128, 8, 512], BF16, tag="x", name="x_t0")
            xr0 = xt[:, 0:QB].rearrange("(n p) m -> p n m", p=128)
            for c in range(4):
                nc.sync.dma_start(wq_t[:, 2 * c:2 * c + 2, :], wqr[:, 2 * c:2 * c + 2, :])
                nc.sync.dma_start(x_t0[:, 2 * c:2 * c + 2, :], xr0[:, 2 * c:2 * c + 2, :])
            x_tiles[0] = x_t0
            for c in range(4):
                nc.scalar.dma_start(wk_t[:, 2 * c:2 * c + 2, :], wkr[:, 2 * c:2 * c + 2, :])
            for c in range(4):
                nc.scalar.dma_start(wv_t[:, 2 * c:2 * c + 2, :], wvr[:, 2 * c:2 * c + 2, :])
            m1 = sm.tile([128, 128], BF16, tag="m1")
            nc.scalar.dma_start(m1[:], mask1[:])
            idn = sm.tile([128, 128], BF16, tag="idn")
            nc.scalar.dma_start(idn[:], ident[:])

            kt_t = big.tile([128, 4, T], BF16, tag="kt")
            va_t = big.tile([128, NKT, HL, 65], BF16, tag="va")
            nc.scalar.dma_start(
                va_t[:, :, :, 64:65].squeeze(3),
                vones[:].rearrange("p (n h) -> p n h", n=NKT))
            wpr = wp[:].rearrange("(n p) m -> p n m", p=128)
            for c in range(2):
                nc.scalar.dma_start(wp_t[:, 2 * c:2 * c + 2, :], wpr[:, 2 * c:2 * c + 2, :])

            q_tiles = {}     # (j, p) -> sbuf tile [128, 512]
            ctx_tiles = {}   # (j, p) -> sbuf tile [128, 512] (normalized pair)

            # ---------------- thunk plumbing ----------------
            # a thunk is (rows, fn); rows = PE rows it will emit (for pacing)
            pending = collections.deque()   # attention work delayed >=1 beat
            qkv_q = collections.deque()     # next-block QKV (must finish)
            proj_q = collections.deque()    # deferred output projections
            late_q = collections.deque()    # deferred head tails (blocks 0-1)

            def gen_q(j):
                x_t = x_tiles[j]
                out = []
                for p in range(4):
                    box = {}
                    for kk in range(8):
                        def f(p=p, kk=kk, box=box, x_t=x_t, j=j):
                            if kk == 0:
                                box["ps"] = paux.tile([128, 512], F32, tag="aux",
                                                      name=f"psq{j}_{p}")
                                box["q"] = qtp.tile([128, 512], BF16, tag="qt",
                                                    name=f"qt{j}_{p}")
                                q_tiles[(j, p)] = box["q"]
                            nc.tensor.matmul(box["ps"][:],
                                             wq_t[:, kk, 128 * p:128 * (p + 1)],
                                             x_t[:, kk, :], start=(kk == 0), stop=(kk == 7))
                            if kk == 7:
                                nc.vector.tensor_copy(box["q"][:], box["ps"][:])
                        out.append((512, f))
                return out

            def gen_kv(j):
                x_t = x_tiles[j]
                out = []
                for p in range(4):
                    box = {}
                    for kk in range(8):
                        def f(p=p, kk=kk, box=box, x_t=x_t, j=j):
                            if kk == 0:
                                box["ps"] = paux.tile([128, 512], F32, tag="aux",
                                                      name=f"psk{j}_{p}")
                            nc.tensor.matmul(box["ps"][:],
                                             wk_t[:, kk, 128 * p:128 * (p + 1)],
                                             x_t[:, kk, :], start=(kk == 0), stop=(kk == 7))
                            if kk == 7:
                                nc.vector.tensor_copy(
                                    kt_t[:, p, QB * j:QB * (j + 1)], box["ps"][:])
                        out.append((512, f))
                for tt in range(4):
                    box = {}
                    for kk in range(8):
                        def f(tt=tt, kk=kk, box=box, x_t=x_t, j=j):
                            if kk == 0:
                                box["ps"] = paux.tile([128, 512], F32, tag="aux",
                                                      name=f"psv{j}_{tt}")
                            nc.tensor.matmul(box["ps"][:],
                                             x_t[:, kk, 128 * tt:128 * (tt + 1)],
                                             wv_t[:, kk, :], start=(kk == 0), stop=(kk == 7))
                            if kk == 7:
                                nc.vector.tensor_copy(
                                    va_t[:, 4 * j + tt, :, 0:64],
                                    box["ps"][:].rearrange("p (h d) -> p h d", h=HL))
                        out.append((512, f))
                return out

            def proj_steps(j, m, ps, use_pst, dma_eng, box):
                """Thunks for output-projection chain m of block j over the
                p-range `ps`."""
                out = []
                for p in ps:
                    def f(m=m, p=p, box=box, j=j, use_pst=use_pst, dma_eng=dma_eng):
                        if "pf" not in box:
                            if use_pst:
                                t = pst.tile([128, 1024], F32, tag="st",
                                             name=f"pf{j}_{m}")
                                box["pf"] = t[:, 0:512]
                            else:
                                box["pf"] = paux.tile([128, 512], F32, tag="aux",
                                                      name=f"pf{j}_{m}")[:]
                        nc.tensor.matmul(box["pf"],
                                         wp_t[:, p, 128 * m:128 * (m + 1)],
                                         ctx_tiles[(j, p)][:], start=(p == 0), stop=(p == 3))
                        if p == 3:
                            ob = osb.tile([128, 512], F32, tag="ob", name=f"ob{j}_{m}")
                            if use_pst:
                                # split the epilogue copies across DVE and the
                                # (idle) ACT engine — Copy lives in the same
                                # act-table set as Exp, and gpsimd can't read
                                # PSUM
                                nc.scalar.copy(ob[:], box["pf"])
                            else:
                                nc.vector.tensor_copy(ob[:], box["pf"])
                            dma_eng.dma_start(
                                outt[128 * m:128 * (m + 1), QB * j:QB * (j + 1)], ob[:])
                    out.append((512, f))
                return out

            part3 = {}

            def gen_proj3_stage_a():
                """Last block's projections, p=0..2: run during the final two
                heads' beats on transient paux slots; partials park in SBUF
                (bf16) so no PSUM is held across the last pair's completion."""
                out = []
                for m in range(8):
                    box = {}
                    for p in range(3):
                        def f(m=m, p=p, box=box):
                            j = NQB - 1
                            if p == 0:
                                box["pf"] = paux.tile([128, 512], F32, tag="aux",
                                                      name=f"pf3a_{m}")[:]
                            nc.tensor.matmul(box["pf"],
                                             wp_t[:, p, 128 * m:128 * (m + 1)],
                                             ctx_tiles[(j, p)][:],
                                             start=(p == 0), stop=(p == 2))
                            if p == 2:
                                pt = pp3.tile([128, 512], BF16, tag="pp",
                                              name=f"part3_{m}")
                                part3[m] = pt
                                nc.vector.tensor_copy(pt[:], box["pf"])
                        out.append((512, f))
                return out

            def gen_proj3_stage_b():
                """Epilogue: re-inject the parked partial via an identity
                matmul, add the p=3 term, write out."""
                out = []
                for m in range(8):
                    def f(m=m):
                        j = NQB - 1
                        if m % 2 == 1:
                            t = pst.tile([128, 1024], F32, tag="st", name=f"pf3b_{m}")
                            pf = t[:, 0:512]
                        else:
                            pf = paux.tile([128, 512], F32, tag="aux",
                                           name=f"pf3b_{m}")[:]
                        nc.tensor.matmul(pf, idn[:], part3[m][:],
                                         start=True, stop=False)
                        nc.tensor.matmul(pf, wp_t[:, 3, 128 * m:128 * (m + 1)],
                                         ctx_tiles[(j, 3)][:], start=False, stop=True)
                        ob = osb.tile([128, 512], F32, tag="ob", name=f"ob3_{m}")
                        if m % 2 == 1:
                            nc.scalar.copy(ob[:], pf)
                        else:
                            nc.vector.tensor_copy(ob[:], pf)
                        dma_eng = nc.scalar if m % 2 == 1 else nc.sync
                        dma_eng.dma_start(
                            outt[128 * m:128 * (m + 1), QB * j:QB * (j + 1)], ob[:])
                    out.append((1024, f))
                return out

            def gen_proj(j):
                last = (j == NQB - 1)
                out = []
                for m in range(8):
                    out.extend(proj_steps(j, m, range(4), last and m % 2 == 1,
                                          nc.scalar if last and m % 2 == 1 else nc.sync,
                                          {}))
                return out

            # ---------------- per-beat emission ----------------
            ROW_TARGET = 2700   # ~1038ns of exp per beat, in PE rows

            state = {"qkv_done": 0, "qkv_total": 0, "beat": 0, "beats_total": 1,
                     "allow_proj": False, "proj_floor": 0,
                     "proj_done": 0, "proj_total": 0, "prefill_proj": False}

            def pop_one_filler():
                """Emit one independent filler matmul; returns its rows or
                None when nothing is available. Both queues are paced evenly
                over the block's beats so late beats (where exp latency
                dominates) still have cover."""
                tgt = -(-state["qkv_total"] * state["beat"] // state["beats_total"])
                if state["qkv_done"] < tgt and qkv_q:
                    r, f = qkv_q.popleft()
                    f()
                    state["qkv_done"] += 1
                    return r
                ptgt = -(-state["proj_total"] * state["beat"] // state["beats_total"])
                if state["prefill_proj"] and state["proj_done"] < ptgt:
                    if late_q:
                        r, f = late_q.popleft()
                        f()
                        state["proj_done"] += 1
                        return r
                    if len(proj_q) > state["proj_floor"]:
                        r, f = proj_q.popleft()
                        f()
                        state["proj_done"] += 1
                        return r
                return None

            def run_beat_tail(rows):
                # a few independent fillers ahead of the
                # dependency-stalled attention thunk
                for _ in range(3):
                    r = pop_one_filler()
                    if r is None:
                        break
                    rows += r
                # one delayed attention thunk
                if pending:
                    r, f = pending.popleft()
                    f()
                    rows += r
                # paced QKV
                tgt = -(-state["qkv_total"] * state["beat"] // state["beats_total"])
                while state["qkv_done"] < tgt and qkv_q:
                    r, f = qkv_q.popleft()
                    f()
                    state["qkv_done"] += 1
                    rows += r
                # top up with deferred tails/projections (paced)
                while rows < ROW_TARGET:
                    ptgt = -(-state["proj_total"] * state["beat"]
                             // state["beats_total"])
                    if not (state["allow_proj"] and state["proj_done"] < ptgt):
                        break
                    if late_q:
                        r, f = late_q.popleft()
                    elif len(proj_q) > state["proj_floor"]:
                        r, f = proj_q.popleft()
                    else:
                        break
                    f()
                    state["proj_done"] += 1
                    rows += r
                # bound the delayed-thunk backlog (shallow blocks append
                # faster than one-per-beat pops)
                while len(pending) > 5:
                    pop_one_filler()
                    r, f = pending.popleft()
                    f()

            def drain_pending(keep=2):
                while len(pending) > keep:
                    r, f = pending.popleft()
                    f()
                    pop_one_filler()

            tt_tiles = {}

            def attn_pair_beat(j, h, ip):
                """Emit one beat: scores pair + exp + mask, then delayed work
                and fillers; queue this pair's ctx matmuls."""
                p, s = divmod(h, 2)
                q_tile = q_tiles[(j, p)]
                hs = slice(64 * s, 64 * s + 64)
                tp = (64 * s, 0)
                i0, i1 = 2 * ip, 2 * ip + 1
                o0, o1 = i0 - 4 * j, i1 - 4 * j
                cs0 = max(0, 128 * o0)
                cs1 = max(0, 128 * o1)
                if ip == 2 * (j + 1) - 1:
                    # first (diagonal-most) pair of the head: allocate the
                    # ctx^T accumulator bank
                    tt_tiles[("c", j, h)] = pctx.tile([128, 260], F32, tag="ctxa",
                                                      name=f"ctxa{j}_{h}")
                st = pst.tile([128, 1024], F32, tag="st", name=f"st{j}_{h}_{ip}")
                # the i1 half writes LEFT-SHIFTED (columns 512:1024-cs1) so
                # the two causal-narrowed ranges are contiguous at column 512
                # and a single exp op covers the whole pair
                w1hi = 1024 - cs1
                nc.tensor.matmul(st[:, cs0:512],
                                 kt_t[hs, p, 128 * i0:128 * (i0 + 1)],
                                 q_tile[hs, cs0:512],
                                 start=True, stop=True, tile_position=tp)
                nc.tensor.matmul(st[:, 512:w1hi],
                                 kt_t[hs, p, 128 * i1:128 * (i1 + 1)],
                                 q_tile[hs, cs1:512],
                                 start=True, stop=True, tile_position=tp)
                e2 = ep.tile([128, 1024], BF16, tag="e", name=f"e{j}_{h}_{ip}")
                nc.scalar.activation(e2[:, cs0:w1hi], st[:, cs0:w1hi], Act.Exp)
                if o0 >= 0:
                    z = slice(128 * o0, 128 * (o0 + 1))
                    nc.gpsimd.tensor_tensor(e2[:, z], e2[:, z], m1[:],
                                            op=Alu.mult)
                if o1 >= 0:
                    # shifted: the i1 diag zone starts exactly at column 512
                    z = slice(512, 640)
                    nc.gpsimd.tensor_tensor(e2[:, z], e2[:, z], m1[:],
                                            op=Alu.mult)
                rows = (512 - cs0) + (512 - cs1)
                return rows, make_ctx(j, h, e2, i0, i1, o0, o1, cs1)

            def make_ctx(j, h, e2, i0, i1, o0, o1, sh1=0):
                nctx = (4 - max(0, o1)) + (4 - max(0, o0))

                def ctx_f(j=j, h=h, e2=e2, i0=i0, i1=i1, o0=o0, o1=o1, sh1=sh1):
                    # the whole [128, 260] accumulator is ONE hardware
                    # accumulation group: start=True clears the full PSUM
                    # bank, so only the head's first matmul (diag ktile,
                    # subtile 3) starts; every other subtile accumulates
                    # onto the cleared bank
                    ctxa = tt_tiles[("c", j, h)]
                    for (i, base, o) in ((i1, 512 - sh1, o1), (i0, 0, o0)):
                        for qq in range(max(0, o), 4):
                            nc.tensor.matmul(
                                ctxa[:, 65 * qq:65 * (qq + 1)],
                                e2[:, base + 128 * qq:base + 128 * (qq + 1)],
                                va_t[:, i, h, :],
                                start=(i == 4 * j + 3 and qq == 3),
                                stop=(i == 0 and qq == 3),
                                skip_group_check=True)
                return (65 * nctx, ctx_f)

            la_e2 = {}

            def emit_la(jn, h):
                """Lookahead: scores+exp for (block jn, head h, k-tiles 0-1)
                emitted a block early, while the ACT engine is otherwise
                idle; e2 parks in SBUF until block jn's ctx matmuls."""
                p, s = divmod(h, 2)
                q_tile = q_tiles[(jn, p)]
                hs = slice(64 * s, 64 * s + 64)
                tp = (64 * s, 0)
                st = pst.tile([128, 1024], F32, tag="st", name=f"lst{jn}_{h}")
                nc.tensor.matmul(st[:, 0:512], kt_t[hs, p, 0:128],
                                 q_tile[hs, :], start=True, stop=True,
                                 tile_position=tp)
                nc.tensor.matmul(st[:, 512:1024], kt_t[hs, p, 128:256],
                                 q_tile[hs, :], start=True, stop=True,
                                 tile_position=tp)
                e2 = elp.tile([128, 1024], BF16, tag="ela", name=f"le{jn}_{h}")
                nc.scalar.activation(e2[:], st[:], Act.Exp)
                la_e2[(jn, h)] = e2
                return 1024

            def tail_a(j, h):
                def f(j=j, h=h):
                    ctxa = tt_tiles[("c", j, h)]
                    # ONE plain copy out of PSUM, so the single ctx^T
                    # accumulator bank frees for the next head after ~400ns;
                    # reciprocal + scaling run from the SBUF copy (2x DVE)
                    raw = ctsp.tile([128, 260], BF16, tag="raw", bufs=2,
                                    name=f"raw{j}_{h}")
                    nc.vector.tensor_copy(raw[:], ctxa[:])
                    rc = rcq.tile([128, 4], F32, tag="rc", name=f"rc{j}_{h}")
                    # blocks 0-1 defer their transposes into blocks 2-3, so
                    # up to 16 cts tiles stay live
                    cts = ctsp.tile([128, 256], BF16, tag="cts", bufs=18,
                                    name=f"cts{j}_{h}")
                    tt_tiles[("s", j, h)] = cts
                    raw4 = raw[:].rearrange("p (q c) -> p q c", q=4)
                    nc.vector.reciprocal(rc[:], raw4[:, :, 64:65].squeeze(2))
                    for qq in range(4):
                        nc.vector.tensor_scalar_mul(
                            cts[:, 64 * qq:64 * (qq + 1)],
                            raw[:, 65 * qq:65 * qq + 64],
                            rc[:, qq:qq + 1])
                return (0, f)

            def tail_b(j, h):
                def f(j=j, h=h):
                    p, s = divmod(h, 2)
                    if s == 0:
                        tt_tiles[("t", j, p)] = ptt.tile([128, 512], BF16, tag="tt",
                                                      name=f"tt{j}_{p}")
                    tt = tt_tiles[("t", j, p)]
                    cts = tt_tiles[("s", j, h)]
                    for qq in range(4):
                        nc.tensor.transpose(
                            tt[64 * s:64 * s + 64, 128 * qq:128 * (qq + 1)],
                            cts[:, 64 * qq:64 * (qq + 1)], idn[:])
                return (512, f)

            def tail_c(j, p):
                def f(j=j, p=p):
                    ctx_pair = cxp.tile([128, 512], BF16, tag="ctx", name=f"ctx{j}_{p}")
                    ctx_tiles[(j, p)] = ctx_pair
                    nc.vector.tensor_copy(ctx_pair[:], tt_tiles[("t", j, p)][:])
                    if p == 3:
                        proj_q.extend(gen_proj(j))
                return (0, f)

            # ---------------- prologue: QKV for block 0 ----------------
            # Q runs kk-major across 4 interleaved accumulation chains (2
            # paux slots + 2 borrowed score slots) so each arriving x-chunk
            # DMA feeds 4 matmuls — the chain-major order would stall on the
            # serialized x0 chunk issues
            psq, qts = [], []
            for p in range(4):
                if p < 2:
                    ps = paux.tile([128, 512], F32, tag="aux", name=f"psq0_{p}")[:]
                else:
                    ps = pst.tile([128, 1024], F32, tag="st", name=f"psq0_{p}")[:, 0:512]
                psq.append(ps)
                qt = qtp.tile([128, 512], BF16, tag="qt", name=f"qt0_{p}")
                q_tiles[(0, p)] = qt
                qts.append(qt)
            for kk in range(8):
                for p in range(4):
                    nc.tensor.matmul(psq[p], wq_t[:, kk, 128 * p:128 * (p + 1)],
                                     x_t0[:, kk, :], start=(kk == 0), stop=(kk == 7))
            for p in range(4):
                nc.vector.tensor_copy(qts[p][:], psq[p])
            for r, f in gen_kv(0):
                f()

            # ---------------- main loop ----------------
            for j in range(NQB):
                qkv_q.clear()
                if j + 1 < NQB:
                    load_x(j + 1)
                    qkv_q.extend(gen_q(j + 1))
                    qkv_q.extend(gen_kv(j + 1))
                npair = 2 * (j + 1)
                state["qkv_total"] = len(qkv_q)
                state["qkv_done"] = 0
                state["beats_total"] = 8 * npair + (8 if j < 3 else 0)
                state["beat"] = 0
                state["allow_proj"] = (j >= 2)
                state["prefill_proj"] = (j == NQB - 1)
                state["proj_floor"] = 8
                state["proj_done"] = 0
                state["proj_total"] = len(proj_q) + len(late_q)
                for h in range(HL):
                    first = True
                    for ip in reversed(range(npair)):
                        state["beat"] += 1
                        if ip == 0 and (j, h) in la_e2:
                            # this pair's scores+exp were precomputed a
                            # block early — only the ctx matmuls remain
                            rows = 0
                            ctx_thunk = make_ctx(j, h, la_e2.pop((j, h)),
                                                 0, 1, -4 * j, 1 - 4 * j)
                        else:
                            rows, ctx_thunk = attn_pair_beat(j, h, ip)
                        if first:
                            # head boundary: clear old ctx thunks behind the
                            # freshly issued scores+exp, but leave the
                            # previous head's tail chain (A/B/C) to spread
                            # over the next beats — B stalls on A's DVE work
                            # if popped in the same beat
                            drain_pending(keep=5 if len(pending) <= 6 else 6)
                            first = False
                        run_beat_tail(rows)
                        pending.append(ctx_thunk)
                    if j < 3:
                        # lookahead beat: next block's far pair for this head
                        state["beat"] += 1
                        run_beat_tail(emit_la(j + 1, h))
                    pending.append(tail_a(j, h))
                    if h % 2 == 1:
                        # transpose + assemble as one atomic per-pair entry
                        # (the single tt PSUM slot must not interleave two
                        # pairs); blocks 0-1 defer theirs into blocks 2-3,
                        # where exp latency otherwise starves the PE
                        ra, fa = tail_b(j, h - 1)
                        rb, fb = tail_b(j, h)
                        rc_, fc = tail_c(j, h // 2)

                        def bc(fa=fa, fb=fb, fc=fc):
                            fa()
                            fb()
                            fc()
                        entry = (ra + rb + rc_, bc)
                        if j < 2:
                            late_q.append(entry)
                        else:
                            pending.append(entry)
                # block end: QKV for next block must be complete
                while qkv_q:
                    r, f = qkv_q.popleft()
                    f()

            # ---------------- epilogue ----------------
            state["allow_proj"] = True
            state["prefill_proj"] = True
            state["proj_floor"] = 0
            drain_pending(keep=0)
            while late_q:
                r, f = late_q.popleft()
                f()
            while proj_q:
                r, f = proj_q.popleft()
                f()
    return nc


def _split_waits(nc, limit=1):
    """This walrus build accepts only one sync wait per TPB_CTRL instruction;
    move excess waits onto preceding same-engine NOPs."""
    import concourse.mybir as mybir
    for f in nc.m.functions:
        for bb in f.blocks:
            new_insts = []
            for inst in bb.instructions:
                si = inst.sync_info
                if si is not None and si.on_wait and len(si.on_wait) > limit:
                    waits = list(si.on_wait)
                    k = 0
                    while len(waits) - k > limit:
                        chunk = waits[k:k + limit]
                        k += limit
                        nop = mybir.InstNoOp(name=f"{inst.name}_ws{k}")
                        nop.engine = inst.engine
                        nop.sync_info = mybir.SyncInfo(on_wait=chunk, on_update=[])
                        new_insts.append(nop)
                    si.on_wait = waits[k:]
                new_insts.append(inst)
            bb.instructions = new_insts


# --------------------------------------------------------------------------
# compile + SPMD execution via PJRT (axon) — jit once, reuse
# --------------------------------------------------------------------------
class _Compiled:
    def __init__(self, n_cores=8):
        import jax
        from jax.sharding import Mesh, PartitionSpec
        from jax.experimental.shard_map import shard_map
        import concourse.mybir as mybir
        from concourse.bass2jax import (_bass_exec_p, install_neuronx_cc_hook,
                                        partition_id_tensor)

        nc = _build_bass()
        _split_waits(nc)
        install_neuronx_cc_hook()
        partition_name = nc.partition_id_tensor.name if nc.partition_id_tensor else None
        in_names, out_names, out_avals, zero_outs = [], [], [], []
        for alloc in nc.m.functions[0].allocations:
            if not isinstance(alloc, mybir.MemoryLocationSet):
                continue
            name = alloc.memorylocations[0].name
            if alloc.kind == "ExternalInput":
                if name != partition_name:
                    in_names.append(name)
            elif alloc.kind == "ExternalOutput":
                shape = tuple(alloc.tensor_shape)
                dtype = mybir.dt.np(alloc.dtype)
                out_names.append(name)
                out_avals.append(jax.core.ShapedArray(shape, dtype))
                zero_outs.append(np.zeros(shape, dtype))
        n_params = len(in_names)
        all_in_names = list(in_names) + list(out_names)
        if partition_name is not None:
            all_in_names.append(partition_name)

        def _body(*args):
            operands = list(args)
            if partition_name is not None:
                operands.append(partition_id_tensor())
            outs = _bass_exec_p.bind(
                *operands,
                out_avals=tuple(out_avals),
                in_names=tuple(all_in_names),
                out_names=tuple(out_names),
                lowering_input_output_aliases=(),
                sim_require_finite=True,
                sim_require_nnan=True,
                nc=nc,
            )
            return tuple(outs)

        devices = jax.devices()[:n_cores]
        assert len(devices) >= n_cores, f"need {n_cores} cores, have {len(devices)}"
        self.n_cores = n_cores
        self.in_names, self.out_names = in_names, out_names
        self.out_avals, self.zero_outs = out_avals, zero_outs
        mesh = Mesh(np.asarray(devices[:n_cores]), ("core",))
        in_specs = (PartitionSpec("core"),) * (n_params + len(out_names))
        out_specs = (PartitionSpec("core"),) * len(out_names)
        self.fn = jax.jit(
            shard_map(_body, mesh=mesh, in_specs=in_specs,
                      out_specs=out_specs, check_rep=False),
            keep_unused=True)

    def run(self, in_maps):
        import jax
        args = []
        for name in self.in_names:
            args.append(np.concatenate([np.asarray(m[name]) for m in in_maps], axis=0))
        for z in self.zero_outs:
            args.append(np.zeros((self.n_cores * z.shape[0], *z.shape[1:]), z.dtype))
        if not getattr(self, "_warm", False):
            # the very first execution after device bring-up can read
            # uninitialized PSUM; do one discarded warm-up pass
            jax.block_until_ready(self.fn(*args))
            self._warm = True
        outs = self.fn(*args)
        jax.block_until_ready(outs)
        res = []
        for c in range(self.n_cores):
            d = {}
            for i, name in enumerate(self.out_names):
                a = np.asarray(outs[i]).reshape(self.n_cores, *self.out_avals[i].shape)[c]
                d[name] = a
            res.append(d)
        return res


# --------------------------------------------------------------------------
# host-side shard / unshard
# --------------------------------------------------------------------------
def _bf16(a):
    import ml_dtypes
    return np.ascontiguousarray(a).astype(ml_dtypes.bfloat16)


def _make_core_inputs(x, Wq, Wk, Wv, Wp, core):
    g = core % 2
    b = core // 2
    rows = slice(512 * g, 512 * (g + 1))
    kl = np.arange(128)
    return {
        "xt": _bf16(x[b].T),
        # fold the 1/sqrt(head_dim) score scale into Wq
        "wq": _bf16(Wq[rows, :].T * 0.125),
        "wk": _bf16(Wk[rows, :].T),
        "wv": _bf16(Wv[rows, :].T),
        "wp": _bf16(Wp[:, rows].T),
        "mask1": _bf16((kl[:, None] <= kl[None, :]).astype(np.float32)),
        "ident": _bf16(np.eye(128, dtype=np.float32)),
        "vones": _bf16(np.ones((128, NKT * HL), np.float32)),
    }


def kernel(x, Wq, Wk, Wv, Wp):
    """Full-input / full-output causal MHA. x: (4, 2048, 1024) fp32;
    Wq/Wk/Wv/Wp: (1024, 1024) fp32. Returns (4, 2048, 1024) fp32."""
    global _COMPILED
    x = np.asarray(x, dtype=np.float32)
    Wq = np.asarray(Wq, dtype=np.float32)
    Wk = np.asarray(Wk, dtype=np.float32)
    Wv = np.asarray(Wv, dtype=np.float32)
    Wp = np.asarray(Wp, dtype=np.float32)
    assert x.shape == (B, T, D), x.shape

    if _COMPILED is None:
        _COMPILED = _Compiled(8)
    in_maps = [_make_core_inputs(x, Wq, Wk, Wv, Wp, c) for c in range(8)]
    results = _COMPILED.run(in_maps)

    out = np.empty((B, T, D), np.float32)
    for b in range(B):
        acc = results[2 * b]["outt"] + results[2 * b + 1]["outt"]
        out[b] = acc.T
    return out


# revision 8
# speedup vs baseline: 1.0263x; 1.0040x over previous
"""Causal multi-head attention (B=4, T=2048, D=1024, H=16) on 8 Trainium2 cores.

Sharding (data + tensor parallel): core c handles batch b = c//2 and head-group
g = c%2 (8 of the 16 heads). Wq/Wk/Wv are column-sharded by head, Wp is
row-sharded; the two per-batch partial outputs are summed on the host (this
replaces the device all-reduce — the host-side sum is the unshard step).

v2 restructure vs the 293us baseline (the cost model charges matmuls by output
free-dim rows only; PE re-ramps to half speed after every idle gap):
  - all matmul operands bf16 (same 1 cyc/row as f32r, but exact causal
    narrowing is allowed — no >=256-wide f32r constraint — and DMA/SBUF halve)
  - scores St [kpos, q] per k-tile pair, exactly causal-narrowed
  - attention*V flipped: stationary = E-tile [128 kpos, 128 q] (slice of the
    already-transposed e2), moving = V-aug [128 kpos, 65] -> ctx^T [128 q, 65]
    costs 65 rows/tile instead of 128; the softmax denominator rides along as
    column 64 via an all-ones column in V-aug
  - normalization on DVE: per-partition reciprocal + tensor_scalar multiply
    while copying ctx^T out of PSUM (q is the partition axis there), then PE
    transposes back to hd-major [64, 512] per head
  - software-pipelined emission: every "beat" issues one score pair + exp,
    then runs one delayed thunk (the previous pair's ctx matmuls, or head-tail
    work), then filler matmuls (next-block QKV paced evenly; output
    projections deferred into block 3 where exp latency would otherwise
    starve the PE)
"""
import collections
import numpy as np

T = 2048
D = 1024
B = 4
H = 16
HL = 8            # heads per core
NP = 4            # head pairs per core
QB = 512          # q-block width
NQB = T // QB     # 4 q-blocks
NKT = T // 128    # 16 k-tiles

_COMPILED = None


# --------------------------------------------------------------------------
# bass kernel build
# --------------------------------------------------------------------------
def _build_bass():
    import concourse.bass as bass
    import concourse.mybir as mybir
    from concourse.tile import TileContext

    F32 = mybir.dt.float32
    BF16 = mybir.dt.bfloat16
    Act = mybir.ActivationFunctionType
    Alu = mybir.AluOpType

    nc = bass.Bass()
    xt = nc.dram_tensor("xt", [D, T], BF16, kind="ExternalInput")
    wq = nc.dram_tensor("wq", [D, 512], BF16, kind="ExternalInput")
    wk = nc.dram_tensor("wk", [D, 512], BF16, kind="ExternalInput")
    wv = nc.dram_tensor("wv", [D, 512], BF16, kind="ExternalInput")
    wp = nc.dram_tensor("wp", [512, D], BF16, kind="ExternalInput")
    mask1 = nc.dram_tensor("mask1", [128, 128], BF16, kind="ExternalInput")
    ident = nc.dram_tensor("ident", [128, 128], BF16, kind="ExternalInput")
    vones = nc.dram_tensor("vones", [128, NKT * HL], BF16, kind="ExternalInput")
    outt = nc.dram_tensor("outt", [D, T], F32, kind="ExternalOutput")

    with TileContext(nc) as tc, nc.allow_low_precision(reason="bf16 pipeline"):
        with tc.tile_pool(name="wts", bufs=1) as wts, \
             tc.tile_pool(name="xp", bufs=2) as xp, \
             tc.tile_pool(name="big", bufs=1) as big, \
             tc.tile_pool(name="qtp", bufs=8) as qtp, \
             tc.tile_pool(name="ep", bufs=8) as ep, \
             tc.tile_pool(name="elp", bufs=18) as elp, \
             tc.tile_pool(name="ctsp", bufs=2) as ctsp, \
             tc.tile_pool(name="rcq", bufs=2) as rcq, \
             tc.tile_pool(name="cxp", bufs=16) as cxp, \
             tc.tile_pool(name="sm", bufs=1) as sm, \
             tc.tile_pool(name="osb", bufs=8) as osb, \
             tc.tile_pool(name="pp3", bufs=8) as pp3, \
             tc.tile_pool(name="pst", bufs=2, space="PSUM") as pst, \
             tc.tile_pool(name="pctx", bufs=1, space="PSUM") as pctx, \
             tc.tile_pool(name="ptt", bufs=1, space="PSUM") as ptt, \
             tc.tile_pool(name="paux", bufs=2, space="PSUM") as paux:

            # ---------------- weights/constants ----------------
            wq_t = wts.tile([128, 8, 512], BF16, tag="wq")
            wk_t = wts.tile([128, 8, 512], BF16, tag="wk")
            wv_t = wts.tile([128, 8, 512], BF16, tag="wv")
            wp_t = wts.tile([128, 4, 1024], BF16, tag="wp")
            wqr = wq[:].rearrange("(n p) m -> p n m", p=128)
            wkr = wk[:].rearrange("(n p) m -> p n m", p=128)
            wvr = wv[:].rearrange("(n p) m -> p n m", p=128)
            x_tiles = {}

            def load_x(tb):
                # 2-slice chunks on the SP queue: 4 issues instead of 8
                x_t = xp.tile([128, 8, 512], BF16, tag="x", name=f"x_t{tb}")
                xr = xt[:, QB * tb:QB * (tb + 1)].rearrange("(n p) m -> p n m", p=128)
                for c in range(4):
                    nc.sync.dma_start(x_t[:, 2 * c:2 * c + 2, :], xr[:, 2 * c:2 * c + 2, :])
                x_tiles[tb] = x_t

            # DMA issue is serialized per DGE queue (~600ns each), so spread
            # the prologue loads across the two HWDGE queues (SP + Act):
            #   SP:  wq, x0 (critical path for the first Q chains), then x1..
            #   Act: wk, wv, wp, constants (needed a few us later)
            x_t0 = xp.tile([128, 8, 512], BF16, tag="x", name="x_t0")
            xr0 = xt[:, 0:QB].rearrange("(n p) m -> p n m", p=128)
            # wq chunks 2-3 ride the Act queue: they arrive earlier than
            # SP's 5th/6th serialized issues would deliver them
            for c in range(2):
                nc.sync.dma_start(wq_t[:, 2 * c:2 * c + 2, :], wqr[:, 2 * c:2 * c + 2, :])
                nc.sync.dma_start(x_t0[:, 2 * c:2 * c + 2, :], xr0[:, 2 * c:2 * c + 2, :])
            for c in range(2, 4):
                nc.scalar.dma_start(wq_t[:, 2 * c:2 * c + 2, :], wqr[:, 2 * c:2 * c + 2, :])
                nc.sync.dma_start(x_t0[:, 2 * c:2 * c + 2, :], xr0[:, 2 * c:2 * c + 2, :])
            x_tiles[0] = x_t0
            for c in range(4):
                nc.scalar.dma_start(wk_t[:, 2 * c:2 * c + 2, :], wkr[:, 2 * c:2 * c + 2, :])
            for c in range(4):
                nc.scalar.dma_start(wv_t[:, 2 * c:2 * c + 2, :], wvr[:, 2 * c:2 * c + 2, :])
            m1 = sm.tile([128, 128], BF16, tag="m1")
            nc.scalar.dma_start(m1[:], mask1[:])
            idn = sm.tile([128, 128], BF16, tag="idn")
            nc.scalar.dma_start(idn[:], ident[:])

            kt_t = big.tile([128, 4, T], BF16, tag="kt")
            va_t = big.tile([128, NKT, HL, 65], BF16, tag="va")
            nc.scalar.dma_start(
                va_t[:, :, :, 64:65].squeeze(3),
                vones[:].rearrange("p (n h) -> p n h", n=NKT))
            wpr = wp[:].rearrange("(n p) m -> p n m", p=128)
            for c in range(2):
                nc.scalar.dma_start(wp_t[:, 2 * c:2 * c + 2, :], wpr[:, 2 * c:2 * c + 2, :])

            q_tiles = {}     # (j, p) -> sbuf tile [128, 512]
            ctx_tiles = {}   # (j, p) -> sbuf tile [128, 512] (normalized pair)

            # ---------------- thunk plumbing ----------------
            # a thunk is (rows, fn); rows = PE rows it will emit (for pacing)
            pending = collections.deque()   # attention work delayed >=1 beat
            qkv_q = collections.deque()     # next-block QKV (must finish)
            proj_q = collections.deque()    # deferred output projections
            late_q = collections.deque()    # deferred head tails (blocks 0-1)

            def gen_q(j):
                x_t = x_tiles[j]
                out = []
                for p in range(4):
                    box = {}
                    for kk in range(8):
                        def f(p=p, kk=kk, box=box, x_t=x_t, j=j):
                            if kk == 0:
                                box["ps"] = paux.tile([128, 512], F32, tag="aux",
                                                      name=f"psq{j}_{p}")
                                box["q"] = qtp.tile([128, 512], BF16, tag="qt",
                                                    name=f"qt{j}_{p}")
                                q_tiles[(j, p)] = box["q"]
                            nc.tensor.matmul(box["ps"][:],
                                             wq_t[:, kk, 128 * p:128 * (p + 1)],
                                             x_t[:, kk, :], start=(kk == 0), stop=(kk == 7))
                            if kk == 7:
                                nc.vector.tensor_copy(box["q"][:], box["ps"][:])
                        out.append((512, f))
                return out

            def gen_kv(j):
                x_t = x_tiles[j]
                out = []
                for p in range(4):
                    box = {}
                    for kk in range(8):
                        def f(p=p, kk=kk, box=box, x_t=x_t, j=j):
                            if kk == 0:
                                box["ps"] = paux.tile([128, 512], F32, tag="aux",
                                                      name=f"psk{j}_{p}")
                            nc.tensor.matmul(box["ps"][:],
                                             wk_t[:, kk, 128 * p:128 * (p + 1)],
                                             x_t[:, kk, :], start=(kk == 0), stop=(kk == 7))
                            if kk == 7:
                                nc.vector.tensor_copy(
                                    kt_t[:, p, QB * j:QB * (j + 1)], box["ps"][:])
                        out.append((512, f))
                for tt in range(4):
                    box = {}
                    for kk in range(8):
                        def f(tt=tt, kk=kk, box=box, x_t=x_t, j=j):
                            if kk == 0:
                                box["ps"] = paux.tile([128, 512], F32, tag="aux",
                                                      name=f"psv{j}_{tt}")
                            nc.tensor.matmul(box["ps"][:],
                                             x_t[:, kk, 128 * tt:128 * (tt + 1)],
                                             wv_t[:, kk, :], start=(kk == 0), stop=(kk == 7))
                            if kk == 7:
                                nc.vector.tensor_copy(
                                    va_t[:, 4 * j + tt, :, 0:64],
                                    box["ps"][:].rearrange("p (h d) -> p h d", h=HL))
                        out.append((512, f))
                return out

            def proj_steps(j, m, ps, use_pst, dma_eng, box):
                """Thunks for output-projection chain m of block j over the
                p-range `ps`."""
                out = []
                for p in ps:
                    def f(m=m, p=p, box=box, j=j, use_pst=use_pst, dma_eng=dma_eng):
                        if "pf" not in box:
                            if use_pst:
                                t = pst.tile([128, 1024], F32, tag="st",
                                             name=f"pf{j}_{m}")
                                box["pf"] = t[:, 0:512]
                            else:
                                box["pf"] = paux.tile([128, 512], F32, tag="aux",
                                                      name=f"pf{j}_{m}")[:]
                        nc.tensor.matmul(box["pf"],
                                         wp_t[:, p, 128 * m:128 * (m + 1)],
                                         ctx_tiles[(j, p)][:], start=(p == 0), stop=(p == 3))
                        if p == 3:
                            ob = osb.tile([128, 512], F32, tag="ob", name=f"ob{j}_{m}")
                            if use_pst:
                                # split the epilogue copies across DVE and the
                                # (idle) ACT engine — Copy lives in the same
                                # act-table set as Exp, and gpsimd can't read
                                # PSUM
                                nc.scalar.copy(ob[:], box["pf"])
                            else:
                                nc.vector.tensor_copy(ob[:], box["pf"])
                            dma_eng.dma_start(
                                outt[128 * m:128 * (m + 1), QB * j:QB * (j + 1)], ob[:])
                    out.append((512, f))
                return out

            part3 = {}

            def gen_proj3_stage_a():
                """Last block's projections, p=0..2: run during the final two
                heads' beats on transient paux slots; partials park in SBUF
                (bf16) so no PSUM is held across the last pair's completion."""
                out = []
                for m in range(8):
                    box = {}
                    for p in range(3):
                        def f(m=m, p=p, box=box):
                            j = NQB - 1
                            if p == 0:
                                box["pf"] = paux.tile([128, 512], F32, tag="aux",
                                                      name=f"pf3a_{m}")[:]
                            nc.tensor.matmul(box["pf"],
                                             wp_t[:, p, 128 * m:128 * (m + 1)],
                                             ctx_tiles[(j, p)][:],
                                             start=(p == 0), stop=(p == 2))
                            if p == 2:
                                pt = pp3.tile([128, 512], BF16, tag="pp",
                                              name=f"part3_{m}")
                                part3[m] = pt
                                nc.vector.tensor_copy(pt[:], box["pf"])
                        out.append((512, f))
                return out

            def gen_proj3_stage_b():
                """Epilogue: re-inject the parked partial via an identity
                matmul, add the p=3 term, write out."""
                out = []
                for m in range(8):
                    def f(m=m):
                        j = NQB - 1
                        if m % 2 == 1:
                            t = pst.tile([128, 1024], F32, tag="st", name=f"pf3b_{m}")
                            pf = t[:, 0:512]
                        else:
                            pf = paux.tile([128, 512], F32, tag="aux",
                                           name=f"pf3b_{m}")[:]
                        nc.tensor.matmul(pf, idn[:], part3[m][:],
                                         start=True, stop=False)
                        nc.tensor.matmul(pf, wp_t[:, 3, 128 * m:128 * (m + 1)],
                                         ctx_tiles[(j, 3)][:], start=False, stop=True)
                        ob = osb.tile([128, 512], F32, tag="ob", name=f"ob3_{m}")
                        if m % 2 == 1:
                            nc.scalar.copy(ob[:], pf)
                        else:
                            nc.vector.tensor_copy(ob[:], pf)
                        dma_eng = nc.scalar if m % 2 == 1 else nc.sync
                        dma_eng.dma_start(
                            outt[128 * m:128 * (m + 1), QB * j:QB * (j + 1)], ob[:])
                    out.append((1024, f))
                return out

            def gen_proj(j):
                last = (j == NQB - 1)
                out = []
                for m in range(8):
                    out.extend(proj_steps(j, m, range(4), last and m % 2 == 1,
                                          nc.scalar if last and m % 2 == 1 else nc.sync,
                                          {}))
                return out

            # ---------------- per-beat emission ----------------
            ROW_TARGET = 2700   # ~1038ns of exp per beat, in PE rows

            state = {"qkv_done": 0, "qkv_total": 0, "beat": 0, "beats_total": 1,
                     "allow_proj": False, "proj_floor": 0,
                     "proj_done": 0, "proj_total": 0, "prefill_proj": False}

            def pop_one_filler():
                """Emit one independent filler matmul; returns its rows or
                None when nothing is available. Both queues are paced evenly
                over the block's beats so late beats (where exp latency
                dominates) still have cover."""
                tgt = -(-state["qkv_total"] * state["beat"] // state["beats_total"])
                if state["qkv_done"] < tgt and qkv_q:
                    r, f = qkv_q.popleft()
                    f()
                    state["qkv_done"] += 1
                    return r
                ptgt = -(-state["proj_total"] * state["beat"] // state["beats_total"])
                if state["prefill_proj"] and state["proj_done"] < ptgt:
                    if late_q:
                        r, f = late_q.popleft()
                        f()
                        state["proj_done"] += 1
                        return r
                    if len(proj_q) > state["proj_floor"]:
                        r, f = proj_q.popleft()
                        f()
                        state["proj_done"] += 1
                        return r
                return None

            def run_beat_tail(rows):
                # a few independent fillers ahead of the
                # dependency-stalled attention thunk
                for _ in range(3):
                    r = pop_one_filler()
                    if r is None:
                        break
                    rows += r
                # one delayed attention thunk
                if pending:
                    r, f = pending.popleft()
                    f()
                    rows += r
                # paced QKV
                tgt = -(-state["qkv_total"] * state["beat"] // state["beats_total"])
                while state["qkv_done"] < tgt and qkv_q:
                    r, f = qkv_q.popleft()
                    f()
                    state["qkv_done"] += 1
                    rows += r
                # top up with deferred tails/projections (paced)
                while rows < ROW_TARGET:
                    ptgt = -(-state["proj_total"] * state["beat"]
                             // state["beats_total"])
                    if not (state["allow_proj"] and state["proj_done"] < ptgt):
                        break
                    if late_q:
                        r, f = late_q.popleft()
                    elif len(proj_q) > state["proj_floor"]:
                        r, f = proj_q.popleft()
                    else:
                        break
                    f()
                    state["proj_done"] += 1
                    rows += r
                # bound the delayed-thunk backlog (shallow blocks append
                # faster than one-per-beat pops)
                while len(pending) > 5:
                    pop_one_filler()
                    r, f = pending.popleft()
                    f()

            def drain_pending(keep=2):
                while len(pending) > keep:
                    r, f = pending.popleft()
                    f()
                    pop_one_filler()

            tt_tiles = {}

            def attn_pair_beat(j, h, ip):
                """Emit one beat: scores pair + exp + mask, then delayed work
                and fillers; queue this pair's ctx matmuls."""
                p, s = divmod(h, 2)
                q_tile = q_tiles[(j, p)]
                hs = slice(64 * s, 64 * s + 64)
                tp = (64 * s, 0)
                i0, i1 = 2 * ip, 2 * ip + 1
                o0, o1 = i0 - 4 * j, i1 - 4 * j
                cs0 = max(0, 128 * o0)
                cs1 = max(0, 128 * o1)
                if ip == 2 * (j + 1) - 1:
                    # first (diagonal-most) pair of the head: allocate the
                    # ctx^T accumulator bank
                    tt_tiles[("c", j, h)] = pctx.tile([128, 260], F32, tag="ctxa",
                                                      name=f"ctxa{j}_{h}")
                st = pst.tile([128, 1024], F32, tag="st", name=f"st{j}_{h}_{ip}")
                # the i1 half writes LEFT-SHIFTED (columns 512:1024-cs1) so
                # the two causal-narrowed ranges are contiguous at column 512
                # and a single exp op covers the whole pair
                w1hi = 1024 - cs1
                nc.tensor.matmul(st[:, cs0:512],
                                 kt_t[hs, p, 128 * i0:128 * (i0 + 1)],
                                 q_tile[hs, cs0:512],
                                 start=True, stop=True, tile_position=tp)
                nc.tensor.matmul(st[:, 512:w1hi],
                                 kt_t[hs, p, 128 * i1:128 * (i1 + 1)],
                                 q_tile[hs, cs1:512],
                                 start=True, stop=True, tile_position=tp)
                e2 = ep.tile([128, 1024], BF16, tag="e", name=f"e{j}_{h}_{ip}")
                nc.scalar.activation(e2[:, cs0:w1hi], st[:, cs0:w1hi], Act.Exp)
                if o0 >= 0:
                    z = slice(128 * o0, 128 * (o0 + 1))
                    nc.gpsimd.tensor_tensor(e2[:, z], e2[:, z], m1[:],
                                            op=Alu.mult)
                if o1 >= 0:
                    # shifted: the i1 diag zone starts exactly at column 512
                    z = slice(512, 640)
                    nc.gpsimd.tensor_tensor(e2[:, z], e2[:, z], m1[:],
                                            op=Alu.mult)
                rows = (512 - cs0) + (512 - cs1)
                return rows, make_ctx(j, h, e2, i0, i1, o0, o1, cs1)

            def make_ctx(j, h, e2, i0, i1, o0, o1, sh1=0):
                nctx = (4 - max(0, o1)) + (4 - max(0, o0))

                def ctx_f(j=j, h=h, e2=e2, i0=i0, i1=i1, o0=o0, o1=o1, sh1=sh1):
                    # the whole [128, 260] accumulator is ONE hardware
                    # accumulation group: start=True clears the full PSUM
                    # bank, so only the head's first matmul (diag ktile,
                    # subtile 3) starts; every other subtile accumulates
                    # onto the cleared bank
                    ctxa = tt_tiles[("c", j, h)]
                    for (i, base, o) in ((i1, 512 - sh1, o1), (i0, 0, o0)):
                        for qq in range(max(0, o), 4):
                            nc.tensor.matmul(
                                ctxa[:, 65 * qq:65 * (qq + 1)],
                                e2[:, base + 128 * qq:base + 128 * (qq + 1)],
                                va_t[:, i, h, :],
                                start=(i == 4 * j + 3 and qq == 3),
                                stop=(i == 0 and qq == 3),
                                skip_group_check=True)
                return (65 * nctx, ctx_f)

            la_e2 = {}

            def emit_la(jn, h):
                """Lookahead: scores+exp for (block jn, head h, k-tiles 0-1)
                emitted a block early, while the ACT engine is otherwise
                idle; e2 parks in SBUF until block jn's ctx matmuls."""
                p, s = divmod(h, 2)
                q_tile = q_tiles[(jn, p)]
                hs = slice(64 * s, 64 * s + 64)
                tp = (64 * s, 0)
                st = pst.tile([128, 1024], F32, tag="st", name=f"lst{jn}_{h}")
                nc.tensor.matmul(st[:, 0:512], kt_t[hs, p, 0:128],
                                 q_tile[hs, :], start=True, stop=True,
                                 tile_position=tp)
                nc.tensor.matmul(st[:, 512:1024], kt_t[hs, p, 128:256],
                                 q_tile[hs, :], start=True, stop=True,
                                 tile_position=tp)
                e2 = elp.tile([128, 1024], BF16, tag="ela", name=f"le{jn}_{h}")
                nc.scalar.activation(e2[:], st[:], Act.Exp)
                la_e2[(jn, h)] = e2
                return 1024

            def tail_a(j, h):
                def f(j=j, h=h):
                    ctxa = tt_tiles[("c", j, h)]
                    # ONE plain copy out of PSUM, so the single ctx^T
                    # accumulator bank frees for the next head after ~400ns;
                    # reciprocal + scaling run from the SBUF copy (2x DVE)
                    raw = ctsp.tile([128, 260], BF16, tag="raw", bufs=2,
                                    name=f"raw{j}_{h}")
                    nc.vector.tensor_copy(raw[:], ctxa[:])
                    rc = rcq.tile([128, 4], F32, tag="rc", name=f"rc{j}_{h}")
                    # blocks 0-1 defer their transposes into blocks 2-3, so
                    # up to 16 cts tiles stay live
                    cts = ctsp.tile([128, 256], BF16, tag="cts", bufs=18,
                                    name=f"cts{j}_{h}")
                    tt_tiles[("s", j, h)] = cts
                    raw4 = raw[:].rearrange("p (q c) -> p q c", q=4)
                    nc.vector.reciprocal(rc[:], raw4[:, :, 64:65].squeeze(2))
                    for qq in range(4):
                        nc.vector.tensor_scalar_mul(
                            cts[:, 64 * qq:64 * (qq + 1)],
                            raw[:, 65 * qq:65 * qq + 64],
                            rc[:, qq:qq + 1])
                return (0, f)

            def tail_b(j, h):
                def f(j=j, h=h):
                    p, s = divmod(h, 2)
                    if s == 0:
                        tt_tiles[("t", j, p)] = ptt.tile([128, 512], BF16, tag="tt",
                                                      name=f"tt{j}_{p}")
                    tt = tt_tiles[("t", j, p)]
                    cts = tt_tiles[("s", j, h)]
                    for qq in range(4):
                        nc.tensor.transpose(
                            tt[64 * s:64 * s + 64, 128 * qq:128 * (qq + 1)],
                            cts[:, 64 * qq:64 * (qq + 1)], idn[:])
                return (512, f)

            def tail_c(j, p):
                def f(j=j, p=p):
                    ctx_pair = cxp.tile([128, 512], BF16, tag="ctx", name=f"ctx{j}_{p}")
                    ctx_tiles[(j, p)] = ctx_pair
                    nc.vector.tensor_copy(ctx_pair[:], tt_tiles[("t", j, p)][:])
                    if p == 3:
                        proj_q.extend(gen_proj(j))
                return (0, f)

            # ---------------- prologue: QKV for block 0 ----------------
            # Q runs kk-major across 4 interleaved accumulation chains (2
            # paux slots + 2 borrowed score slots) so each arriving x-chunk
            # DMA feeds 4 matmuls — the chain-major order would stall on the
            # serialized x0 chunk issues
            psq, qts = [], []
            for p in range(4):
                if p < 2:
                    ps = paux.tile([128, 512], F32, tag="aux", name=f"psq0_{p}")[:]
                else:
                    ps = pst.tile([128, 1024], F32, tag="st", name=f"psq0_{p}")[:, 0:512]
                psq.append(ps)
                qt = qtp.tile([128, 512], BF16, tag="qt", name=f"qt0_{p}")
                q_tiles[(0, p)] = qt
                qts.append(qt)
            for kk in range(8):
                for p in range(4):
                    nc.tensor.matmul(psq[p], wq_t[:, kk, 128 * p:128 * (p + 1)],
                                     x_t0[:, kk, :], start=(kk == 0), stop=(kk == 7))
            for p in range(4):
                nc.vector.tensor_copy(qts[p][:], psq[p])
            for r, f in gen_kv(0):
                f()

            # ---------------- main loop ----------------
            for j in range(NQB):
                qkv_q.clear()
                if j + 1 < NQB:
                    load_x(j + 1)
                    qkv_q.extend(gen_q(j + 1))
                    qkv_q.extend(gen_kv(j + 1))
                npair = 2 * (j + 1)
                state["qkv_total"] = len(qkv_q)
                state["qkv_done"] = 0
                state["beats_total"] = 8 * npair + (8 if j < 3 else 0)
                state["beat"] = 0
                state["allow_proj"] = (j >= 2)
                state["prefill_proj"] = (j == NQB - 1)
                state["proj_floor"] = 8
                state["proj_done"] = 0
                state["proj_total"] = len(proj_q) + len(late_q)
                for h in range(HL):
                    first = True
                    for ip in reversed(range(npair)):
                        state["beat"] += 1
                        if ip == 0 and (j, h) in la_e2:
                            # this pair's scores+exp were precomputed a
                            # block early — only the ctx matmuls remain
                            rows = 0
                            ctx_thunk = make_ctx(j, h, la_e2.pop((j, h)),
                                                 0, 1, -4 * j, 1 - 4 * j)
                        else:
                            rows, ctx_thunk = attn_pair_beat(j, h, ip)
                        if first:
                            # head boundary: clear old ctx thunks behind the
                            # freshly issued scores+exp, but leave the
                            # previous head's tail chain (A/B/C) to spread
                            # over the next beats — B stalls on A's DVE work
                            # if popped in the same beat
                            drain_pending(keep=5 if len(pending) <= 6 else 6)
                            first = False
                        run_beat_tail(rows)
                        pending.append(ctx_thunk)
                    if j < 3:
                        # lookahead beat: next block's far pair for this head
                        state["beat"] += 1
                        run_beat_tail(emit_la(j + 1, h))
                    pending.append(tail_a(j, h))
                    if h % 2 == 1:
                        # transpose + assemble as one atomic per-pair entry
                        # (the single tt PSUM slot must not interleave two
                        # pairs); blocks 0-1 defer theirs into blocks 2-3,
                        # where exp latency otherwise starves the PE
                        ra, fa = tail_b(j, h - 1)
                        rb, fb = tail_b(j, h)
                        rc_, fc = tail_c(j, h // 2)

                        def bc(fa=fa, fb=fb, fc=fc):
                            fa()
                            fb()
                            fc()
                        entry = (ra + rb + rc_, bc)
                        if j < 2:
                            late_q.append(entry)
                        else:
                            pending.append(entry)
                # block end: QKV for next block must be complete
                while qkv_q:
                    r, f = qkv_q.popleft()
                    f()

            # ---------------- epilogue ----------------
            state["allow_proj"] = True
            state["prefill_proj"] = True
            state["proj_floor"] = 0
            drain_pending(keep=0)
            while late_q:
                r, f = late_q.popleft()
                f()
            while proj_q:
                r, f = proj_q.popleft()
                f()
    return nc


def _split_waits(nc, limit=1):
    """This walrus build accepts only one sync wait per TPB_CTRL instruction;
    move excess waits onto preceding same-engine NOPs."""
    import concourse.mybir as mybir
    for f in nc.m.functions:
        for bb in f.blocks:
            new_insts = []
            for inst in bb.instructions:
                si = inst.sync_info
                if si is not None and si.on_wait and len(si.on_wait) > limit:
                    waits = list(si.on_wait)
                    k = 0
                    while len(waits) - k > limit:
                        chunk = waits[k:k + limit]
                        k += limit
                        nop = mybir.InstNoOp(name=f"{inst.name}_ws{k}")
                        nop.engine = inst.engine
                        nop.sync_info = mybir.SyncInfo(on_wait=chunk, on_update=[])
                        new_insts.append(nop)
                    si.on_wait = waits[k:]
                new_insts.append(inst)
            bb.instructions = new_insts


# --------------------------------------------------------------------------
# compile + SPMD execution via PJRT (axon) — jit once, reuse
# --------------------------------------------------------------------------
class _Compiled:
    def __init__(self, n_cores=8):
        import jax
        from jax.sharding import Mesh, PartitionSpec
        from jax.experimental.shard_map import shard_map
        import concourse.mybir as mybir
        from concourse.bass2jax import (_bass_exec_p, install_neuronx_cc_hook,
                                        partition_id_tensor)

        nc = _build_bass()
        _split_waits(nc)
        install_neuronx_cc_hook()
        partition_name = nc.partition_id_tensor.name if nc.partition_id_tensor else None
        in_names, out_names, out_avals, zero_outs = [], [], [], []
        for alloc in nc.m.functions[0].allocations:
            if not isinstance(alloc, mybir.MemoryLocationSet):
                continue
            name = alloc.memorylocations[0].name
            if alloc.kind == "ExternalInput":
                if name != partition_name:
                    in_names.append(name)
            elif alloc.kind == "ExternalOutput":
                shape = tuple(alloc.tensor_shape)
                dtype = mybir.dt.np(alloc.dtype)
                out_names.append(name)
                out_avals.append(jax.core.ShapedArray(shape, dtype))
                zero_outs.append(np.zeros(shape, dtype))
        n_params = len(in_names)
        all_in_names = list(in_names) + list(out_names)
        if partition_name is not None:
            all_in_names.append(partition_name)

        def _body(*args):
            operands = list(args)
            if partition_name is not None:
                operands.append(partition_id_tensor())
            outs = _bass_exec_p.bind(
                *operands,
                out_avals=tuple(out_avals),
                in_names=tuple(all_in_names),
                out_names=tuple(out_names),
                lowering_input_output_aliases=(),
                sim_require_finite=True,
                sim_require_nnan=True,
                nc=nc,
            )
            return tuple(outs)

        devices = jax.devices()[:n_cores]
        assert len(devices) >= n_cores, f"need {n_cores} cores, have {len(devices)}"
        self.n_cores = n_cores
        self.in_names, self.out_names = in_names, out_names
        self.out_avals, self.zero_outs = out_avals, zero_outs
        mesh = Mesh(np.asarray(devices[:n_cores]), ("core",))
        in_specs = (PartitionSpec("core"),) * (n_params + len(out_names))
        out_specs = (PartitionSpec("core"),) * len(out_names)
        self.fn = jax.jit(
            shard_map(_body, mesh=mesh, in_specs=in_specs,
                      out_specs=out_specs, check_rep=False),
            keep_unused=True)

    def run(self, in_maps):
        import jax
        args = []
        for name in self.in_names:
            args.append(np.concatenate([np.asarray(m[name]) for m in in_maps], axis=0))
        for z in self.zero_outs:
            args.append(np.zeros((self.n_cores * z.shape[0], *z.shape[1:]), z.dtype))
        if not getattr(self, "_warm", False):
            # the very first execution after device bring-up can read
            # uninitialized PSUM; do one discarded warm-up pass
            jax.block_until_ready(self.fn(*args))
            self._warm = True
        outs = self.fn(*args)
        jax.block_until_ready(outs)
        res = []
        for c in range(self.n_cores):
            d = {}
            for i, name in enumerate(self.out_names):
                a = np.asarray(outs[i]).reshape(self.n_cores, *self.out_avals[i].shape)[c]
                d[name] = a
            res.append(d)
        return res


# --------------------------------------------------------------------------
# host-side shard / unshard
# --------------------------------------------------------------------------
def _bf16(a):
    import ml_dtypes
    return np.ascontiguousarray(a).astype(ml_dtypes.bfloat16)


def _make_core_inputs(x, Wq, Wk, Wv, Wp, core):
    g = core % 2
    b = core // 2
    rows = slice(512 * g, 512 * (g + 1))
    kl = np.arange(128)
    return {
        "xt": _bf16(x[b].T),
        # fold the 1/sqrt(head_dim) score scale into Wq
        "wq": _bf16(Wq[rows, :].T * 0.125),
        "wk": _bf16(Wk[rows, :].T),
        "wv": _bf16(Wv[rows, :].T),
        "wp": _bf16(Wp[:, rows].T),
        "mask1": _bf16((kl[:, None] <= kl[None, :]).astype(np.float32)),
        "ident": _bf16(np.eye(128, dtype=np.float32)),
        "vones": _bf16(np.ones((128, NKT * HL), np.float32)),
    }


def kernel(x, Wq, Wk, Wv, Wp):
    """Full-input / full-output causal MHA. x: (4, 2048, 1024) fp32;
    Wq/Wk/Wv/Wp: (1024, 1024) fp32. Returns (4, 2048, 1024) fp32."""
    global _COMPILED
    x = np.asarray(x, dtype=np.float32)
    Wq = np.asarray(Wq, dtype=np.float32)
    Wk = np.asarray(Wk, dtype=np.float32)
    Wv = np.asarray(Wv, dtype=np.float32)
    Wp = np.asarray(Wp, dtype=np.float32)
    assert x.shape == (B, T, D), x.shape

    if _COMPILED is None:
        _COMPILED = _Compiled(8)
    in_maps = [_make_core_inputs(x, Wq, Wk, Wv, Wp, c) for c in range(8)]
    results = _COMPILED.run(in_maps)

    out = np.empty((B, T, D), np.float32)
    for b in range(B):
        acc = results[2 * b]["outt"] + results[2 * b + 1]["outt"]
        out[b] = acc.T
    return out


# revision 9
# speedup vs baseline: 1.0279x; 1.0016x over previous
"""Causal multi-head attention (B=4, T=2048, D=1024, H=16) on 8 Trainium2 cores.

Sharding (data + tensor parallel): core c handles batch b = c//2 and head-group
g = c%2 (8 of the 16 heads). Wq/Wk/Wv are column-sharded by head, Wp is
row-sharded; the two per-batch partial outputs are summed on the host (this
replaces the device all-reduce — the host-side sum is the unshard step).

v2 restructure vs the 293us baseline (the cost model charges matmuls by output
free-dim rows only; PE re-ramps to half speed after every idle gap):
  - all matmul operands bf16 (same 1 cyc/row as f32r, but exact causal
    narrowing is allowed — no >=256-wide f32r constraint — and DMA/SBUF halve)
  - scores St [kpos, q] per k-tile pair, exactly causal-narrowed
  - attention*V flipped: stationary = E-tile [128 kpos, 128 q] (slice of the
    already-transposed e2), moving = V-aug [128 kpos, 65] -> ctx^T [128 q, 65]
    costs 65 rows/tile instead of 128; the softmax denominator rides along as
    column 64 via an all-ones column in V-aug
  - normalization on DVE: per-partition reciprocal + tensor_scalar multiply
    while copying ctx^T out of PSUM (q is the partition axis there), then PE
    transposes back to hd-major [64, 512] per head
  - software-pipelined emission: every "beat" issues one score pair + exp,
    then runs one delayed thunk (the previous pair's ctx matmuls, or head-tail
    work), then filler matmuls (next-block QKV paced evenly; output
    projections deferred into block 3 where exp latency would otherwise
    starve the PE)
"""
import collections
import numpy as np

T = 2048
D = 1024
B = 4
H = 16
HL = 8            # heads per core
NP = 4            # head pairs per core
QB = 512          # q-block width
NQB = T // QB     # 4 q-blocks
NKT = T // 128    # 16 k-tiles

_COMPILED = None


# --------------------------------------------------------------------------
# bass kernel build
# --------------------------------------------------------------------------
def _build_bass():
    import concourse.bass as bass
    import concourse.mybir as mybir
    from concourse.tile import TileContext

    F32 = mybir.dt.float32
    BF16 = mybir.dt.bfloat16
    Act = mybir.ActivationFunctionType
    Alu = mybir.AluOpType

    nc = bass.Bass()
    xt = nc.dram_tensor("xt", [D, T], BF16, kind="ExternalInput")
    wq = nc.dram_tensor("wq", [D, 512], BF16, kind="ExternalInput")
    wk = nc.dram_tensor("wk", [D, 512], BF16, kind="ExternalInput")
    wv = nc.dram_tensor("wv", [D, 512], BF16, kind="ExternalInput")
    wp = nc.dram_tensor("wp", [512, D], BF16, kind="ExternalInput")
    mask1 = nc.dram_tensor("mask1", [128, 128], BF16, kind="ExternalInput")
    ident = nc.dram_tensor("ident", [128, 128], BF16, kind="ExternalInput")
    vones = nc.dram_tensor("vones", [128, NKT * HL], BF16, kind="ExternalInput")
    outt = nc.dram_tensor("outt", [D, T], BF16, kind="ExternalOutput")

    with TileContext(nc) as tc, nc.allow_low_precision(reason="bf16 pipeline"):
        with tc.tile_pool(name="wts", bufs=1) as wts, \
             tc.tile_pool(name="xp", bufs=2) as xp, \
             tc.tile_pool(name="big", bufs=1) as big, \
             tc.tile_pool(name="qtp", bufs=8) as qtp, \
             tc.tile_pool(name="ep", bufs=8) as ep, \
             tc.tile_pool(name="elp", bufs=18) as elp, \
             tc.tile_pool(name="ctsp", bufs=2) as ctsp, \
             tc.tile_pool(name="rcq", bufs=2) as rcq, \
             tc.tile_pool(name="cxp", bufs=16) as cxp, \
             tc.tile_pool(name="sm", bufs=1) as sm, \
             tc.tile_pool(name="osb", bufs=8) as osb, \
             tc.tile_pool(name="pp3", bufs=8) as pp3, \
             tc.tile_pool(name="pst", bufs=2, space="PSUM") as pst, \
             tc.tile_pool(name="pctx", bufs=1, space="PSUM") as pctx, \
             tc.tile_pool(name="ptt", bufs=1, space="PSUM") as ptt, \
             tc.tile_pool(name="paux", bufs=2, space="PSUM") as paux:

            # ---------------- weights/constants ----------------
            wq_t = wts.tile([128, 8, 512], BF16, tag="wq")
            wk_t = wts.tile([128, 8, 512], BF16, tag="wk")
            wv_t = wts.tile([128, 8, 512], BF16, tag="wv")
            wp_t = wts.tile([128, 4, 1024], BF16, tag="wp")
            wqr = wq[:].rearrange("(n p) m -> p n m", p=128)
            wkr = wk[:].rearrange("(n p) m -> p n m", p=128)
            wvr = wv[:].rearrange("(n p) m -> p n m", p=128)
            x_tiles = {}

            def load_x(tb):
                # 2-slice chunks on the SP queue: 4 issues instead of 8
                x_t = xp.tile([128, 8, 512], BF16, tag="x", name=f"x_t{tb}")
                xr = xt[:, QB * tb:QB * (tb + 1)].rearrange("(n p) m -> p n m", p=128)
                for c in range(4):
                    nc.sync.dma_start(x_t[:, 2 * c:2 * c + 2, :], xr[:, 2 * c:2 * c + 2, :])
                x_tiles[tb] = x_t

            # DMA issue is serialized per DGE queue (~600ns each), so spread
            # the prologue loads across the two HWDGE queues (SP + Act):
            #   SP:  wq, x0 (critical path for the first Q chains), then x1..
            #   Act: wk, wv, wp, constants (needed a few us later)
            x_t0 = xp.tile([128, 8, 512], BF16, tag="x", name="x_t0")
            xr0 = xt[:, 0:QB].rearrange("(n p) m -> p n m", p=128)
            # wq chunks 2-3 ride the Act queue: they arrive earlier than
            # SP's 5th/6th serialized issues would deliver them
            for c in range(2):
                nc.sync.dma_start(wq_t[:, 2 * c:2 * c + 2, :], wqr[:, 2 * c:2 * c + 2, :])
                nc.sync.dma_start(x_t0[:, 2 * c:2 * c + 2, :], xr0[:, 2 * c:2 * c + 2, :])
            for c in range(2, 4):
                nc.scalar.dma_start(wq_t[:, 2 * c:2 * c + 2, :], wqr[:, 2 * c:2 * c + 2, :])
                nc.sync.dma_start(x_t0[:, 2 * c:2 * c + 2, :], xr0[:, 2 * c:2 * c + 2, :])
            x_tiles[0] = x_t0
            for c in range(4):
                nc.scalar.dma_start(wk_t[:, 2 * c:2 * c + 2, :], wkr[:, 2 * c:2 * c + 2, :])
            for c in range(4):
                nc.scalar.dma_start(wv_t[:, 2 * c:2 * c + 2, :], wvr[:, 2 * c:2 * c + 2, :])
            m1 = sm.tile([128, 128], BF16, tag="m1")
            nc.scalar.dma_start(m1[:], mask1[:])
            idn = sm.tile([128, 128], BF16, tag="idn")
            nc.scalar.dma_start(idn[:], ident[:])

            kt_t = big.tile([128, 4, T], BF16, tag="kt")
            va_t = big.tile([128, NKT, HL, 65], BF16, tag="va")
            nc.scalar.dma_start(
                va_t[:, :, :, 64:65].squeeze(3),
                vones[:].rearrange("p (n h) -> p n h", n=NKT))
            wpr = wp[:].rearrange("(n p) m -> p n m", p=128)
            for c in range(2):
                nc.scalar.dma_start(wp_t[:, 2 * c:2 * c + 2, :], wpr[:, 2 * c:2 * c + 2, :])

            q_tiles = {}     # (j, p) -> sbuf tile [128, 512]
            ctx_tiles = {}   # (j, p) -> sbuf tile [128, 512] (normalized pair)

            # ---------------- thunk plumbing ----------------
            # a thunk is (rows, fn); rows = PE rows it will emit (for pacing)
            pending = collections.deque()   # attention work delayed >=1 beat
            qkv_q = collections.deque()     # next-block QKV (must finish)
            proj_q = collections.deque()    # deferred output projections
            late_q = collections.deque()    # deferred head tails (blocks 0-1)

            def gen_q(j):
                x_t = x_tiles[j]
                out = []
                for p in range(4):
                    box = {}
                    for kk in range(8):
                        def f(p=p, kk=kk, box=box, x_t=x_t, j=j):
                            if kk == 0:
                                box["ps"] = paux.tile([128, 512], F32, tag="aux",
                                                      name=f"psq{j}_{p}")
                                box["q"] = qtp.tile([128, 512], BF16, tag="qt",
                                                    name=f"qt{j}_{p}")
                                q_tiles[(j, p)] = box["q"]
                            nc.tensor.matmul(box["ps"][:],
                                             wq_t[:, kk, 128 * p:128 * (p + 1)],
                                             x_t[:, kk, :], start=(kk == 0), stop=(kk == 7))
                            if kk == 7:
                                nc.vector.tensor_copy(box["q"][:], box["ps"][:])
                        out.append((512, f))
                return out

            def gen_kv(j):
                x_t = x_tiles[j]
                out = []
                for p in range(4):
                    box = {}
                    for kk in range(8):
                        def f(p=p, kk=kk, box=box, x_t=x_t, j=j):
                            if kk == 0:
                                box["ps"] = paux.tile([128, 512], F32, tag="aux",
                                                      name=f"psk{j}_{p}")
                            nc.tensor.matmul(box["ps"][:],
                                             wk_t[:, kk, 128 * p:128 * (p + 1)],
                                             x_t[:, kk, :], start=(kk == 0), stop=(kk == 7))
                            if kk == 7:
                                nc.vector.tensor_copy(
                                    kt_t[:, p, QB * j:QB * (j + 1)], box["ps"][:])
                        out.append((512, f))
                for tt in range(4):
                    box = {}
                    for kk in range(8):
                        def f(tt=tt, kk=kk, box=box, x_t=x_t, j=j):
                            if kk == 0:
                                box["ps"] = paux.tile([128, 512], F32, tag="aux",
                                                      name=f"psv{j}_{tt}")
                            nc.tensor.matmul(box["ps"][:],
                                             x_t[:, kk, 128 * tt:128 * (tt + 1)],
                                             wv_t[:, kk, :], start=(kk == 0), stop=(kk == 7))
                            if kk == 7:
                                nc.vector.tensor_copy(
                                    va_t[:, 4 * j + tt, :, 0:64],
                                    box["ps"][:].rearrange("p (h d) -> p h d", h=HL))
                        out.append((512, f))
                return out

            def proj_steps(j, m, ps, use_pst, dma_eng, box):
                """Thunks for output-projection chain m of block j over the
                p-range `ps`."""
                out = []
                for p in ps:
                    def f(m=m, p=p, box=box, j=j, use_pst=use_pst, dma_eng=dma_eng):
                        if "pf" not in box:
                            if use_pst:
                                t = pst.tile([128, 1024], F32, tag="st",
                                             name=f"pf{j}_{m}")
                                box["pf"] = t[:, 0:512]
                            else:
                                box["pf"] = paux.tile([128, 512], F32, tag="aux",
                                                      name=f"pf{j}_{m}")[:]
                        nc.tensor.matmul(box["pf"],
                                         wp_t[:, p, 128 * m:128 * (m + 1)],
                                         ctx_tiles[(j, p)][:], start=(p == 0), stop=(p == 3))
                        if p == 3:
                            ob = osb.tile([128, 512], BF16, tag="ob", name=f"ob{j}_{m}")
                            if use_pst:
                                # split the epilogue copies across DVE and the
                                # (idle) ACT engine — Copy lives in the same
                                # act-table set as Exp, and gpsimd can't read
                                # PSUM
                                nc.scalar.copy(ob[:], box["pf"])
                            else:
                                nc.vector.tensor_copy(ob[:], box["pf"])
                            dma_eng.dma_start(
                                outt[128 * m:128 * (m + 1), QB * j:QB * (j + 1)], ob[:])
                    out.append((512, f))
                return out

            part3 = {}

            def gen_proj3_stage_a():
                """Last block's projections, p=0..2: run during the final two
                heads' beats on transient paux slots; partials park in SBUF
                (bf16) so no PSUM is held across the last pair's completion."""
                out = []
                for m in range(8):
                    box = {}
                    for p in range(3):
                        def f(m=m, p=p, box=box):
                            j = NQB - 1
                            if p == 0:
                                box["pf"] = paux.tile([128, 512], F32, tag="aux",
                                                      name=f"pf3a_{m}")[:]
                            nc.tensor.matmul(box["pf"],
                                             wp_t[:, p, 128 * m:128 * (m + 1)],
                                             ctx_tiles[(j, p)][:],
                                             start=(p == 0), stop=(p == 2))
                            if p == 2:
                                pt = pp3.tile([128, 512], BF16, tag="pp",
                                              name=f"part3_{m}")
                                part3[m] = pt
                                nc.vector.tensor_copy(pt[:], box["pf"])
                        out.append((512, f))
                return out

            def gen_proj3_stage_b():
                """Epilogue: re-inject the parked partial via an identity
                matmul, add the p=3 term, write out."""
                out = []
                for m in range(8):
                    def f(m=m):
                        j = NQB - 1
                        if m % 2 == 1:
                            t = pst.tile([128, 1024], F32, tag="st", name=f"pf3b_{m}")
                            pf = t[:, 0:512]
                        else:
                            pf = paux.tile([128, 512], F32, tag="aux",
                                           name=f"pf3b_{m}")[:]
                        nc.tensor.matmul(pf, idn[:], part3[m][:],
                                         start=True, stop=False)
                        nc.tensor.matmul(pf, wp_t[:, 3, 128 * m:128 * (m + 1)],
                                         ctx_tiles[(j, 3)][:], start=False, stop=True)
                        ob = osb.tile([128, 512], BF16, tag="ob", name=f"ob3_{m}")
                        if m % 2 == 1:
                            nc.scalar.copy(ob[:], pf)
                        else:
                            nc.vector.tensor_copy(ob[:], pf)
                        dma_eng = nc.scalar if m % 2 == 1 else nc.sync
                        dma_eng.dma_start(
                            outt[128 * m:128 * (m + 1), QB * j:QB * (j + 1)], ob[:])
                    out.append((1024, f))
                return out

            def gen_proj(j):
                last = (j == NQB - 1)
                out = []
                for m in range(8):
                    out.extend(proj_steps(j, m, range(4), last and m % 2 == 1,
                                          nc.scalar if last and m % 2 == 1 else nc.sync,
                                          {}))
                return out

            # ---------------- per-beat emission ----------------
            ROW_TARGET = 2700   # ~1038ns of exp per beat, in PE rows

            state = {"qkv_done": 0, "qkv_total": 0, "beat": 0, "beats_total": 1,
                     "allow_proj": False, "proj_floor": 0,
                     "proj_done": 0, "proj_total": 0, "prefill_proj": False}

            def pop_one_filler():
                """Emit one independent filler matmul; returns its rows or
                None when nothing is available. Both queues are paced evenly
                over the block's beats so late beats (where exp latency
                dominates) still have cover."""
                tgt = -(-state["qkv_total"] * state["beat"] // state["beats_total"])
                if state["qkv_done"] < tgt and qkv_q:
                    r, f = qkv_q.popleft()
                    f()
                    state["qkv_done"] += 1
                    return r
                ptgt = -(-state["proj_total"] * state["beat"] // state["beats_total"])
                if state["prefill_proj"] and state["proj_done"] < ptgt:
                    if late_q:
                        r, f = late_q.popleft()
                        f()
                        state["proj_done"] += 1
                        return r
                    if len(proj_q) > state["proj_floor"]:
                        r, f = proj_q.popleft()
                        f()
                        state["proj_done"] += 1
                        return r
                return None

            def run_beat_tail(rows):
                # a few independent fillers ahead of the
                # dependency-stalled attention thunk
                for _ in range(3):
                    r = pop_one_filler()
                    if r is None:
                        break
                    rows += r
                # one delayed attention thunk
                if pending:
                    r, f = pending.popleft()
                    f()
                    rows += r
                # paced QKV
                tgt = -(-state["qkv_total"] * state["beat"] // state["beats_total"])
                while state["qkv_done"] < tgt and qkv_q:
                    r, f = qkv_q.popleft()
                    f()
                    state["qkv_done"] += 1
                    rows += r
                # top up with deferred tails/projections (paced)
                while rows < ROW_TARGET:
                    ptgt = -(-state["proj_total"] * state["beat"]
                             // state["beats_total"])
                    if not (state["allow_proj"] and state["proj_done"] < ptgt):
                        break
                    if late_q:
                        r, f = late_q.popleft()
                    elif len(proj_q) > state["proj_floor"]:
                        r, f = proj_q.popleft()
                    else:
                        break
                    f()
                    state["proj_done"] += 1
                    rows += r
                # bound the delayed-thunk backlog (shallow blocks append
                # faster than one-per-beat pops)
                while len(pending) > 5:
                    pop_one_filler()
                    r, f = pending.popleft()
                    f()

            def drain_pending(keep=2):
                while len(pending) > keep:
                    r, f = pending.popleft()
                    f()
                    pop_one_filler()

            tt_tiles = {}

            def attn_pair_beat(j, h, ip):
                """Emit one beat: scores pair + exp + mask, then delayed work
                and fillers; queue this pair's ctx matmuls."""
                p, s = divmod(h, 2)
                q_tile = q_tiles[(j, p)]
                hs = slice(64 * s, 64 * s + 64)
                tp = (64 * s, 0)
                i0, i1 = 2 * ip, 2 * ip + 1
                o0, o1 = i0 - 4 * j, i1 - 4 * j
                cs0 = max(0, 128 * o0)
                cs1 = max(0, 128 * o1)
                if ip == 2 * (j + 1) - 1:
                    # first (diagonal-most) pair of the head: allocate the
                    # ctx^T accumulator bank
                    tt_tiles[("c", j, h)] = pctx.tile([128, 260], F32, tag="ctxa",
                                                      name=f"ctxa{j}_{h}")
                st = pst.tile([128, 1024], F32, tag="st", name=f"st{j}_{h}_{ip}")
                # the i1 half writes LEFT-SHIFTED (columns 512:1024-cs1) so
                # the two causal-narrowed ranges are contiguous at column 512
                # and a single exp op covers the whole pair
                w1hi = 1024 - cs1
                nc.tensor.matmul(st[:, cs0:512],
                                 kt_t[hs, p, 128 * i0:128 * (i0 + 1)],
                                 q_tile[hs, cs0:512],
                                 start=True, stop=True, tile_position=tp)
                nc.tensor.matmul(st[:, 512:w1hi],
                                 kt_t[hs, p, 128 * i1:128 * (i1 + 1)],
                                 q_tile[hs, cs1:512],
                                 start=True, stop=True, tile_position=tp)
                e2 = ep.tile([128, 1024], BF16, tag="e", name=f"e{j}_{h}_{ip}")
                nc.scalar.activation(e2[:, cs0:w1hi], st[:, cs0:w1hi], Act.Exp)
                if o0 >= 0:
                    z = slice(128 * o0, 128 * (o0 + 1))
                    nc.gpsimd.tensor_tensor(e2[:, z], e2[:, z], m1[:],
                                            op=Alu.mult)
                if o1 >= 0:
                    # shifted: the i1 diag zone starts exactly at column 512
                    z = slice(512, 640)
                    nc.gpsimd.tensor_tensor(e2[:, z], e2[:, z], m1[:],
                                            op=Alu.mult)
                rows = (512 - cs0) + (512 - cs1)
                return rows, make_ctx(j, h, e2, i0, i1, o0, o1, cs1)

            def make_ctx(j, h, e2, i0, i1, o0, o1, sh1=0):
                nctx = (4 - max(0, o1)) + (4 - max(0, o0))

                def ctx_f(j=j, h=h, e2=e2, i0=i0, i1=i1, o0=o0, o1=o1, sh1=sh1):
                    # the whole [128, 260] accumulator is ONE hardware
                    # accumulation group: start=True clears the full PSUM
                    # bank, so only the head's first matmul (diag ktile,
                    # subtile 3) starts; every other subtile accumulates
                    # onto the cleared bank
                    ctxa = tt_tiles[("c", j, h)]
                    for (i, base, o) in ((i1, 512 - sh1, o1), (i0, 0, o0)):
                        for qq in range(max(0, o), 4):
                            nc.tensor.matmul(
                                ctxa[:, 65 * qq:65 * (qq + 1)],
                                e2[:, base + 128 * qq:base + 128 * (qq + 1)],
                                va_t[:, i, h, :],
                                start=(i == 4 * j + 3 and qq == 3),
                                stop=(i == 0 and qq == 3),
                                skip_group_check=True)
                return (65 * nctx, ctx_f)

            la_e2 = {}

            def emit_la(jn, h):
                """Lookahead: scores+exp for (block jn, head h, k-tiles 0-1)
                emitted a block early, while the ACT engine is otherwise
                idle; e2 parks in SBUF until block jn's ctx matmuls."""
                p, s = divmod(h, 2)
                q_tile = q_tiles[(jn, p)]
                hs = slice(64 * s, 64 * s + 64)
                tp = (64 * s, 0)
                st = pst.tile([128, 1024], F32, tag="st", name=f"lst{jn}_{h}")
                nc.tensor.matmul(st[:, 0:512], kt_t[hs, p, 0:128],
                                 q_tile[hs, :], start=True, stop=True,
                                 tile_position=tp)
                nc.tensor.matmul(st[:, 512:1024], kt_t[hs, p, 128:256],
                                 q_tile[hs, :], start=True, stop=True,
                                 tile_position=tp)
                e2 = elp.tile([128, 1024], BF16, tag="ela", name=f"le{jn}_{h}")
                nc.scalar.activation(e2[:], st[:], Act.Exp)
                la_e2[(jn, h)] = e2
                return 1024

            def tail_a(j, h):
                def f(j=j, h=h):
                    ctxa = tt_tiles[("c", j, h)]
                    # ONE plain copy out of PSUM, so the single ctx^T
                    # accumulator bank frees for the next head after ~400ns;
                    # reciprocal + scaling run from the SBUF copy (2x DVE)
                    raw = ctsp.tile([128, 260], BF16, tag="raw", bufs=2,
                                    name=f"raw{j}_{h}")
                    nc.vector.tensor_copy(raw[:], ctxa[:])
                    rc = rcq.tile([128, 4], F32, tag="rc", name=f"rc{j}_{h}")
                    # blocks 0-1 defer their transposes into blocks 2-3, so
                    # up to 16 cts tiles stay live
                    cts = ctsp.tile([128, 256], BF16, tag="cts", bufs=18,
                                    name=f"cts{j}_{h}")
                    tt_tiles[("s", j, h)] = cts
                    raw4 = raw[:].rearrange("p (q c) -> p q c", q=4)
                    nc.vector.reciprocal(rc[:], raw4[:, :, 64:65].squeeze(2))
                    for qq in range(4):
                        nc.vector.tensor_scalar_mul(
                            cts[:, 64 * qq:64 * (qq + 1)],
                            raw[:, 65 * qq:65 * qq + 64],
                            rc[:, qq:qq + 1])
                return (0, f)

            def tail_b(j, h):
                def f(j=j, h=h):
                    p, s = divmod(h, 2)
                    if s == 0:
                        tt_tiles[("t", j, p)] = ptt.tile([128, 512], BF16, tag="tt",
                                                      name=f"tt{j}_{p}")
                    tt = tt_tiles[("t", j, p)]
                    cts = tt_tiles[("s", j, h)]
                    for qq in range(4):
                        nc.tensor.transpose(
                            tt[64 * s:64 * s + 64, 128 * qq:128 * (qq + 1)],
                            cts[:, 64 * qq:64 * (qq + 1)], idn[:])
                return (512, f)

            def tail_c(j, p):
                def f(j=j, p=p):
                    ctx_pair = cxp.tile([128, 512], BF16, tag="ctx", name=f"ctx{j}_{p}")
                    ctx_tiles[(j, p)] = ctx_pair
                    nc.vector.tensor_copy(ctx_pair[:], tt_tiles[("t", j, p)][:])
                    if p == 3:
                        proj_q.extend(gen_proj(j))
                return (0, f)

            # ---------------- prologue: QKV for block 0 ----------------
            # Q runs kk-major across 4 interleaved accumulation chains (2
            # paux slots + 2 borrowed score slots) so each arriving x-chunk
            # DMA feeds 4 matmuls — the chain-major order would stall on the
            # serialized x0 chunk issues
            psq, qts = [], []
            for p in range(4):
                if p < 2:
                    ps = paux.tile([128, 512], F32, tag="aux", name=f"psq0_{p}")[:]
                else:
                    ps = pst.tile([128, 1024], F32, tag="st", name=f"psq0_{p}")[:, 0:512]
                psq.append(ps)
                qt = qtp.tile([128, 512], BF16, tag="qt", name=f"qt0_{p}")
                q_tiles[(0, p)] = qt
                qts.append(qt)
            for kk in range(8):
                for p in range(4):
                    nc.tensor.matmul(psq[p], wq_t[:, kk, 128 * p:128 * (p + 1)],
                                     x_t0[:, kk, :], start=(kk == 0), stop=(kk == 7))
            for p in range(4):
                nc.vector.tensor_copy(qts[p][:], psq[p])
            for r, f in gen_kv(0):
                f()

            # ---------------- main loop ----------------
            for j in range(NQB):
                qkv_q.clear()
                if j + 1 < NQB:
                    load_x(j + 1)
                    qkv_q.extend(gen_q(j + 1))
                    qkv_q.extend(gen_kv(j + 1))
                npair = 2 * (j + 1)
                state["qkv_total"] = len(qkv_q)
                state["qkv_done"] = 0
                state["beats_total"] = 8 * npair + (8 if j < 3 else 0)
                state["beat"] = 0
                state["allow_proj"] = (j >= 2)
                state["prefill_proj"] = (j == NQB - 1)
                state["proj_floor"] = 8
                state["proj_done"] = 0
                state["proj_total"] = len(proj_q) + len(late_q)
                for h in range(HL):
                    first = True
                    for ip in reversed(range(npair)):
                        state["beat"] += 1
                        if ip == 0 and (j, h) in la_e2:
                            # this pair's scores+exp were precomputed a
                            # block early — only the ctx matmuls remain
                            rows = 0
                            ctx_thunk = make_ctx(j, h, la_e2.pop((j, h)),
                                                 0, 1, -4 * j, 1 - 4 * j)
                        else:
                            rows, ctx_thunk = attn_pair_beat(j, h, ip)
                        if first:
                            # head boundary: clear old ctx thunks behind the
                            # freshly issued scores+exp, but leave the
                            # previous head's tail chain (A/B/C) to spread
                            # over the next beats — B stalls on A's DVE work
                            # if popped in the same beat
                            drain_pending(keep=5 if len(pending) <= 6 else 6)
                            first = False
                        run_beat_tail(rows)
                        pending.append(ctx_thunk)
                    if j < 3:
                        # lookahead beat: next block's far pair for this head
                        state["beat"] += 1
                        run_beat_tail(emit_la(j + 1, h))
                    pending.append(tail_a(j, h))
                    if h % 2 == 1:
                        # transpose + assemble as one atomic per-pair entry
                        # (the single tt PSUM slot must not interleave two
                        # pairs); blocks 0-1 defer theirs into blocks 2-3,
                        # where exp latency otherwise starves the PE
                        ra, fa = tail_b(j, h - 1)
                        rb, fb = tail_b(j, h)
                        rc_, fc = tail_c(j, h // 2)

                        def bc(fa=fa, fb=fb, fc=fc):
                            fa()
                            fb()
                            fc()
                        entry = (ra + rb + rc_, bc)
                        if j < 2:
                            late_q.append(entry)
                        else:
                            pending.append(entry)
                # block end: QKV for next block must be complete
                while qkv_q:
                    r, f = qkv_q.popleft()
                    f()

            # ---------------- epilogue ----------------
            state["allow_proj"] = True
            state["prefill_proj"] = True
            state["proj_floor"] = 0
            drain_pending(keep=0)
            while late_q:
                r, f = late_q.popleft()
                f()
            while proj_q:
                r, f = proj_q.popleft()
                f()
    return nc


def _split_waits(nc, limit=1):
    """This walrus build accepts only one sync wait per TPB_CTRL instruction;
    move excess waits onto preceding same-engine NOPs."""
    import concourse.mybir as mybir
    for f in nc.m.functions:
        for bb in f.blocks:
            new_insts = []
            for inst in bb.instructions:
                si = inst.sync_info
                if si is not None and si.on_wait and len(si.on_wait) > limit:
                    waits = list(si.on_wait)
                    k = 0
                    while len(waits) - k > limit:
                        chunk = waits[k:k + limit]
                        k += limit
                        nop = mybir.InstNoOp(name=f"{inst.name}_ws{k}")
                        nop.engine = inst.engine
                        nop.sync_info = mybir.SyncInfo(on_wait=chunk, on_update=[])
                        new_insts.append(nop)
                    si.on_wait = waits[k:]
                new_insts.append(inst)
            bb.instructions = new_insts


# --------------------------------------------------------------------------
# compile + SPMD execution via PJRT (axon) — jit once, reuse
# --------------------------------------------------------------------------
class _Compiled:
    def __init__(self, n_cores=8):
        import jax
        from jax.sharding import Mesh, PartitionSpec
        from jax.experimental.shard_map import shard_map
        import concourse.mybir as mybir
        from concourse.bass2jax import (_bass_exec_p, install_neuronx_cc_hook,
                                        partition_id_tensor)

        nc = _build_bass()
        _split_waits(nc)
        install_neuronx_cc_hook()
        partition_name = nc.partition_id_tensor.name if nc.partition_id_tensor else None
        in_names, out_names, out_avals, zero_outs = [], [], [], []
        for alloc in nc.m.functions[0].allocations:
            if not isinstance(alloc, mybir.MemoryLocationSet):
                continue
            name = alloc.memorylocations[0].name
            if alloc.kind == "ExternalInput":
                if name != partition_name:
                    in_names.append(name)
            elif alloc.kind == "ExternalOutput":
                shape = tuple(alloc.tensor_shape)
                dtype = mybir.dt.np(alloc.dtype)
                out_names.append(name)
                out_avals.append(jax.core.ShapedArray(shape, dtype))
                zero_outs.append(np.zeros(shape, dtype))
        n_params = len(in_names)
        all_in_names = list(in_names) + list(out_names)
        if partition_name is not None:
            all_in_names.append(partition_name)

        def _body(*args):
            operands = list(args)
            if partition_name is not None:
                operands.append(partition_id_tensor())
            outs = _bass_exec_p.bind(
                *operands,
                out_avals=tuple(out_avals),
                in_names=tuple(all_in_names),
                out_names=tuple(out_names),
                lowering_input_output_aliases=(),
                sim_require_finite=True,
                sim_require_nnan=True,
                nc=nc,
            )
            return tuple(outs)

        devices = jax.devices()[:n_cores]
        assert len(devices) >= n_cores, f"need {n_cores} cores, have {len(devices)}"
        self.n_cores = n_cores
        self.in_names, self.out_names = in_names, out_names
        self.out_avals, self.zero_outs = out_avals, zero_outs
        mesh = Mesh(np.asarray(devices[:n_cores]), ("core",))
        in_specs = (PartitionSpec("core"),) * (n_params + len(out_names))
        out_specs = (PartitionSpec("core"),) * len(out_names)
        self.fn = jax.jit(
            shard_map(_body, mesh=mesh, in_specs=in_specs,
                      out_specs=out_specs, check_rep=False),
            keep_unused=True)

    def run(self, in_maps):
        import jax
        args = []
        for name in self.in_names:
            args.append(np.concatenate([np.asarray(m[name]) for m in in_maps], axis=0))
        for z in self.zero_outs:
            args.append(np.zeros((self.n_cores * z.shape[0], *z.shape[1:]), z.dtype))
        if not getattr(self, "_warm", False):
            # the very first execution after device bring-up can read
            # uninitialized PSUM; do one discarded warm-up pass
            jax.block_until_ready(self.fn(*args))
            self._warm = True
        outs = self.fn(*args)
        jax.block_until_ready(outs)
        res = []
        for c in range(self.n_cores):
            d = {}
            for i, name in enumerate(self.out_names):
                a = np.asarray(outs[i]).reshape(self.n_cores, *self.out_avals[i].shape)[c]
                d[name] = a
            res.append(d)
        return res


# --------------------------------------------------------------------------
# host-side shard / unshard
# --------------------------------------------------------------------------
def _bf16(a):
    import ml_dtypes
    return np.ascontiguousarray(a).astype(ml_dtypes.bfloat16)


def _make_core_inputs(x, Wq, Wk, Wv, Wp, core):
    g = core % 2
    b = core // 2
    rows = slice(512 * g, 512 * (g + 1))
    kl = np.arange(128)
    return {
        "xt": _bf16(x[b].T),
        # fold the 1/sqrt(head_dim) score scale into Wq
        "wq": _bf16(Wq[rows, :].T * 0.125),
        "wk": _bf16(Wk[rows, :].T),
        "wv": _bf16(Wv[rows, :].T),
        "wp": _bf16(Wp[:, rows].T),
        "mask1": _bf16((kl[:, None] <= kl[None, :]).astype(np.float32)),
        "ident": _bf16(np.eye(128, dtype=np.float32)),
        "vones": _bf16(np.ones((128, NKT * HL), np.float32)),
    }


def kernel(x, Wq, Wk, Wv, Wp):
    """Full-input / full-output causal MHA. x: (4, 2048, 1024) fp32;
    Wq/Wk/Wv/Wp: (1024, 1024) fp32. Returns (4, 2048, 1024) fp32."""
    global _COMPILED
    x = np.asarray(x, dtype=np.float32)
    Wq = np.asarray(Wq, dtype=np.float32)
    Wk = np.asarray(Wk, dtype=np.float32)
    Wv = np.asarray(Wv, dtype=np.float32)
    Wp = np.asarray(Wp, dtype=np.float32)
    assert x.shape == (B, T, D), x.shape

    if _COMPILED is None:
        _COMPILED = _Compiled(8)
    in_maps = [_make_core_inputs(x, Wq, Wk, Wv, Wp, c) for c in range(8)]
    results = _COMPILED.run(in_maps)

    out = np.empty((B, T, D), np.float32)
    for b in range(B):
        acc = (results[2 * b]["outt"].astype(np.float32)
               + results[2 * b + 1]["outt"].astype(np.float32))
        out[b] = acc.T
    return out


# revision 10
# speedup vs baseline: 1.0300x; 1.0020x over previous
"""Causal multi-head attention (B=4, T=2048, D=1024, H=16) on 8 Trainium2 cores.

Sharding (data + tensor parallel): core c handles batch b = c//2 and head-group
g = c%2 (8 of the 16 heads). Wq/Wk/Wv are column-sharded by head, Wp is
row-sharded; the two per-batch partial outputs are summed on the host (this
replaces the device all-reduce — the host-side sum is the unshard step).

v2 restructure vs the 293us baseline (the cost model charges matmuls by output
free-dim rows only; PE re-ramps to half speed after every idle gap):
  - all matmul operands bf16 (same 1 cyc/row as f32r, but exact causal
    narrowing is allowed — no >=256-wide f32r constraint — and DMA/SBUF halve)
  - scores St [kpos, q] per k-tile pair, exactly causal-narrowed
  - attention*V flipped: stationary = E-tile [128 kpos, 128 q] (slice of the
    already-transposed e2), moving = V-aug [128 kpos, 65] -> ctx^T [128 q, 65]
    costs 65 rows/tile instead of 128; the softmax denominator rides along as
    column 64 via an all-ones column in V-aug
  - normalization on DVE: per-partition reciprocal + tensor_scalar multiply
    while copying ctx^T out of PSUM (q is the partition axis there), then PE
    transposes back to hd-major [64, 512] per head
  - software-pipelined emission: every "beat" issues one score pair + exp,
    then runs one delayed thunk (the previous pair's ctx matmuls, or head-tail
    work), then filler matmuls (next-block QKV paced evenly; output
    projections deferred into block 3 where exp latency would otherwise
    starve the PE)
"""
import collections
import numpy as np

T = 2048
D = 1024
B = 4
H = 16
HL = 8            # heads per core
NP = 4            # head pairs per core
QB = 512          # q-block width
NQB = T // QB     # 4 q-blocks
NKT = T // 128    # 16 k-tiles

_COMPILED = None


# --------------------------------------------------------------------------
# bass kernel build
# --------------------------------------------------------------------------
def _build_bass():
    import concourse.bass as bass
    import concourse.mybir as mybir
    from concourse.tile import TileContext

    F32 = mybir.dt.float32
    BF16 = mybir.dt.bfloat16
    Act = mybir.ActivationFunctionType
    Alu = mybir.AluOpType

    nc = bass.Bass()
    xt = nc.dram_tensor("xt", [D, T], BF16, kind="ExternalInput")
    wq = nc.dram_tensor("wq", [D, 512], BF16, kind="ExternalInput")
    wk = nc.dram_tensor("wk", [D, 512], BF16, kind="ExternalInput")
    wv = nc.dram_tensor("wv", [D, 512], BF16, kind="ExternalInput")
    wp = nc.dram_tensor("wp", [512, D], BF16, kind="ExternalInput")
    mask1 = nc.dram_tensor("mask1", [128, 128], BF16, kind="ExternalInput")
    ident = nc.dram_tensor("ident", [128, 128], BF16, kind="ExternalInput")
    vones = nc.dram_tensor("vones", [128, NKT * HL], BF16, kind="ExternalInput")
    outt = nc.dram_tensor("outt", [D, T], BF16, kind="ExternalOutput")

    with TileContext(nc) as tc, nc.allow_low_precision(reason="bf16 pipeline"):
        with tc.tile_pool(name="wts", bufs=1) as wts, \
             tc.tile_pool(name="xp", bufs=2) as xp, \
             tc.tile_pool(name="big", bufs=1) as big, \
             tc.tile_pool(name="qtp", bufs=8) as qtp, \
             tc.tile_pool(name="ep", bufs=8) as ep, \
             tc.tile_pool(name="elp", bufs=18) as elp, \
             tc.tile_pool(name="ctsp", bufs=2) as ctsp, \
             tc.tile_pool(name="rcq", bufs=2) as rcq, \
             tc.tile_pool(name="cxp", bufs=16) as cxp, \
             tc.tile_pool(name="sm", bufs=1) as sm, \
             tc.tile_pool(name="osb", bufs=8) as osb, \
             tc.tile_pool(name="pp3", bufs=8) as pp3, \
             tc.tile_pool(name="pst", bufs=2, space="PSUM") as pst, \
             tc.tile_pool(name="pctx", bufs=1, space="PSUM") as pctx, \
             tc.tile_pool(name="ptt", bufs=1, space="PSUM") as ptt, \
             tc.tile_pool(name="paux", bufs=2, space="PSUM") as paux:

            # ---------------- weights/constants ----------------
            wq_t = wts.tile([128, 8, 512], BF16, tag="wq")
            wk_t = wts.tile([128, 8, 512], BF16, tag="wk")
            wv_t = wts.tile([128, 8, 512], BF16, tag="wv")
            wp_t = wts.tile([128, 4, 1024], BF16, tag="wp")
            wqr = wq[:].rearrange("(n p) m -> p n m", p=128)
            wkr = wk[:].rearrange("(n p) m -> p n m", p=128)
            wvr = wv[:].rearrange("(n p) m -> p n m", p=128)
            x_tiles = {}

            def load_x(tb):
                # 2-slice chunks on the SP queue: 4 issues instead of 8
                x_t = xp.tile([128, 8, 512], BF16, tag="x", name=f"x_t{tb}")
                xr = xt[:, QB * tb:QB * (tb + 1)].rearrange("(n p) m -> p n m", p=128)
                for c in range(4):
                    nc.sync.dma_start(x_t[:, 2 * c:2 * c + 2, :], xr[:, 2 * c:2 * c + 2, :])
                x_tiles[tb] = x_t

            # DMA issue is serialized per DGE queue (~600ns each), so spread
            # the prologue loads across the two HWDGE queues (SP + Act):
            #   SP:  wq, x0 (critical path for the first Q chains), then x1..
            #   Act: wk, wv, wp, constants (needed a few us later)
            x_t0 = xp.tile([128, 8, 512], BF16, tag="x", name="x_t0")
            xr0 = xt[:, 0:QB].rearrange("(n p) m -> p n m", p=128)
            # wq chunks 2-3 ride the Act queue: they arrive earlier than
            # SP's 5th/6th serialized issues would deliver them
            for c in range(2):
                nc.sync.dma_start(wq_t[:, 2 * c:2 * c + 2, :], wqr[:, 2 * c:2 * c + 2, :])
                nc.sync.dma_start(x_t0[:, 2 * c:2 * c + 2, :], xr0[:, 2 * c:2 * c + 2, :])
            for c in range(2, 4):
                nc.scalar.dma_start(wq_t[:, 2 * c:2 * c + 2, :], wqr[:, 2 * c:2 * c + 2, :])
                nc.sync.dma_start(x_t0[:, 2 * c:2 * c + 2, :], xr0[:, 2 * c:2 * c + 2, :])
            x_tiles[0] = x_t0
            for c in range(4):
                nc.scalar.dma_start(wk_t[:, 2 * c:2 * c + 2, :], wkr[:, 2 * c:2 * c + 2, :])
            for c in range(4):
                nc.scalar.dma_start(wv_t[:, 2 * c:2 * c + 2, :], wvr[:, 2 * c:2 * c + 2, :])
            m1 = sm.tile([128, 128], BF16, tag="m1")
            nc.scalar.dma_start(m1[:], mask1[:])
            idn = sm.tile([128, 128], BF16, tag="idn")
            nc.scalar.dma_start(idn[:], ident[:])

            kt_t = big.tile([128, 4, T], BF16, tag="kt")
            va_t = big.tile([128, NKT, HL, 65], BF16, tag="va")
            nc.scalar.dma_start(
                va_t[:, :, :, 64:65].squeeze(3),
                vones[:].rearrange("p (n h) -> p n h", n=NKT))
            wpr = wp[:].rearrange("(n p) m -> p n m", p=128)
            for c in range(2):
                nc.scalar.dma_start(wp_t[:, 2 * c:2 * c + 2, :], wpr[:, 2 * c:2 * c + 2, :])

            q_tiles = {}     # (j, p) -> sbuf tile [128, 512]
            ctx_tiles = {}   # (j, p) -> sbuf tile [128, 512] (normalized pair)

            # ---------------- thunk plumbing ----------------
            # a thunk is (rows, fn); rows = PE rows it will emit (for pacing)
            pending = collections.deque()   # attention work delayed >=1 beat
            qkv_q = collections.deque()     # next-block QKV (must finish)
            proj_q = collections.deque()    # deferred output projections
            late_q = collections.deque()    # deferred head tails (blocks 0-1)

            def gen_q(j):
                x_t = x_tiles[j]
                out = []
                for p in range(4):
                    box = {}
                    for kk in range(8):
                        def f(p=p, kk=kk, box=box, x_t=x_t, j=j):
                            if kk == 0:
                                box["ps"] = paux.tile([128, 512], F32, tag="aux",
                                                      name=f"psq{j}_{p}")
                                box["q"] = qtp.tile([128, 512], BF16, tag="qt",
                                                    name=f"qt{j}_{p}")
                                q_tiles[(j, p)] = box["q"]
                            nc.tensor.matmul(box["ps"][:],
                                             wq_t[:, kk, 128 * p:128 * (p + 1)],
                                             x_t[:, kk, :], start=(kk == 0), stop=(kk == 7))
                            if kk == 7:
                                nc.vector.tensor_copy(box["q"][:], box["ps"][:])
                        out.append((512, f))
                return out

            def gen_kv(j):
                x_t = x_tiles[j]
                out = []
                for p in range(4):
                    box = {}
                    for kk in range(8):
                        def f(p=p, kk=kk, box=box, x_t=x_t, j=j):
                            if kk == 0:
                                box["ps"] = paux.tile([128, 512], F32, tag="aux",
                                                      name=f"psk{j}_{p}")
                            nc.tensor.matmul(box["ps"][:],
                                             wk_t[:, kk, 128 * p:128 * (p + 1)],
                                             x_t[:, kk, :], start=(kk == 0), stop=(kk == 7))
                            if kk == 7:
                                nc.vector.tensor_copy(
                                    kt_t[:, p, QB * j:QB * (j + 1)], box["ps"][:])
                        out.append((512, f))
                for tt in range(4):
                    box = {}
                    for kk in range(8):
                        def f(tt=tt, kk=kk, box=box, x_t=x_t, j=j):
                            if kk == 0:
                                box["ps"] = paux.tile([128, 512], F32, tag="aux",
                                                      name=f"psv{j}_{tt}")
                            nc.tensor.matmul(box["ps"][:],
                                             x_t[:, kk, 128 * tt:128 * (tt + 1)],
                                             wv_t[:, kk, :], start=(kk == 0), stop=(kk == 7))
                            if kk == 7:
                                nc.vector.tensor_copy(
                                    va_t[:, 4 * j + tt, :, 0:64],
                                    box["ps"][:].rearrange("p (h d) -> p h d", h=HL))
                        out.append((512, f))
                return out

            def proj_steps(j, m, ps, use_pst, dma_eng, box):
                """Thunks for output-projection chain m of block j over the
                p-range `ps`."""
                out = []
                for p in ps:
                    def f(m=m, p=p, box=box, j=j, use_pst=use_pst, dma_eng=dma_eng):
                        if "pf" not in box:
                            if use_pst:
                                t = pst.tile([128, 1024], F32, tag="st",
                                             name=f"pf{j}_{m}")
                                box["pf"] = t[:, 0:512]
                            else:
                                box["pf"] = paux.tile([128, 512], F32, tag="aux",
                                                      name=f"pf{j}_{m}")[:]
                        nc.tensor.matmul(box["pf"],
                                         wp_t[:, p, 128 * m:128 * (m + 1)],
                                         ctx_tiles[(j, p)][:], start=(p == 0), stop=(p == 3))
                        if p == 3:
                            ob = osb.tile([128, 512], BF16, tag="ob", name=f"ob{j}_{m}")
                            if use_pst:
                                # split the epilogue copies across DVE and the
                                # (idle) ACT engine — Copy lives in the same
                                # act-table set as Exp, and gpsimd can't read
                                # PSUM
                                nc.scalar.copy(ob[:], box["pf"])
                            else:
                                nc.vector.tensor_copy(ob[:], box["pf"])
                            dma_eng.dma_start(
                                outt[128 * m:128 * (m + 1), QB * j:QB * (j + 1)], ob[:])
                    out.append((512, f))
                return out

            part3 = {}

            def gen_proj3_stage_a():
                """Last block's projections, p=0..2: run during the final two
                heads' beats on transient paux slots; partials park in SBUF
                (bf16) so no PSUM is held across the last pair's completion."""
                out = []
                for m in range(8):
                    box = {}
                    for p in range(3):
                        def f(m=m, p=p, box=box):
                            j = NQB - 1
                            if p == 0:
                                box["pf"] = paux.tile([128, 512], F32, tag="aux",
                                                      name=f"pf3a_{m}")[:]
                            nc.tensor.matmul(box["pf"],
                                             wp_t[:, p, 128 * m:128 * (m + 1)],
                                             ctx_tiles[(j, p)][:],
                                             start=(p == 0), stop=(p == 2))
                            if p == 2:
                                pt = pp3.tile([128, 512], BF16, tag="pp",
                                              name=f"part3_{m}")
                                part3[m] = pt
                                nc.vector.tensor_copy(pt[:], box["pf"])
                        out.append((512, f))
                return out

            def gen_proj3_stage_b():
                """Epilogue: re-inject the parked partial via an identity
                matmul, add the p=3 term, write out."""
                out = []
                for m in range(8):
                    def f(m=m):
                        j = NQB - 1
                        if m % 2 == 1:
                            t = pst.tile([128, 1024], F32, tag="st", name=f"pf3b_{m}")
                            pf = t[:, 0:512]
                        else:
                            pf = paux.tile([128, 512], F32, tag="aux",
                                           name=f"pf3b_{m}")[:]
                        nc.tensor.matmul(pf, idn[:], part3[m][:],
                                         start=True, stop=False)
                        nc.tensor.matmul(pf, wp_t[:, 3, 128 * m:128 * (m + 1)],
                                         ctx_tiles[(j, 3)][:], start=False, stop=True)
                        ob = osb.tile([128, 512], BF16, tag="ob", name=f"ob3_{m}")
                        if m % 2 == 1:
                            nc.scalar.copy(ob[:], pf)
                        else:
                            nc.vector.tensor_copy(ob[:], pf)
                        dma_eng = nc.scalar if m % 2 == 1 else nc.sync
                        dma_eng.dma_start(
                            outt[128 * m:128 * (m + 1), QB * j:QB * (j + 1)], ob[:])
                    out.append((1024, f))
                return out

            def gen_proj(j):
                last = (j == NQB - 1)
                out = []
                for m in range(8):
                    out.extend(proj_steps(j, m, range(4), last and m % 2 == 1,
                                          nc.scalar if last and m % 2 == 1 else nc.sync,
                                          {}))
                return out

            # ---------------- per-beat emission ----------------
            ROW_TARGET = 2700   # ~1038ns of exp per beat, in PE rows

            state = {"qkv_done": 0, "qkv_total": 0, "beat": 0, "beats_total": 1,
                     "allow_proj": False, "proj_floor": 0,
                     "proj_done": 0, "proj_total": 0, "prefill_proj": False}

            def pop_one_filler():
                """Emit one independent filler matmul; returns its rows or
                None when nothing is available. Both queues are paced evenly
                over the block's beats so late beats (where exp latency
                dominates) still have cover."""
                tgt = -(-state["qkv_total"] * state["beat"] // state["beats_total"])
                if state["qkv_done"] < tgt and qkv_q:
                    r, f = qkv_q.popleft()
                    f()
                    state["qkv_done"] += 1
                    return r
                ptgt = -(-state["proj_total"] * state["beat"] // state["beats_total"])
                if state["prefill_proj"] and state["proj_done"] < ptgt:
                    if late_q:
                        r, f = late_q.popleft()
                        f()
                        state["proj_done"] += 1
                        return r
                    if len(proj_q) > state["proj_floor"]:
                        r, f = proj_q.popleft()
                        f()
                        state["proj_done"] += 1
                        return r
                return None

            def run_beat_tail(rows):
                # a few independent fillers ahead of the
                # dependency-stalled attention thunk
                for _ in range(4):
                    r = pop_one_filler()
                    if r is None:
                        break
                    rows += r
                # one delayed attention thunk
                if pending:
                    r, f = pending.popleft()
                    f()
                    rows += r
                # paced QKV
                tgt = -(-state["qkv_total"] * state["beat"] // state["beats_total"])
                while state["qkv_done"] < tgt and qkv_q:
                    r, f = qkv_q.popleft()
                    f()
                    state["qkv_done"] += 1
                    rows += r
                # top up with deferred tails/projections (paced)
                while rows < ROW_TARGET:
                    ptgt = -(-state["proj_total"] * state["beat"]
                             // state["beats_total"])
                    if not (state["allow_proj"] and state["proj_done"] < ptgt):
                        break
                    if late_q:
                        r, f = late_q.popleft()
                    elif len(proj_q) > state["proj_floor"]:
                        r, f = proj_q.popleft()
                    else:
                        break
                    f()
                    state["proj_done"] += 1
                    rows += r
                # bound the delayed-thunk backlog (shallow blocks append
                # faster than one-per-beat pops)
                while len(pending) > 5:
                    pop_one_filler()
                    r, f = pending.popleft()
                    f()

            def drain_pending(keep=2):
                while len(pending) > keep:
                    r, f = pending.popleft()
                    f()
                    pop_one_filler()

            tt_tiles = {}

            def attn_pair_beat(j, h, ip):
                """Emit one beat: scores pair + exp + mask, then delayed work
                and fillers; queue this pair's ctx matmuls."""
                p, s = divmod(h, 2)
                q_tile = q_tiles[(j, p)]
                hs = slice(64 * s, 64 * s + 64)
                tp = (64 * s, 0)
                i0, i1 = 2 * ip, 2 * ip + 1
                o0, o1 = i0 - 4 * j, i1 - 4 * j
                cs0 = max(0, 128 * o0)
                cs1 = max(0, 128 * o1)
                if ip == 2 * (j + 1) - 1:
                    # first (diagonal-most) pair of the head: allocate the
                    # ctx^T accumulator bank
                    tt_tiles[("c", j, h)] = pctx.tile([128, 260], F32, tag="ctxa",
                                                      name=f"ctxa{j}_{h}")
                st = pst.tile([128, 1024], F32, tag="st", name=f"st{j}_{h}_{ip}")
                # the i1 half writes LEFT-SHIFTED (columns 512:1024-cs1) so
                # the two causal-narrowed ranges are contiguous at column 512
                # and a single exp op covers the whole pair
                w1hi = 1024 - cs1
                nc.tensor.matmul(st[:, cs0:512],
                                 kt_t[hs, p, 128 * i0:128 * (i0 + 1)],
                                 q_tile[hs, cs0:512],
                                 start=True, stop=True, tile_position=tp)
                nc.tensor.matmul(st[:, 512:w1hi],
                                 kt_t[hs, p, 128 * i1:128 * (i1 + 1)],
                                 q_tile[hs, cs1:512],
                                 start=True, stop=True, tile_position=tp)
                e2 = ep.tile([128, 1024], BF16, tag="e", name=f"e{j}_{h}_{ip}")
                nc.scalar.activation(e2[:, cs0:w1hi], st[:, cs0:w1hi], Act.Exp)
                if o0 >= 0:
                    z = slice(128 * o0, 128 * (o0 + 1))
                    nc.gpsimd.tensor_tensor(e2[:, z], e2[:, z], m1[:],
                                            op=Alu.mult)
                if o1 >= 0:
                    # shifted: the i1 diag zone starts exactly at column 512
                    z = slice(512, 640)
                    nc.gpsimd.tensor_tensor(e2[:, z], e2[:, z], m1[:],
                                            op=Alu.mult)
                rows = (512 - cs0) + (512 - cs1)
                return rows, make_ctx(j, h, e2, i0, i1, o0, o1, cs1)

            def make_ctx(j, h, e2, i0, i1, o0, o1, sh1=0):
                nctx = (4 - max(0, o1)) + (4 - max(0, o0))

                def ctx_f(j=j, h=h, e2=e2, i0=i0, i1=i1, o0=o0, o1=o1, sh1=sh1):
                    # the whole [128, 260] accumulator is ONE hardware
                    # accumulation group: start=True clears the full PSUM
                    # bank, so only the head's first matmul (diag ktile,
                    # subtile 3) starts; every other subtile accumulates
                    # onto the cleared bank
                    ctxa = tt_tiles[("c", j, h)]
                    for (i, base, o) in ((i1, 512 - sh1, o1), (i0, 0, o0)):
                        for qq in range(max(0, o), 4):
                            nc.tensor.matmul(
                                ctxa[:, 65 * qq:65 * (qq + 1)],
                                e2[:, base + 128 * qq:base + 128 * (qq + 1)],
                                va_t[:, i, h, :],
                                start=(i == 4 * j + 3 and qq == 3),
                                stop=(i == 0 and qq == 3),
                                skip_group_check=True)
                return (65 * nctx, ctx_f)

            la_e2 = {}

            def emit_la(jn, h):
                """Lookahead: scores+exp for (block jn, head h, k-tiles 0-1)
                emitted a block early, while the ACT engine is otherwise
                idle; e2 parks in SBUF until block jn's ctx matmuls."""
                p, s = divmod(h, 2)
                q_tile = q_tiles[(jn, p)]
                hs = slice(64 * s, 64 * s + 64)
                tp = (64 * s, 0)
                st = pst.tile([128, 1024], F32, tag="st", name=f"lst{jn}_{h}")
                nc.tensor.matmul(st[:, 0:512], kt_t[hs, p, 0:128],
                                 q_tile[hs, :], start=True, stop=True,
                                 tile_position=tp)
                nc.tensor.matmul(st[:, 512:1024], kt_t[hs, p, 128:256],
                                 q_tile[hs, :], start=True, stop=True,
                                 tile_position=tp)
                e2 = elp.tile([128, 1024], BF16, tag="ela", name=f"le{jn}_{h}")
                nc.scalar.activation(e2[:], st[:], Act.Exp)
                la_e2[(jn, h)] = e2
                return 1024

            def tail_a(j, h):
                def f(j=j, h=h):
                    ctxa = tt_tiles[("c", j, h)]
                    # ONE plain copy out of PSUM, so the single ctx^T
                    # accumulator bank frees for the next head after ~400ns;
                    # reciprocal + scaling run from the SBUF copy (2x DVE)
                    raw = ctsp.tile([128, 260], BF16, tag="raw", bufs=2,
                                    name=f"raw{j}_{h}")
                    nc.vector.tensor_copy(raw[:], ctxa[:])
                    rc = rcq.tile([128, 4], F32, tag="rc", name=f"rc{j}_{h}")
                    # blocks 0-1 defer their transposes into blocks 2-3, so
                    # up to 16 cts tiles stay live
                    cts = ctsp.tile([128, 256], BF16, tag="cts", bufs=18,
                                    name=f"cts{j}_{h}")
                    tt_tiles[("s", j, h)] = cts
                    raw4 = raw[:].rearrange("p (q c) -> p q c", q=4)
                    nc.vector.reciprocal(rc[:], raw4[:, :, 64:65].squeeze(2))
                    for qq in range(4):
                        nc.vector.tensor_scalar_mul(
                            cts[:, 64 * qq:64 * (qq + 1)],
                            raw[:, 65 * qq:65 * qq + 64],
                            rc[:, qq:qq + 1])
                return (0, f)

            def tail_b(j, h):
                def f(j=j, h=h):
                    p, s = divmod(h, 2)
                    if s == 0:
                        tt_tiles[("t", j, p)] = ptt.tile([128, 512], BF16, tag="tt",
                                                      name=f"tt{j}_{p}")
                    tt = tt_tiles[("t", j, p)]
                    cts = tt_tiles[("s", j, h)]
                    for qq in range(4):
                        nc.tensor.transpose(
                            tt[64 * s:64 * s + 64, 128 * qq:128 * (qq + 1)],
                            cts[:, 64 * qq:64 * (qq + 1)], idn[:])
                return (512, f)

            def tail_c(j, p):
                def f(j=j, p=p):
                    ctx_pair = cxp.tile([128, 512], BF16, tag="ctx", name=f"ctx{j}_{p}")
                    ctx_tiles[(j, p)] = ctx_pair
                    nc.vector.tensor_copy(ctx_pair[:], tt_tiles[("t", j, p)][:])
                    if p == 3:
                        proj_q.extend(gen_proj(j))
                return (0, f)

            # ---------------- prologue: QKV for block 0 ----------------
            # Q runs kk-major across 4 interleaved accumulation chains (2
            # paux slots + 2 borrowed score slots) so each arriving x-chunk
            # DMA feeds 4 matmuls — the chain-major order would stall on the
            # serialized x0 chunk issues
            psq, qts = [], []
            for p in range(4):
                if p < 2:
                    ps = paux.tile([128, 512], F32, tag="aux", name=f"psq0_{p}")[:]
                else:
                    ps = pst.tile([128, 1024], F32, tag="st", name=f"psq0_{p}")[:, 0:512]
                psq.append(ps)
                qt = qtp.tile([128, 512], BF16, tag="qt", name=f"qt0_{p}")
                q_tiles[(0, p)] = qt
                qts.append(qt)
            for kk in range(8):
                for p in range(4):
                    nc.tensor.matmul(psq[p], wq_t[:, kk, 128 * p:128 * (p + 1)],
                                     x_t0[:, kk, :], start=(kk == 0), stop=(kk == 7))
            for p in range(4):
                nc.vector.tensor_copy(qts[p][:], psq[p])
            for r, f in gen_kv(0):
                f()

            # ---------------- main loop ----------------
            for j in range(NQB):
                qkv_q.clear()
                if j + 1 < NQB:
                    load_x(j + 1)
                    qkv_q.extend(gen_q(j + 1))
                    qkv_q.extend(gen_kv(j + 1))
                npair = 2 * (j + 1)
                state["qkv_total"] = len(qkv_q)
                state["qkv_done"] = 0
                state["beats_total"] = 8 * npair + (8 if j < 3 else 0)
                state["beat"] = 0
                state["allow_proj"] = (j >= 2)
                state["prefill_proj"] = (j == NQB - 1)
                state["proj_floor"] = 8
                state["proj_done"] = 0
                state["proj_total"] = len(proj_q) + len(late_q)
                for h in range(HL):
                    first = True
                    for ip in reversed(range(npair)):
                        state["beat"] += 1
                        if ip == 0 and (j, h) in la_e2:
                            # this pair's scores+exp were precomputed a
                            # block early — only the ctx matmuls remain
                            rows = 0
                            ctx_thunk = make_ctx(j, h, la_e2.pop((j, h)),
                                                 0, 1, -4 * j, 1 - 4 * j)
                        else:
                            rows, ctx_thunk = attn_pair_beat(j, h, ip)
                        if first:
                            # head boundary: clear old ctx thunks behind the
                            # freshly issued scores+exp, but leave the
                            # previous head's tail chain (A/B/C) to spread
                            # over the next beats — B stalls on A's DVE work
                            # if popped in the same beat
                            drain_pending(keep=5 if len(pending) <= 6 else 6)
                            first = False
                        run_beat_tail(rows)
                        pending.append(ctx_thunk)
                    if j < 3:
                        # lookahead beat: next block's far pair for this head
                        state["beat"] += 1
                        run_beat_tail(emit_la(j + 1, h))
                    pending.append(tail_a(j, h))
                    if h % 2 == 1:
                        # transpose + assemble as one atomic per-pair entry
                        # (the single tt PSUM slot must not interleave two
                        # pairs); blocks 0-1 defer theirs into blocks 2-3,
                        # where exp latency otherwise starves the PE
                        ra, fa = tail_b(j, h - 1)
                        rb, fb = tail_b(j, h)
                        rc_, fc = tail_c(j, h // 2)

                        def bc(fa=fa, fb=fb, fc=fc):
                            fa()
                            fb()
                            fc()
                        entry = (ra + rb + rc_, bc)
                        if j < 2:
                            late_q.append(entry)
                        else:
                            pending.append(entry)
                # block end: QKV for next block must be complete
                while qkv_q:
                    r, f = qkv_q.popleft()
                    f()

            # ---------------- epilogue ----------------
            state["allow_proj"] = True
            state["prefill_proj"] = True
            state["proj_floor"] = 0
            drain_pending(keep=0)
            while late_q:
                r, f = late_q.popleft()
                f()
            while proj_q:
                r, f = proj_q.popleft()
                f()
    return nc


def _split_waits(nc, limit=1):
    """This walrus build accepts only one sync wait per TPB_CTRL instruction;
    move excess waits onto preceding same-engine NOPs."""
    import concourse.mybir as mybir
    for f in nc.m.functions:
        for bb in f.blocks:
            new_insts = []
            for inst in bb.instructions:
                si = inst.sync_info
                if si is not None and si.on_wait and len(si.on_wait) > limit:
                    waits = list(si.on_wait)
                    k = 0
                    while len(waits) - k > limit:
                        chunk = waits[k:k + limit]
                        k += limit
                        nop = mybir.InstNoOp(name=f"{inst.name}_ws{k}")
                        nop.engine = inst.engine
                        nop.sync_info = mybir.SyncInfo(on_wait=chunk, on_update=[])
                        new_insts.append(nop)
                    si.on_wait = waits[k:]
                new_insts.append(inst)
            bb.instructions = new_insts


# --------------------------------------------------------------------------
# compile + SPMD execution via PJRT (axon) — jit once, reuse
# --------------------------------------------------------------------------
class _Compiled:
    def __init__(self, n_cores=8):
        import jax
        from jax.sharding import Mesh, PartitionSpec
        from jax.experimental.shard_map import shard_map
        import concourse.mybir as mybir
        from concourse.bass2jax import (_bass_exec_p, install_neuronx_cc_hook,
                                        partition_id_tensor)

        nc = _build_bass()
        _split_waits(nc)
        install_neuronx_cc_hook()
        partition_name = nc.partition_id_tensor.name if nc.partition_id_tensor else None
        in_names, out_names, out_avals, zero_outs = [], [], [], []
        for alloc in nc.m.functions[0].allocations:
            if not isinstance(alloc, mybir.MemoryLocationSet):
                continue
            name = alloc.memorylocations[0].name
            if alloc.kind == "ExternalInput":
                if name != partition_name:
                    in_names.append(name)
            elif alloc.kind == "ExternalOutput":
                shape = tuple(alloc.tensor_shape)
                dtype = mybir.dt.np(alloc.dtype)
                out_names.append(name)
                out_avals.append(jax.core.ShapedArray(shape, dtype))
                zero_outs.append(np.zeros(shape, dtype))
        n_params = len(in_names)
        all_in_names = list(in_names) + list(out_names)
        if partition_name is not None:
            all_in_names.append(partition_name)

        def _body(*args):
            operands = list(args)
            if partition_name is not None:
                operands.append(partition_id_tensor())
            outs = _bass_exec_p.bind(
                *operands,
                out_avals=tuple(out_avals),
                in_names=tuple(all_in_names),
                out_names=tuple(out_names),
                lowering_input_output_aliases=(),
                sim_require_finite=True,
                sim_require_nnan=True,
                nc=nc,
            )
            return tuple(outs)

        devices = jax.devices()[:n_cores]
        assert len(devices) >= n_cores, f"need {n_cores} cores, have {len(devices)}"
        self.n_cores = n_cores
        self.in_names, self.out_names = in_names, out_names
        self.out_avals, self.zero_outs = out_avals, zero_outs
        mesh = Mesh(np.asarray(devices[:n_cores]), ("core",))
        in_specs = (PartitionSpec("core"),) * (n_params + len(out_names))
        out_specs = (PartitionSpec("core"),) * len(out_names)
        self.fn = jax.jit(
            shard_map(_body, mesh=mesh, in_specs=in_specs,
                      out_specs=out_specs, check_rep=False),
            keep_unused=True)

    def run(self, in_maps):
        import jax
        args = []
        for name in self.in_names:
            args.append(np.concatenate([np.asarray(m[name]) for m in in_maps], axis=0))
        for z in self.zero_outs:
            args.append(np.zeros((self.n_cores * z.shape[0], *z.shape[1:]), z.dtype))
        if not getattr(self, "_warm", False):
            # the very first execution after device bring-up can read
            # uninitialized PSUM; do one discarded warm-up pass
            jax.block_until_ready(self.fn(*args))
            self._warm = True
        outs = self.fn(*args)
        jax.block_until_ready(outs)
        res = []
        for c in range(self.n_cores):
            d = {}
            for i, name in enumerate(self.out_names):
                a = np.asarray(outs[i]).reshape(self.n_cores, *self.out_avals[i].shape)[c]
                d[name] = a
            res.append(d)
        return res


# --------------------------------------------------------------------------
# host-side shard / unshard
# --------------------------------------------------------------------------
def _bf16(a):
    import ml_dtypes
    return np.ascontiguousarray(a).astype(ml_dtypes.bfloat16)


def _make_core_inputs(x, Wq, Wk, Wv, Wp, core):
    g = core % 2
    b = core // 2
    rows = slice(512 * g, 512 * (g + 1))
    kl = np.arange(128)
    return {
        "xt": _bf16(x[b].T),
        # fold the 1/sqrt(head_dim) score scale into Wq
        "wq": _bf16(Wq[rows, :].T * 0.125),
        "wk": _bf16(Wk[rows, :].T),
        "wv": _bf16(Wv[rows, :].T),
        "wp": _bf16(Wp[:, rows].T),
        "mask1": _bf16((kl[:, None] <= kl[None, :]).astype(np.float32)),
        "ident": _bf16(np.eye(128, dtype=np.float32)),
        "vones": _bf16(np.ones((128, NKT * HL), np.float32)),
    }


def kernel(x, Wq, Wk, Wv, Wp):
    """Full-input / full-output causal MHA. x: (4, 2048, 1024) fp32;
    Wq/Wk/Wv/Wp: (1024, 1024) fp32. Returns (4, 2048, 1024) fp32."""
    global _COMPILED
    x = np.asarray(x, dtype=np.float32)
    Wq = np.asarray(Wq, dtype=np.float32)
    Wk = np.asarray(Wk, dtype=np.float32)
    Wv = np.asarray(Wv, dtype=np.float32)
    Wp = np.asarray(Wp, dtype=np.float32)
    assert x.shape == (B, T, D), x.shape

    if _COMPILED is None:
        _COMPILED = _Compiled(8)
    in_maps = [_make_core_inputs(x, Wq, Wk, Wv, Wp, c) for c in range(8)]
    results = _COMPILED.run(in_maps)

    out = np.empty((B, T, D), np.float32)
    for b in range(B):
        acc = (results[2 * b]["outt"].astype(np.float32)
               + results[2 * b + 1]["outt"].astype(np.float32))
        out[b] = acc.T
    return out
